# revision 15
# baseline (speedup 1.0000x reference)
"""Trainium2 Bass kernel for nn_AMKPDModel (linear-attention transformer,
K=4 blocks x 2 cycles, ConvSwiGLU FFN, 32k-vocab LM head) on 8 NeuronCores.

Sharding: 2 data-parallel groups of 4 cores (one per batch element).
Within a group: attention heads sharded 2/core, FFN inner dim sharded
384/core, lm_head vocab sharded 8000/core. Two group-local AllReduces
per block eval ([1024,512] bf16), chunked for compute/comm overlap.
"""

import sys

if "/opt/trn_rl_repo" not in sys.path:
    sys.path.insert(0, "/opt/trn_rl_repo")

import numpy as np

# model dims
B, N, D = 2, 1024, 512
K = 4
V = 32000
INNER = 1536
CK = 3
EPS = 1e-5
CYCLES = 2  # H_CYCLES runs of the 4-block stack

# sharding
GRP = 4            # cores per batch group
P = 128
NT = N // P        # 8 token tiles
DT = D // P        # 4 feature tiles
JT = 6             # up-proj 768/128 local tiles (3 G + 3 U)
CT = 3             # local inner tiles (384/128)
VC = V // GRP      # 8000 vocab rows per core
VCH = 500
NVC = VC // VCH    # 16 lm chunks
NCHUNK = 2         # AR chunks per block output (token-split)
TC_PER_CHUNK = NT // NCHUNK

RG = [[0, 1, 2, 3], [4, 5, 6, 7]]

_CACHE = {}
DEBUG = False


def _build():
    import concourse.bass as bass
    import concourse.mybir as mybir
    import concourse.tile as tile
    from concourse import bacc
    from concourse.masks import make_identity

    f32 = mybir.dt.float32
    bf16 = mybir.dt.bfloat16
    i32 = mybir.dt.int32
    AF = mybir.ActivationFunctionType
    OP = mybir.AluOpType

    nc = bacc.Bacc(None, target_bir_lowering=False, debug=False, num_devices=8)

    # ---------------- DRAM params ----------------
    ids_d = nc.declare_dram_parameter("ids", [N], i32, isOutput=False)
    q0_d = nc.declare_dram_parameter("q0", [N, D], f32, isOutput=False)
    emb_d = nc.declare_dram_parameter("emb", [V, D], f32, isOutput=False)
    pos_d = nc.declare_dram_parameter("posn", [N, D], f32, isOutput=False)
    ins_d = nc.declare_dram_parameter("ins_v", [D], f32, isOutput=False)
    inb_d = nc.declare_dram_parameter("inb_v", [D], f32, isOutput=False)
    fins_d = nc.declare_dram_parameter("fins_v", [D], f32, isOutput=False)
    finb_d = nc.declare_dram_parameter("finb_v", [D], f32, isOutput=False)
    dtv_d = nc.declare_dram_parameter("dtv", [1, K], f32, isOutput=False)
    wq_d = nc.declare_dram_parameter("wq", [K, D, P], f32, isOutput=False)
    wk_d = nc.declare_dram_parameter("wk", [K, D, P], f32, isOutput=False)
    wv_d = nc.declare_dram_parameter("wv", [K, D, P], f32, isOutput=False)
    wo_d = nc.declare_dram_parameter("wo", [K, P, D], f32, isOutput=False)
    wup_d = nc.declare_dram_parameter("wup", [K, D, JT * P], f32, isOutput=False)
    dww_d = nc.declare_dram_parameter("dww", [K, CT * P, CK], f32, isOutput=False)
    dwb_d = nc.declare_dram_parameter("dwb", [K, CT * P], f32, isOutput=False)
    wdn_d = nc.declare_dram_parameter("wdn", [K, CT * P, D], f32, isOutput=False)
    n1s_d = nc.declare_dram_parameter("n1s", [K, D], f32, isOutput=False)
    n1b_d = nc.declare_dram_parameter("n1b", [K, D], f32, isOutput=False)
    n2s_d = nc.declare_dram_parameter("n2s", [K, D], f32, isOutput=False)
    n2b_d = nc.declare_dram_parameter("n2b", [K, D], f32, isOutput=False)
    lmt_d = nc.declare_dram_parameter("lmt", [D, VC], f32, isOutput=False)
    hwt_d = nc.declare_dram_parameter("hwt", [D, 2], f32, isOutput=False)
    hb_d = nc.declare_dram_parameter("hb", [1, 2], f32, isOutput=False)

    logits_d = nc.declare_dram_parameter("logits", [N, VC], f32, isOutput=True)
    qlog_d = nc.declare_dram_parameter("qlog", [1, 2], f32, isOutput=True)
    dbg = {}
    if DEBUG:
        bf16_ = mybir.dt.bfloat16
        for nm, shp, dt_ in (
            ("dbg_xb", [P, NT, D], bf16_),
            ("dbg_z", [P, NT, D], bf16_),
            ("dbg_zt", [P, DT, N], bf16_),
            ("dbg_phiq", [P, N], bf16_),
            ("dbg_phik", [P, N], bf16_),
            ("dbg_vpt", [P, N], bf16_),
            ("dbg_wt", [P, NT, N], bf16_),
            ("dbg_mcat", [P, N], bf16_),
            ("dbg_y", [P, NT, D], bf16_),
            ("dbg_q1", [P, NT, D], f32),
            ("dbg_hf", [P, CT, N + 2], bf16_),
            ("dbg_hcv", [P, CT, N], bf16_),
            ("dbg_y2", [P, NT, D], bf16_),
            ("dbg_q2", [P, NT, D], f32),
        ):
            dbg[nm] = nc.declare_dram_parameter(nm, shp, dt_, isOutput=True)

    with tile.TileContext(nc) as tc:
        pers = tc.alloc_tile_pool(name="pers", bufs=1)
        ps_big = tc.alloc_tile_pool(name="psb", bufs=2, space="PSUM")
        ps_sml = tc.alloc_tile_pool(name="pss", bufs=4, space="PSUM")
        dram = tc.alloc_tile_pool(name="dram", bufs=2, space="DRAM")

        # ---------------- persistent tiles ----------------
        eps_t = pers.tile([P, 1], f32, tag="eps")
        nc.vector.memset(eps_t[:], EPS)
        ones1 = pers.tile([1, P], bf16, tag="ones1")
        nc.vector.memset(ones1[:], 1.0)
        ident = pers.tile([P, P], bf16, tag="ident")
        make_identity(nc, ident[:])

        ids_sb = pers.tile([P, NT], i32, tag="ids")
        nc.sync.dma_start(ids_sb[:], ids_d[:].rearrange("(t p) -> p t", p=P))
        q_res = pers.tile([P, NT, D], f32, tag="qres")
        nc.sync.dma_start(q_res[:], q0_d[:].rearrange("(t p) d -> p t d", p=P))

        def bcast_row(src_d, tag):
            t = pers.tile([P, D], f32, tag=tag)
            nc.sync.dma_start(t[:], src_d[:][None, :].to_broadcast([P, D]))
            return t

        fins_bc = bcast_row(fins_d, "finsbc")
        finb_bc = bcast_row(finb_d, "finbbc")

        # softplus(dt) broadcast to all partitions
        dtv_sb = pers.tile([1, K], f32, tag="dtv")
        nc.sync.dma_start(dtv_sb[:], dtv_d[:])
        spe = pers.tile([1, K], f32, tag="spe")
        nc.scalar.activation(spe[:], dtv_sb[:], AF.Exp)
        nc.vector.tensor_scalar_add(spe[:], spe[:], 1.0)
        nc.scalar.activation(spe[:], spe[:], AF.Ln)
        sp_bc = pers.tile([P, K], f32, tag="spbc")
        nc.gpsimd.partition_broadcast(sp_bc[:], spe[:])

        # per-block norm scale vectors (transposed layouts)
        n1s_sb = pers.tile([P, K, DT], f32, tag="n1s")
        nc.sync.dma_start(n1s_sb[:], n1s_d[:].rearrange("k (dt p) -> p k dt", p=P))
        n2s_sb = pers.tile([P, K, DT], f32, tag="n2s")
        nc.sync.dma_start(n2s_sb[:], n2s_d[:].rearrange("k (dt p) -> p k dt", p=P))

        # weights (loaded f32, cast bf16)
        wq_sb = pers.tile([P, K, DT, P], bf16, tag="wq")
        wk_sb = pers.tile([P, K, DT, P], bf16, tag="wk")
        wv_sb = pers.tile([P, K, DT, P], bf16, tag="wv")
        wo_sb = pers.tile([P, K, D], bf16, tag="wo")
        wup_sb = pers.tile([P, K, DT, JT, P], bf16, tag="wup")
        wdn_sb = pers.tile([P, K, CT, D], bf16, tag="wdn")
        dww_sb = pers.tile([P, K, CT, CK], f32, tag="dww")
        nc.sync.dma_start(dww_sb[:], dww_d[:].rearrange("k (ct p) c -> p k ct c", p=P))
        dwb_sb = pers.tile([P, K, CT], f32, tag="dwb")
        nc.sync.dma_start(dwb_sb[:], dwb_d[:].rearrange("k (ct p) -> p k ct", p=P))
        diag_sb = pers.tile([P, K, CT, CK, P], bf16, tag="diag")
        b1t_sb = pers.tile([P, DT, K], bf16, tag="b1t")
        b2t_sb = pers.tile([P, DT, K], bf16, tag="b2t")
        b2w_sb = pers.tile([P, K, JT], f32, tag="b2w")
        hwt_sb = pers.tile([P, DT, 2], f32, tag="hwt")
        nc.sync.dma_start(hwt_sb[:], hwt_d[:].rearrange("(dt p) c -> p dt c", p=P))
        hb_sb = pers.tile([1, 2], f32, tag="hb")
        nc.sync.dma_start(hb_sb[:], hb_d[:])

        xw_sb = pers.tile([P, K, 3, N], bf16, tag="xw")
        vp_aug = pers.tile([P, NT, 2, 64], bf16, tag="vpaug")
        onescol = pers.tile([P, 1], bf16, tag="onescol")
        nc.vector.memset(onescol[:], 1.0)
        wt_sb = pers.tile([P, NT, N], bf16, tag="wt")  # one head at a time
        hf_sb = pers.tile([P, CT, N + 2], bf16, tag="hf")
        nc.vector.memset(hf_sb[:], 0.0)
        zft_sb = pers.tile([P, DT, N], bf16, tag="zft")

        # z / zt shared across LN sites (persistent; evals are serial anyway)
        z_sb = pers.tile([P, NT, D], bf16, tag="z")
        zt_sb = pers.tile([P, DT, N], bf16, tag="zt")
        mcat_sb = pers.tile([P, N], bf16, tag="mcat")

        # ---------------- setup (scoped transients) ----------------
        with tc.tile_pool(name="setup", bufs=1) as setup, tc.tile_pool(
            name="setup2", bufs=2
        ) as setup2:
            # cast helper: DMA f32 -> stage, cast to dst (bf16)
            def load_cast(dst_ap, src_ap, shape, tag, eng):
                s = setup.tile(shape, f32, tag="wstage")
                nc.sync.dma_start(s[:], src_ap)
                if eng == 0:
                    nc.vector.tensor_copy(dst_ap, s[:])
                else:
                    nc.scalar.copy(dst_ap, s[:])

            for i, (dst, src) in enumerate(
                ((wq_sb, wq_d), (wk_sb, wk_d), (wv_sb, wv_d))
            ):
                load_cast(
                    dst[:],
                    src[:].rearrange("k (dt p) j -> p k dt j", p=P),
                    [P, K, DT, P],
                    "wstage",
                    i % 2,
                )
            load_cast(
                wo_sb[:],
                wo_d[:].rearrange("k p d -> p k d"),
                [P, K, D],
                "wstage",
                1,
            )
            for k in range(K):
                load_cast(
                    wup_sb[:, k],
                    wup_d[k].rearrange("(dt p) (jt jj) -> p dt jt jj", p=P, jj=P),
                    [P, DT, JT, P],
                    "wstage",
                    k % 2,
                )
                load_cast(
                    wdn_sb[:, k],
                    wdn_d[k].rearrange("(ct p) d -> p ct d", p=P),
                    [P, CT, D],
                    "wstage",
                    (k + 1) % 2,
                )
            for bd, bt in ((n1b_d, b1t_sb), (n2b_d, b2t_sb)):
                bcol = setup2.tile([P, DT, K], f32, tag="bcol")
                for k in range(K):
                    for dtt in range(DT):
                        nc.sync.dma_start(
                            bcol[:, dtt, k : k + 1],
                            bd[k, dtt * P : (dtt + 1) * P][:, None],
                        )
                nc.vector.tensor_copy(bt[:], bcol[:])

            # depthwise conv as matmul: diagonal weight mats per (k, ct, tap)
            for k in range(K):
                for ct in range(CT):
                    for tap in range(CK):
                        nc.vector.tensor_scalar_mul(
                            diag_sb[:, k, ct, tap, :],
                            ident[:],
                            dww_sb[:, k, ct, tap : tap + 1],
                        )

            # ---- embedding + input LN -> X (bf16) -> XT
            ins_bc = setup.tile([P, D], f32, tag="insbc")
            nc.sync.dma_start(ins_bc[:], ins_d[:][None, :].to_broadcast([P, D]))
            inb_bc = setup.tile([P, D], f32, tag="inbbc")
            nc.sync.dma_start(inb_bc[:], inb_d[:][None, :].to_broadcast([P, D]))

            xb_sb = setup.tile([P, NT, D], bf16, tag="xb0")
            st = setup.tile([P, NT, 6], f32, tag="st0")
            mv = setup.tile([P, NT, 2], f32, tag="mv0")
            rstd = setup.tile([P, NT], f32, tag="rstd0")
            for t in range(NT):
                xrt = setup2.tile([P, D], f32, tag="xrt")
                nc.gpsimd.indirect_dma_start(
                    out=xrt[:],
                    out_offset=None,
                    in_=emb_d[:],
                    in_offset=bass.IndirectOffsetOnAxis(ap=ids_sb[:, t : t + 1], axis=0),
                )
                ps = setup2.tile([P, D], f32, tag="posst")
                nc.sync.dma_start(
                    ps[:], pos_d[:].rearrange("(t p) d -> p t d", p=P)[:, t, :]
                )
                nc.vector.tensor_tensor(xrt[:], xrt[:], ps[:], op=OP.add)
                nc.vector.bn_stats(st[:, t], xrt[:])
                nc.vector.bn_aggr(mv[:, t], st[:, t])
                nc.scalar.activation(
                    rstd[:, t : t + 1], mv[:, t, 1:2], AF.Ln, bias=eps_t[:]
                )
                nc.scalar.activation(
                    rstd[:, t : t + 1], rstd[:, t : t + 1], AF.Exp, scale=-0.5
                )
                nc.vector.tensor_scalar(
                    xb_sb[:, t],
                    xrt[:],
                    mv[:, t, 0:1],
                    rstd[:, t : t + 1],
                    op0=OP.subtract,
                    op1=OP.mult,
                )
                nc.vector.tensor_tensor(xb_sb[:, t], xb_sb[:, t], ins_bc[:], op=OP.mult)
                nc.vector.tensor_tensor(xb_sb[:, t], xb_sb[:, t], inb_bc[:], op=OP.add)
            if DEBUG:
                nc.sync.dma_start(dbg["dbg_xb"][:], xb_sb[:])
            xt_sb = setup.tile([P, DT, N], bf16, tag="xt")
            for t in range(NT):
                for dtt in range(DT):
                    nc.sync.dma_start_transpose(
                        xt_sb[:, dtt, t * P : (t + 1) * P],
                        xb_sb[:, t, dtt * P : (dtt + 1) * P],
                    )

            # ---- XW = (b1 + X) @ W  (raw weights, before s1 fold)
            for k in range(K):
                for pi, wsb in enumerate((wq_sb, wk_sb, wv_sb)):
                    pb = ps_sml.tile([P, 512], f32, tag="sml")
                    for dtt in range(DT):
                        nc.tensor.matmul(
                            pb[:, 0:1],
                            lhsT=wsb[:, k, dtt, :],
                            rhs=b1t_sb[:, dtt, k : k + 1],
                            start=(dtt == 0),
                            stop=(dtt == DT - 1),
                        )
                    b1w = setup2.tile([P, 1], f32, tag="b1w")
                    nc.vector.tensor_copy(b1w[:], pb[:, 0:1])
                    for nb in range(2):
                        pp = ps_sml.tile([P, 512], f32, tag="sml")
                        for dtt in range(DT):
                            nc.tensor.matmul(
                                pp[:],
                                lhsT=wsb[:, k, dtt, :],
                                rhs=xt_sb[:, dtt, nb * 512 : (nb + 1) * 512],
                                start=(dtt == 0),
                                stop=(dtt == DT - 1),
                            )
                        nc.vector.tensor_scalar(
                            xw_sb[:, k, pi, nb * 512 : (nb + 1) * 512],
                            pp[:],
                            b1w[:],
                            None,
                            op0=OP.add,
                        )

            # ---- b2W per (k, jt) (raw wup)
            for k in range(K):
                for jt in range(JT):
                    pb = ps_sml.tile([P, 512], f32, tag="sml")
                    for dtt in range(DT):
                        nc.tensor.matmul(
                            pb[:, 0:1],
                            lhsT=wup_sb[:, k, dtt, jt, :],
                            rhs=b2t_sb[:, dtt, k : k + 1],
                            start=(dtt == 0),
                            stop=(dtt == DT - 1),
                        )
                    eng = nc.vector if (k + jt) % 2 == 0 else nc.scalar
                    if eng is nc.vector:
                        nc.vector.tensor_copy(b2w_sb[:, k, jt : jt + 1], pb[:, 0:1])
                    else:
                        nc.scalar.copy(b2w_sb[:, k, jt : jt + 1], pb[:, 0:1])

            # ---- fold s1 into wq/wk/wv, s2 into wup, softplus(dt) into wo
            for wsb in (wq_sb, wk_sb, wv_sb):
                for k in range(K):
                    nc.vector.tensor_tensor(
                        wsb[:, k],
                        wsb[:, k],
                        n1s_sb[:, k, :, None].to_broadcast([P, DT, P]),
                        op=OP.mult,
                    )
            for k in range(K):
                nc.vector.tensor_tensor(
                    wup_sb[:, k],
                    wup_sb[:, k],
                    n2s_sb[:, k, :, None, None].to_broadcast([P, DT, JT, P]),
                    op=OP.mult,
                )
                nc.vector.tensor_tensor(
                    wo_sb[:, k],
                    wo_sb[:, k],
                    sp_bc[:, k : k + 1].to_broadcast([P, D]),
                    op=OP.mult,
                )

        # ---------------- block evals ----------------
        with tc.tile_pool(name="work", bufs=1) as work, tc.tile_pool(
            name="work2", bufs=2
        ) as work2:

            def layernorm(src, t_lo, t_hi, out, scale_bc=None, bias_bc=None):
                """LN over d for token tiles [t_lo, t_hi) of src [P,NT,D] f32.
                Writes (x-mu)*rstd (optionally *scale+bias) to out (bf16)."""
                nt = t_hi - t_lo
                st = work.tile([P, NT, 6], f32, tag="st")
                mv = work.tile([P, NT, 2], f32, tag="mv")
                for t in range(t_lo, t_hi):
                    nc.vector.bn_stats(st[:, t], src[:, t])
                    nc.vector.bn_aggr(mv[:, t], st[:, t])
                rstd = work.tile([P, NT], f32, tag="rstd")
                nc.scalar.activation(
                    rstd[:, t_lo:t_hi], mv[:, t_lo:t_hi, 1], AF.Ln, bias=eps_t[:]
                )
                nc.scalar.activation(
                    rstd[:, t_lo:t_hi], rstd[:, t_lo:t_hi], AF.Exp, scale=-0.5
                )
                nmr = work.tile([P, NT], f32, tag="nmr")
                nc.vector.tensor_tensor(
                    nmr[:, t_lo:t_hi], mv[:, t_lo:t_hi, 0], rstd[:, t_lo:t_hi], op=OP.mult
                )
                nc.vector.tensor_scalar_mul(nmr[:, t_lo:t_hi], nmr[:, t_lo:t_hi], -1.0)
                for t in range(t_lo, t_hi):
                    if t % 2 == 0:
                        nc.vector.tensor_scalar(
                            out[:, t],
                            src[:, t],
                            mv[:, t, 0:1],
                            rstd[:, t : t + 1],
                            op0=OP.subtract,
                            op1=OP.mult,
                        )
                    else:
                        nc.scalar.activation(
                            out[:, t],
                            src[:, t],
                            AF.Identity,
                            bias=nmr[:, t : t + 1],
                            scale=rstd[:, t : t + 1],
                        )

            def transpose_tiles(src, dst, t_lo, t_hi):
                for t in range(t_lo, t_hi):
                    for dtt in range(DT):
                        nc.sync.dma_start_transpose(
                            dst[:, dtt, t * P : (t + 1) * P],
                            src[:, t, dtt * P : (dtt + 1) * P],
                        )

            def all_reduce_chunked(y, scaled_add_dst):
                """AllReduce y [P,NT,D] bf16 in NCHUNK token chunks; add into
                scaled_add_dst (q_res) in place."""
                for c in range(NCHUNK):
                    t0, t1 = c * TC_PER_CHUNK, (c + 1) * TC_PER_CHUNK
                    yb = dram.tile([P, TC_PER_CHUNK, D], bf16, tag="arin")
                    ab = dram.tile([P, TC_PER_CHUNK, D], bf16, tag="arout")
                    nc.sync.dma_start(yb[:], y[:, t0:t1])
                    nc.gpsimd.collective_compute(
                        "AllReduce",
                        OP.add,
                        replica_groups=RG,
                        ins=[yb[:].opt()],
                        outs=[ab[:].opt()],
                    )
                    ar = work2.tile([P, TC_PER_CHUNK, D], bf16, tag="ar")
                    nc.sync.dma_start(ar[:], ab[:])
                    for i, t in enumerate(range(t0, t1)):
                        eng = nc.vector if t % 2 == 0 else nc.gpsimd
                        eng.tensor_tensor(
                            scaled_add_dst[:, t],
                            scaled_add_dst[:, t],
                            ar[:, i],
                            op=OP.add,
                        )

            def block_eval(k, dump=False):
                # ---- LN1 -> z -> zT
                layernorm(q_res, 0, NT, z_sb)
                transpose_tiles(z_sb, zt_sb, 0, NT)
                if dump:
                    nc.sync.dma_start(dbg["dbg_z"][:], z_sb[:])
                    nc.sync.dma_start(dbg["dbg_zt"][:], zt_sb[:])

                # ---- projections (q, k, v)
                phi = [None, None, None]
                for pi, wsb in enumerate((wq_sb, wk_sb, wv_sb)):
                    pp = ps_big.tile([P, 1024], f32, tag="big")
                    for nb in range(2):
                        for dtt in range(DT):
                            nc.tensor.matmul(
                                pp[:, nb * 512 : (nb + 1) * 512],
                                lhsT=wsb[:, k, dtt, :],
                                rhs=zt_sb[:, dtt, nb * 512 : (nb + 1) * 512],
                                start=(dtt == 0),
                                stop=(dtt == DT - 1),
                            )
                    tag = ("phiq", "phik", "vpt")[pi]
                    res = work.tile([P, N], bf16, tag=tag)
                    if pi < 2:
                        ts_t = work2.tile([P, N], bf16, tag="tsum")
                        nc.vector.tensor_tensor(
                            ts_t[:], pp[:], xw_sb[:, k, pi, :], op=OP.add
                        )
                        rel = work.tile([P, N], bf16, tag="rel")
                        nc.vector.tensor_scalar_max(rel[:], ts_t[:], 0.0)
                        mn = work.tile([P, N], bf16, tag="mn")
                        nc.vector.tensor_scalar_min(mn[:], ts_t[:], 0.0)
                        ex = work.tile([P, N], bf16, tag="ex")
                        nc.scalar.activation(ex[:], mn[:], AF.Exp)
                        nc.vector.tensor_tensor(res[:], ex[:], rel[:], op=OP.add)
                    else:
                        nc.vector.tensor_tensor(
                            res[:], pp[:], xw_sb[:, k, pi, :], op=OP.add
                        )
                        for mt in range(NT):
                            for h in range(2):
                                nc.sync.dma_start_transpose(
                                    vp_aug[:, mt, h, 0:64],
                                    res[h * 64 : (h + 1) * 64, mt * P : (mt + 1) * P],
                                )
                    phi[pi] = res
                phiq, phik, vpt = phi
                if dump:
                    nc.sync.dma_start(dbg["dbg_phiq"][:], phiq[:])
                    nc.sync.dma_start(dbg["dbg_phik"][:], phik[:])
                    nc.sync.dma_start(dbg["dbg_vpt"][:], vpt[:])

                # ---- attention, one head at a time
                for h in range(2):
                    hs = h * 64
                    # QK^T (transposed W) + square
                    for mt in range(NT):
                        for nb in range(2):
                            pw = ps_sml.tile([P, 512], f32, tag="sml")
                            nc.tensor.matmul(
                                pw[:],
                                lhsT=phik[hs : hs + 64, mt * P : (mt + 1) * P],
                                rhs=phiq[hs : hs + 64, nb * 512 : (nb + 1) * 512],
                                start=True,
                                stop=True,
                            )
                            dst = wt_sb[:, mt, nb * 512 : (nb + 1) * 512]
                            if (mt + nb) % 2 == 0:
                                wc = work2.tile([P, 512], bf16, tag="wc")
                                nc.vector.tensor_copy(wc[:], pw[:])
                                nc.gpsimd.tensor_tensor(dst, wc[:], wc[:], op=OP.mult)
                            else:
                                nc.scalar.activation(dst, pw[:], AF.Square)
                    if dump and h == 0:
                        nc.sync.dma_start(dbg["dbg_wt"][:], wt_sb[:])
                    # AV -> AttrT at partitions [hs, hs+64); S via ones matmul
                    for nb in range(2):
                        nsl = slice(nb * 512, (nb + 1) * 512)
                        pa = ps_sml.tile([P, 512], f32, tag="sml")
                        pss = ps_sml.tile([P, 512], f32, tag="sml")
                        for mt in range(NT):
                            nc.tensor.matmul(
                                pa[hs : hs + 64, :],
                                lhsT=vp_aug[:, mt, h, :],
                                rhs=wt_sb[:, mt, nsl],
                                start=(mt == 0),
                                stop=(mt == NT - 1),
                                tile_position=(0, hs),
                            )
                            nc.tensor.matmul(
                                pss[0:1, :],
                                lhsT=onescol[:],
                                rhs=wt_sb[:, mt, nsl],
                                start=(mt == 0),
                                stop=(mt == NT - 1),
                            )
                        rr = work.tile([1, 512], f32, tag="rr")
                        nc.vector.tensor_scalar_add(rr[:], pss[0:1, :], 1.0)
                        nc.vector.reciprocal_approx_fast(rr[:], rr[:])
                        rrb = work.tile([1, 512], bf16, tag="rrb")
                        nc.vector.tensor_copy(rrb[:], rr[:])
                        # replicate R across partitions via DRAM stride-0 read
                        rsc = dram.tile([1, 512], bf16, tag="rsc")
                        nc.sync.dma_start(rsc[:], rrb[:])
                        prr = work.tile([P, 512], bf16, tag="prr")
                        nc.sync.dma_start(
                            prr[hs : hs + 64, :],
                            rsc[0][None, :].to_broadcast([64, 512]),
                        )
                        at = work.tile([P, 512], bf16, tag="atr")
                        if nb == 0:
                            nc.vector.tensor_copy(at[hs : hs + 64, :], pa[hs : hs + 64, :])
                        else:
                            nc.scalar.copy(at[hs : hs + 64, :], pa[hs : hs + 64, :])
                        tm = work.tile([P, 512], bf16, tag="tm")
                        nc.vector.tensor_tensor(
                            tm[hs : hs + 64, :], at[hs : hs + 64, :], prr[hs : hs + 64, :], op=OP.mult
                        )
                        nc.gpsimd.tensor_tensor(
                            mcat_sb[hs : hs + 64, nsl],
                            tm[hs : hs + 64, :],
                            vpt[hs : hs + 64, nsl],
                            op=OP.subtract,
                        )

                # ---- out-proj (wo pre-scaled by softplus(dt)) + chunked AR
                y = work.tile([P, NT, D], bf16, tag="y")
                for nt in range(NT):
                    po = ps_sml.tile([P, 512], f32, tag="sml")
                    nc.tensor.matmul(
                        po[:],
                        lhsT=mcat_sb[:, nt * P : (nt + 1) * P],
                        rhs=wo_sb[:, k, :],
                        start=True,
                        stop=True,
                    )
                    if nt % 2 == 0:
                        nc.vector.tensor_copy(y[:, nt], po[:])
                    else:
                        nc.scalar.copy(y[:, nt], po[:])
                if dump:
                    nc.sync.dma_start(dbg["dbg_mcat"][:], mcat_sb[:])
                    nc.sync.dma_start(dbg["dbg_y"][:], y[:])
                all_reduce_chunked(y, q_res)
                if dump:
                    nc.sync.dma_start(dbg["dbg_q1"][:], q_res[:])

                # ---- LN2 -> z2 -> z2T
                layernorm(q_res, 0, NT, z_sb)
                transpose_tiles(z_sb, zt_sb, 0, NT)

                # ---- up-proj + SwiGLU -> hf
                for nb in range(2):
                    for jp in range(CT):
                        pg = ps_sml.tile([P, 512], f32, tag="sml")
                        for dtt in range(DT):
                            nc.tensor.matmul(
                                pg[:],
                                lhsT=wup_sb[:, k, dtt, jp, :],
                                rhs=zt_sb[:, dtt, nb * 512 : (nb + 1) * 512],
                                start=(dtt == 0),
                                stop=(dtt == DT - 1),
                            )
                        pu = ps_sml.tile([P, 512], f32, tag="sml")
                        for dtt in range(DT):
                            nc.tensor.matmul(
                                pu[:],
                                lhsT=wup_sb[:, k, dtt, jp + CT, :],
                                rhs=zt_sb[:, dtt, nb * 512 : (nb + 1) * 512],
                                start=(dtt == 0),
                                stop=(dtt == DT - 1),
                            )
                        sg = work2.tile([P, 512], bf16, tag="sg")
                        nc.scalar.activation(
                            sg[:], pg[:], AF.Silu, bias=b2w_sb[:, k, jp : jp + 1]
                        )
                        uu = work2.tile([P, 512], bf16, tag="uu")
                        nc.vector.tensor_scalar(
                            uu[:], pu[:], b2w_sb[:, k, jp + CT : jp + CT + 1], None, op0=OP.add
                        )
                        nc.gpsimd.tensor_tensor(
                            hf_sb[:, jp, 1 + nb * 512 : 1 + (nb + 1) * 512],
                            sg[:],
                            uu[:],
                            op=OP.mult,
                        )

                # ---- depthwise conv (as 3 diag matmuls) + silu -> hcv
                hcv = work.tile([P, CT, N], bf16, tag="hcv")
                for ct in range(CT):
                    for nb in range(2):
                        pc = ps_sml.tile([P, 512], f32, tag="sml")
                        for tap in range(CK):
                            nc.tensor.matmul(
                                pc[:],
                                lhsT=diag_sb[:, k, ct, tap, :],
                                rhs=hf_sb[:, ct, nb * 512 + tap : nb * 512 + tap + 512],
                                start=(tap == 0),
                                stop=(tap == CK - 1),
                            )
                        nc.scalar.activation(
                            hcv[:, ct, nb * 512 : (nb + 1) * 512],
                            pc[:],
                            AF.Silu,
                            bias=dwb_sb[:, k, ct : ct + 1],
                        )

                if dump:
                    nc.sync.dma_start(dbg["dbg_hf"][:], hf_sb[:])
                    nc.sync.dma_start(dbg["dbg_hcv"][:], hcv[:])
                # ---- down-proj + chunked AR
                y2 = work.tile([P, NT, D], bf16, tag="y")
                for nt in range(NT):
                    pd = ps_sml.tile([P, 512], f32, tag="sml")
                    for ct in range(CT):
                        nc.tensor.matmul(
                            pd[:],
                            lhsT=hcv[:, ct, nt * P : (nt + 1) * P],
                            rhs=wdn_sb[:, k, ct, :],
                            start=(ct == 0),
                            stop=(ct == CT - 1),
                        )
                    if nt % 2 == 0:
                        nc.vector.tensor_copy(y2[:, nt], pd[:])
                    else:
                        nc.scalar.copy(y2[:, nt], pd[:])
                if dump:
                    nc.sync.dma_start(dbg["dbg_y2"][:], y2[:])
                all_reduce_chunked(y2, q_res)
                if dump:
                    nc.sync.dma_start(dbg["dbg_q2"][:], q_res[:])

            for _cyc in range(CYCLES):
                for k in range(K):
                    block_eval(k, dump=(DEBUG and _cyc == 0 and k == 0))

            # ---------------- final LN (with fin scale/bias) ----------------
            layernorm(q_res, 0, NT, z_sb)
            nc.vector.tensor_tensor(
                z_sb[:], z_sb[:], fins_bc[:, None, :].to_broadcast([P, NT, D]), op=OP.mult
            )
            nc.vector.tensor_tensor(
                z_sb[:], z_sb[:], finb_bc[:, None, :].to_broadcast([P, NT, D]), op=OP.add
            )
            transpose_tiles(z_sb, zft_sb, 0, NT)

            # ---- q_logits = mean_n(Qn) @ halt_w.T + halt_b
            qm = work.tile([P, DT], f32, tag="qm")
            nc.vector.reduce_sum(qm[:], zft_sb[:], axis=mybir.AxisListType.X)
            pq = ps_sml.tile([P, 512], f32, tag="sml")
            for dtt in range(DT):
                nc.tensor.matmul(
                    pq[0:1, 0:2],
                    lhsT=qm[:, dtt : dtt + 1],
                    rhs=hwt_sb[:, dtt, :],
                    start=(dtt == 0),
                    stop=(dtt == DT - 1),
                )
            ql = work.tile([1, 2], f32, tag="ql")
            nc.vector.tensor_scalar_mul(ql[:], pq[0:1, 0:2], 1.0 / N)
            nc.vector.tensor_tensor(ql[:], ql[:], hb_sb[:], op=OP.add)
            nc.sync.dma_start(qlog_d[:], ql[:])

        # ---------------- lm head (vocab-sharded) ----------------
        with tc.tile_pool(name="lmp", bufs=2) as lmp:
            lg = logits_d[:].rearrange("(nt p) v -> p nt v", p=P)
            lmsrc = lmt_d[:].rearrange("(dt p) v -> p dt v", p=P)
            for vc in range(NVC):
                stage = lmp.tile([P, DT, VCH], f32, tag="lstage")
                nc.sync.dma_start(stage[:], lmsrc[:, :, vc * VCH : (vc + 1) * VCH])
                lc = lmp.tile([P, DT, VCH], bf16, tag="lc")
                if vc % 2 == 0:
                    nc.vector.tensor_copy(lc[:], stage[:])
                else:
                    nc.scalar.copy(lc[:], stage[:])
                for nt in range(NT):
                    pl = ps_sml.tile([P, 512], f32, tag="sml")
                    for dtt in range(DT):
                        nc.tensor.matmul(
                            pl[:, 0:VCH],
                            lhsT=zft_sb[:, dtt, nt * P : (nt + 1) * P],
                            rhs=lc[:, dtt, :],
                            start=(dtt == 0),
                            stop=(dtt == DT - 1),
                        )
                    ob = lmp.tile([P, VCH], f32, tag="ob")
                    if (vc + nt) % 2 == 0:
                        nc.vector.tensor_copy(ob[:], pl[:, 0:VCH])
                    else:
                        nc.scalar.copy(ob[:], pl[:, 0:VCH])
                    nc.sync.dma_start(lg[:, nt, vc * VCH : (vc + 1) * VCH], ob[:])

        dram.release()
        ps_sml.release()
        ps_big.release()
        pers.release()

    nc.compile()
    return nc


def _get_nc():
    if "nc" not in _CACHE:
        _CACHE["nc"] = _build()
    return _CACHE["nc"]


def _prep_in_maps(inputs):
    ii = {k: np.asarray(v) for k, v in inputs.items()}
    hm = ii["carry_halted"].astype(bool)
    ids = np.where(hm[:, None], ii["inputs"], ii["carry_inputs"]).astype(np.int32)
    init_h = ii["init_hidden"].astype(np.float32)
    q0 = np.where(
        hm[:, None, None],
        np.broadcast_to(init_h[None, None, :], (B, N, D)),
        ii["carry_hidden"].astype(np.float32),
    ).astype(np.float32)
    emb = np.ascontiguousarray(ii["emb"].astype(np.float32))
    posn = np.ascontiguousarray(ii["pos"].astype(np.float32)[:N])
    lmT = np.ascontiguousarray(ii["lm_w"].astype(np.float32).T)  # [D, V]
    hwT = np.ascontiguousarray(ii["halt_w"].astype(np.float32).T)  # [D, 2]
    hb = ii["halt_b"].astype(np.float32).reshape(1, 2)
    dtv = ii["dt"].astype(np.float32).reshape(1, K)
    wq = ii["W_Q"].astype(np.float32)
    wk = ii["W_K"].astype(np.float32)
    wv = ii["W_V"].astype(np.float32)
    wo = ii["W_O"].astype(np.float32)
    wup = ii["W_up"].astype(np.float32)
    dww = ii["dw_w"].astype(np.float32)[:, :, 0, :]  # [K, INNER, CK]
    dwb = ii["dw_b"].astype(np.float32)
    wdn = ii["W_down"].astype(np.float32)

    in_maps = []
    for c in range(8):
        b, g = c // GRP, c % GRP
        jlo = g * P  # head-col slice (2 heads x 64)
        clo = g * CT * P  # inner slice (384)
        m = {
            "ids": np.ascontiguousarray(ids[b]),
            "q0": np.ascontiguousarray(q0[b]),
            "emb": emb,
            "posn": posn,
            "ins_v": ii["in_s"].astype(np.float32),
            "inb_v": ii["in_b"].astype(np.float32),
            "fins_v": ii["fin_s"].astype(np.float32),
            "finb_v": ii["fin_b"].astype(np.float32),
            "dtv": dtv,
            "wq": np.ascontiguousarray(wq[:, :, jlo : jlo + P]),
            "wk": np.ascontiguousarray(wk[:, :, jlo : jlo + P]),
            "wv": np.ascontiguousarray(wv[:, :, jlo : jlo + P]),
            "wo": np.ascontiguousarray(wo[:, jlo : jlo + P, :]),
            "wup": np.ascontiguousarray(
                np.concatenate(
                    (
                        wup[:, :, clo : clo + CT * P],
                        wup[:, :, INNER + clo : INNER + clo + CT * P],
                    ),
                    axis=-1,
                )
            ),
            "dww": np.ascontiguousarray(dww[:, clo : clo + CT * P, :]),
            "dwb": np.ascontiguousarray(dwb[:, clo : clo + CT * P]),
            "wdn": np.ascontiguousarray(wdn[:, clo : clo + CT * P, :]),
            "n1s": ii["n1_s"].astype(np.float32),
            "n1b": ii["n1_b"].astype(np.float32),
            "n2s": ii["n2_s"].astype(np.float32),
            "n2b": ii["n2_b"].astype(np.float32),
            "lmt": np.ascontiguousarray(lmT[:, g * VC : (g + 1) * VC]),
            "hwt": hwT,
            "hb": hb,
        }
        in_maps.append(m)
    return in_maps


def kernel(**inputs):
    from concourse.bass_utils import run_bass_kernel_spmd

    nc = _get_nc()
    in_maps = _prep_in_maps(inputs)
    res = run_bass_kernel_spmd(nc, in_maps, core_ids=list(range(8))).results
    logits = np.zeros((B, N, V), np.float32)
    for c in range(8):
        b, g = c // GRP, c % GRP
        logits[b, :, g * VC : (g + 1) * VC] = res[c]["logits"]
    q_logits = np.stack([res[0]["qlog"][0], res[GRP]["qlog"][0]])
    return logits, q_logits


# revision 19
# speedup vs baseline: 1.1877x; 1.1877x over previous
"""Trainium2 Bass kernel for nn_AMKPDModel (linear-attention transformer,
K=4 blocks x 2 cycles, ConvSwiGLU FFN, 32k-vocab LM head) on 8 NeuronCores.

Sharding: 2 data-parallel groups of 4 cores (one per batch element).
Within a group: attention heads sharded 2/core, FFN inner dim sharded
384/core, lm_head vocab sharded 8000/core. Two group-local AllReduces
per block eval ([1024,512] bf16), chunked for compute/comm overlap.
"""

import sys

if "/opt/trn_rl_repo" not in sys.path:
    sys.path.insert(0, "/opt/trn_rl_repo")

import numpy as np

# model dims
B, N, D = 2, 1024, 512
K = 4
V = 32000
INNER = 1536
CK = 3
EPS = 1e-5
CYCLES = 2  # H_CYCLES runs of the 4-block stack

# sharding
GRP = 4            # cores per batch group
P = 128
NT = N // P        # 8 token tiles
DT = D // P        # 4 feature tiles
JT = 6             # up-proj 768/128 local tiles (3 G + 3 U)
CT = 3             # local inner tiles (384/128)
VC = V // GRP      # 8000 vocab rows per core
VCH = 500
NVC = VC // VCH    # 16 lm chunks
NCHUNK = 2         # AR chunks per block output (token-split)
TC_PER_CHUNK = NT // NCHUNK

RG = [[0, 1, 2, 3], [4, 5, 6, 7]]

_CACHE = {}
DEBUG = False
SKIP_COLLECTIVE = False  # timing-only: replace AR with local copy


def _build():
    import concourse.bass as bass
    import concourse.mybir as mybir
    import concourse.tile as tile
    from concourse import bacc
    from concourse.masks import make_identity

    f32 = mybir.dt.float32
    bf16 = mybir.dt.bfloat16
    i32 = mybir.dt.int32
    AF = mybir.ActivationFunctionType
    OP = mybir.AluOpType

    nc = bacc.Bacc(None, target_bir_lowering=False, debug=False, num_devices=8)

    # ---------------- DRAM params ----------------
    ids_d = nc.declare_dram_parameter("ids", [N], i32, isOutput=False)
    q0_d = nc.declare_dram_parameter("q0", [N, D], f32, isOutput=False)
    emb_d = nc.declare_dram_parameter("emb", [V, D], f32, isOutput=False)
    pos_d = nc.declare_dram_parameter("posn", [N, D], f32, isOutput=False)
    ins_d = nc.declare_dram_parameter("ins_v", [D], f32, isOutput=False)
    inb_d = nc.declare_dram_parameter("inb_v", [D], f32, isOutput=False)
    fins_d = nc.declare_dram_parameter("fins_v", [D], f32, isOutput=False)
    finb_d = nc.declare_dram_parameter("finb_v", [D], f32, isOutput=False)
    dtv_d = nc.declare_dram_parameter("dtv", [1, K], f32, isOutput=False)
    wq_d = nc.declare_dram_parameter("wq", [K, D, P], f32, isOutput=False)
    wk_d = nc.declare_dram_parameter("wk", [K, D, P], f32, isOutput=False)
    wv_d = nc.declare_dram_parameter("wv", [K, D, P], f32, isOutput=False)
    wo_d = nc.declare_dram_parameter("wo", [K, P, D], f32, isOutput=False)
    wup_d = nc.declare_dram_parameter("wup", [K, D, JT * P], f32, isOutput=False)
    dww_d = nc.declare_dram_parameter("dww", [K, CT * P, CK], f32, isOutput=False)
    dwb_d = nc.declare_dram_parameter("dwb", [K, CT * P], f32, isOutput=False)
    wdn_d = nc.declare_dram_parameter("wdn", [K, CT * P, D], f32, isOutput=False)
    n1s_d = nc.declare_dram_parameter("n1s", [K, D], f32, isOutput=False)
    n1b_d = nc.declare_dram_parameter("n1b", [K, D], f32, isOutput=False)
    n2s_d = nc.declare_dram_parameter("n2s", [K, D], f32, isOutput=False)
    n2b_d = nc.declare_dram_parameter("n2b", [K, D], f32, isOutput=False)
    lmt_d = nc.declare_dram_parameter("lmt", [D, VC], f32, isOutput=False)
    hwt_d = nc.declare_dram_parameter("hwt", [D, 2], f32, isOutput=False)
    hb_d = nc.declare_dram_parameter("hb", [1, 2], f32, isOutput=False)

    logits_d = nc.declare_dram_parameter("logits", [N, VC], f32, isOutput=True)
    qlog_d = nc.declare_dram_parameter("qlog", [1, 2], f32, isOutput=True)
    dbg = {}
    if DEBUG:
        bf16_ = mybir.dt.bfloat16
        for nm, shp, dt_ in (
            ("dbg_xb", [P, NT, D], bf16_),
            ("dbg_z", [P, NT, D], bf16_),
            ("dbg_zt", [P, DT, N], bf16_),
            ("dbg_phiq", [P, N], bf16_),
            ("dbg_phik", [P, N], bf16_),
            ("dbg_vpt", [P, N], bf16_),
            ("dbg_wt", [P, NT, N], bf16_),
            ("dbg_mcat", [P, N], bf16_),
            ("dbg_y", [P, NT, D], bf16_),
            ("dbg_q1", [P, NT, D], f32),
            ("dbg_hf", [P, CT, N + 2], bf16_),
            ("dbg_hcv", [P, CT, N], bf16_),
            ("dbg_y2", [P, NT, D], bf16_),
            ("dbg_q2", [P, NT, D], f32),
        ):
            dbg[nm] = nc.declare_dram_parameter(nm, shp, dt_, isOutput=True)

    with tile.TileContext(nc) as tc:
        pers = tc.alloc_tile_pool(name="pers", bufs=1)
        ps_big = tc.alloc_tile_pool(name="psb", bufs=2, space="PSUM")
        ps_sml = tc.alloc_tile_pool(name="pss", bufs=4, space="PSUM")
        dram = tc.alloc_tile_pool(name="dram", bufs=2, space="DRAM")

        # ---------------- persistent tiles ----------------
        eps_t = pers.tile([P, 1], f32, tag="eps")
        nc.vector.memset(eps_t[:], EPS)
        ones1 = pers.tile([1, P], bf16, tag="ones1")
        nc.vector.memset(ones1[:], 1.0)
        ident = pers.tile([P, P], bf16, tag="ident")
        make_identity(nc, ident[:])

        ids_sb = pers.tile([P, NT], i32, tag="ids")
        nc.sync.dma_start(ids_sb[:], ids_d[:].rearrange("(t p) -> p t", p=P))
        q_res = pers.tile([P, NT, D], f32, tag="qres")
        nc.sync.dma_start(q_res[:], q0_d[:].rearrange("(t p) d -> p t d", p=P))

        def bcast_row(src_d, tag):
            t = pers.tile([P, D], f32, tag=tag)
            nc.sync.dma_start(t[:], src_d[:][None, :].to_broadcast([P, D]))
            return t

        fins_bc = bcast_row(fins_d, "finsbc")
        finb_bc = bcast_row(finb_d, "finbbc")

        # softplus(dt) broadcast to all partitions
        dtv_sb = pers.tile([1, K], f32, tag="dtv")
        nc.sync.dma_start(dtv_sb[:], dtv_d[:])
        spe = pers.tile([1, K], f32, tag="spe")
        nc.scalar.activation(spe[:], dtv_sb[:], AF.Exp)
        nc.vector.tensor_scalar_add(spe[:], spe[:], 1.0)
        nc.scalar.activation(spe[:], spe[:], AF.Ln)
        sp_bc = pers.tile([P, K], f32, tag="spbc")
        nc.gpsimd.partition_broadcast(sp_bc[:], spe[:])

        # per-block norm scale vectors (transposed layouts)
        n1s_sb = pers.tile([P, K, DT], f32, tag="n1s")
        nc.sync.dma_start(n1s_sb[:], n1s_d[:].rearrange("k (dt p) -> p k dt", p=P))
        n2s_sb = pers.tile([P, K, DT], f32, tag="n2s")
        nc.sync.dma_start(n2s_sb[:], n2s_d[:].rearrange("k (dt p) -> p k dt", p=P))

        # weights (loaded f32, cast bf16)
        wq_sb = pers.tile([P, K, DT, P], bf16, tag="wq")
        wk_sb = pers.tile([P, K, DT, P], bf16, tag="wk")
        wv_sb = pers.tile([P, K, DT, P], bf16, tag="wv")
        wo_sb = pers.tile([P, K, D], bf16, tag="wo")
        wup_sb = pers.tile([P, K, DT, JT, P], bf16, tag="wup")
        wdn_sb = pers.tile([P, K, CT, D], bf16, tag="wdn")
        dww_sb = pers.tile([P, K, CT, CK], f32, tag="dww")
        nc.sync.dma_start(dww_sb[:], dww_d[:].rearrange("k (ct p) c -> p k ct c", p=P))
        dwb_sb = pers.tile([P, K, CT], f32, tag="dwb")
        nc.sync.dma_start(dwb_sb[:], dwb_d[:].rearrange("k (ct p) -> p k ct", p=P))
        diag_sb = pers.tile([P, K, CT, CK, P], bf16, tag="diag")
        b1t_sb = pers.tile([P, DT, K], bf16, tag="b1t")
        b2t_sb = pers.tile([P, DT, K], bf16, tag="b2t")
        b2w_sb = pers.tile([P, K, JT], f32, tag="b2w")
        hwt_sb = pers.tile([P, DT, 2], f32, tag="hwt")
        nc.sync.dma_start(hwt_sb[:], hwt_d[:].rearrange("(dt p) c -> p dt c", p=P))
        hb_sb = pers.tile([1, 2], f32, tag="hb")
        nc.sync.dma_start(hb_sb[:], hb_d[:])

        xw_sb = pers.tile([P, K, 3, N], bf16, tag="xw")
        vp_aug = pers.tile([P, NT, 2, 64], bf16, tag="vpaug")
        onescol = pers.tile([P, 1], bf16, tag="onescol")
        nc.vector.memset(onescol[:], 1.0)
        wt_sb = pers.tile([P, NT, 2, 512], bf16, tag="wt")  # both heads, one n-block
        hf_sb = pers.tile([P, CT, N + 2], bf16, tag="hf")
        nc.vector.memset(hf_sb[:], 0.0)
        zft_sb = pers.tile([P, DT, N], bf16, tag="zft")

        # z / zt shared across LN sites (persistent; evals are serial anyway)
        z_sb = pers.tile([P, NT, D], bf16, tag="z")
        zt_sb = pers.tile([P, DT, N], bf16, tag="zt")
        mcat_sb = pers.tile([P, N], bf16, tag="mcat")

        # ---------------- setup (scoped transients) ----------------
        with tc.tile_pool(name="setup", bufs=1) as setup, tc.tile_pool(
            name="setup2", bufs=2
        ) as setup2:
            # cast helper: DMA f32 -> stage, cast to dst (bf16)
            def load_cast(dst_ap, src_ap, shape, tag, eng):
                s = setup.tile(shape, f32, tag="wstage")
                nc.sync.dma_start(s[:], src_ap)
                if eng == 0:
                    nc.vector.tensor_copy(dst_ap, s[:])
                else:
                    nc.scalar.copy(dst_ap, s[:])

            for i, (dst, src) in enumerate(
                ((wq_sb, wq_d), (wk_sb, wk_d), (wv_sb, wv_d))
            ):
                load_cast(
                    dst[:],
                    src[:].rearrange("k (dt p) j -> p k dt j", p=P),
                    [P, K, DT, P],
                    "wstage",
                    i % 2,
                )
            load_cast(
                wo_sb[:],
                wo_d[:].rearrange("k p d -> p k d"),
                [P, K, D],
                "wstage",
                1,
            )
            for k in range(K):
                load_cast(
                    wup_sb[:, k],
                    wup_d[k].rearrange("(dt p) (jt jj) -> p dt jt jj", p=P, jj=P),
                    [P, DT, JT, P],
                    "wstage",
                    k % 2,
                )
                load_cast(
                    wdn_sb[:, k],
                    wdn_d[k].rearrange("(ct p) d -> p ct d", p=P),
                    [P, CT, D],
                    "wstage",
                    (k + 1) % 2,
                )
            for bd, bt in ((n1b_d, b1t_sb), (n2b_d, b2t_sb)):
                bcol = setup2.tile([P, DT, K], f32, tag="bcol")
                for k in range(K):
                    for dtt in range(DT):
                        nc.sync.dma_start(
                            bcol[:, dtt, k : k + 1],
                            bd[k, dtt * P : (dtt + 1) * P][:, None],
                        )
                nc.vector.tensor_copy(bt[:], bcol[:])

            # depthwise conv as matmul: diagonal weight mats per (k, ct, tap)
            for k in range(K):
                for ct in range(CT):
                    for tap in range(CK):
                        nc.vector.tensor_scalar_mul(
                            diag_sb[:, k, ct, tap, :],
                            ident[:],
                            dww_sb[:, k, ct, tap : tap + 1],
                        )

            # ---- embedding + input LN -> X (bf16) -> XT
            ins_bc = setup.tile([P, D], f32, tag="insbc")
            nc.sync.dma_start(ins_bc[:], ins_d[:][None, :].to_broadcast([P, D]))
            inb_bc = setup.tile([P, D], f32, tag="inbbc")
            nc.sync.dma_start(inb_bc[:], inb_d[:][None, :].to_broadcast([P, D]))

            xb_sb = setup.tile([P, NT, D], bf16, tag="xb0")
            st = setup.tile([P, NT, 6], f32, tag="st0")
            mv = setup.tile([P, NT, 2], f32, tag="mv0")
            rstd = setup.tile([P, NT], f32, tag="rstd0")
            for t in range(NT):
                xrt = setup2.tile([P, D], f32, tag="xrt")
                nc.gpsimd.indirect_dma_start(
                    out=xrt[:],
                    out_offset=None,
                    in_=emb_d[:],
                    in_offset=bass.IndirectOffsetOnAxis(ap=ids_sb[:, t : t + 1], axis=0),
                )
                ps = setup2.tile([P, D], f32, tag="posst")
                nc.sync.dma_start(
                    ps[:], pos_d[:].rearrange("(t p) d -> p t d", p=P)[:, t, :]
                )
                nc.vector.tensor_tensor(xrt[:], xrt[:], ps[:], op=OP.add)
                nc.vector.bn_stats(st[:, t], xrt[:])
                nc.vector.bn_aggr(mv[:, t], st[:, t])
                nc.scalar.activation(
                    rstd[:, t : t + 1], mv[:, t, 1:2], AF.Ln, bias=eps_t[:]
                )
                nc.scalar.activation(
                    rstd[:, t : t + 1], rstd[:, t : t + 1], AF.Exp, scale=-0.5
                )
                nc.vector.tensor_scalar(
                    xb_sb[:, t],
                    xrt[:],
                    mv[:, t, 0:1],
                    rstd[:, t : t + 1],
                    op0=OP.subtract,
                    op1=OP.mult,
                )
                nc.vector.tensor_tensor(xb_sb[:, t], xb_sb[:, t], ins_bc[:], op=OP.mult)
                nc.vector.tensor_tensor(xb_sb[:, t], xb_sb[:, t], inb_bc[:], op=OP.add)
            if DEBUG:
                nc.sync.dma_start(dbg["dbg_xb"][:], xb_sb[:])
            xt_sb = setup.tile([P, DT, N], bf16, tag="xt")
            for t in range(NT):
                for dtt in range(DT):
                    nc.sync.dma_start_transpose(
                        xt_sb[:, dtt, t * P : (t + 1) * P],
                        xb_sb[:, t, dtt * P : (dtt + 1) * P],
                    )

            # ---- XW = (b1 + X) @ W  (raw weights, before s1 fold)
            for k in range(K):
                for pi, wsb in enumerate((wq_sb, wk_sb, wv_sb)):
                    pb = ps_sml.tile([P, 512], f32, tag="sml")
                    for dtt in range(DT):
                        nc.tensor.matmul(
                            pb[:, 0:1],
                            lhsT=wsb[:, k, dtt, :],
                            rhs=b1t_sb[:, dtt, k : k + 1],
                            start=(dtt == 0),
                            stop=(dtt == DT - 1),
                        )
                    b1w = setup2.tile([P, 1], f32, tag="b1w")
                    nc.vector.tensor_copy(b1w[:], pb[:, 0:1])
                    for nb in range(2):
                        pp = ps_sml.tile([P, 512], f32, tag="sml")
                        for dtt in range(DT):
                            nc.tensor.matmul(
                                pp[:],
                                lhsT=wsb[:, k, dtt, :],
                                rhs=xt_sb[:, dtt, nb * 512 : (nb + 1) * 512],
                                start=(dtt == 0),
                                stop=(dtt == DT - 1),
                            )
                        nc.vector.tensor_scalar(
                            xw_sb[:, k, pi, nb * 512 : (nb + 1) * 512],
                            pp[:],
                            b1w[:],
                            None,
                            op0=OP.add,
                        )

            # ---- b2W per (k, jt) (raw wup)
            for k in range(K):
                for jt in range(JT):
                    pb = ps_sml.tile([P, 512], f32, tag="sml")
                    for dtt in range(DT):
                        nc.tensor.matmul(
                            pb[:, 0:1],
                            lhsT=wup_sb[:, k, dtt, jt, :],
                            rhs=b2t_sb[:, dtt, k : k + 1],
                            start=(dtt == 0),
                            stop=(dtt == DT - 1),
                        )
                    eng = nc.vector if (k + jt) % 2 == 0 else nc.scalar
                    if eng is nc.vector:
                        nc.vector.tensor_copy(b2w_sb[:, k, jt : jt + 1], pb[:, 0:1])
                    else:
                        nc.scalar.copy(b2w_sb[:, k, jt : jt + 1], pb[:, 0:1])

            # ---- fold s1 into wq/wk/wv, s2 into wup, softplus(dt) into wo
            for wsb in (wq_sb, wk_sb, wv_sb):
                for k in range(K):
                    nc.vector.tensor_tensor(
                        wsb[:, k],
                        wsb[:, k],
                        n1s_sb[:, k, :, None].to_broadcast([P, DT, P]),
                        op=OP.mult,
                    )
            for k in range(K):
                nc.vector.tensor_tensor(
                    wup_sb[:, k],
                    wup_sb[:, k],
                    n2s_sb[:, k, :, None, None].to_broadcast([P, DT, JT, P]),
                    op=OP.mult,
                )
                nc.vector.tensor_tensor(
                    wo_sb[:, k],
                    wo_sb[:, k],
                    sp_bc[:, k : k + 1].to_broadcast([P, D]),
                    op=OP.mult,
                )

        # ---------------- block evals ----------------
        with tc.tile_pool(name="work", bufs=1) as work, tc.tile_pool(
            name="work2", bufs=2
        ) as work2:

            def layernorm(src, t_lo, t_hi, out, scale_bc=None, bias_bc=None):
                """LN over d for token tiles [t_lo, t_hi) of src [P,NT,D] f32.
                Writes (x-mu)*rstd (optionally *scale+bias) to out (bf16)."""
                nt = t_hi - t_lo
                st = work.tile([P, NT, 6], f32, tag="st")
                mv = work.tile([P, NT, 2], f32, tag="mv")
                for t in range(t_lo, t_hi):
                    nc.vector.bn_stats(st[:, t], src[:, t])
                    nc.vector.bn_aggr(mv[:, t], st[:, t])
                rstd = work.tile([P, NT], f32, tag="rstd")
                nc.scalar.activation(
                    rstd[:, t_lo:t_hi], mv[:, t_lo:t_hi, 1], AF.Ln, bias=eps_t[:]
                )
                nc.scalar.activation(
                    rstd[:, t_lo:t_hi], rstd[:, t_lo:t_hi], AF.Exp, scale=-0.5
                )
                nmr = work.tile([P, NT], f32, tag="nmr")
                nc.vector.tensor_tensor(
                    nmr[:, t_lo:t_hi], mv[:, t_lo:t_hi, 0], rstd[:, t_lo:t_hi], op=OP.mult
                )
                nc.vector.tensor_scalar_mul(nmr[:, t_lo:t_hi], nmr[:, t_lo:t_hi], -1.0)
                for t in range(t_lo, t_hi):
                    if t % 2 == 0:
                        nc.vector.tensor_scalar(
                            out[:, t],
                            src[:, t],
                            mv[:, t, 0:1],
                            rstd[:, t : t + 1],
                            op0=OP.subtract,
                            op1=OP.mult,
                        )
                    else:
                        nc.scalar.activation(
                            out[:, t],
                            src[:, t],
                            AF.Identity,
                            bias=nmr[:, t : t + 1],
                            scale=rstd[:, t : t + 1],
                        )

            def transpose_tiles(src, dst, t_lo, t_hi):
                for t in range(t_lo, t_hi):
                    for dtt in range(DT):
                        eng = nc.sync if (t + dtt) % 2 == 0 else nc.scalar
                        eng.dma_start_transpose(
                            dst[:, dtt, t * P : (t + 1) * P],
                            src[:, t, dtt * P : (dtt + 1) * P],
                        )

            def all_reduce_chunked(y, scaled_add_dst):
                """AllReduce y [P,NT,D] bf16 in NCHUNK token chunks; add into
                scaled_add_dst (q_res) in place."""
                for c in range(NCHUNK):
                    t0, t1 = c * TC_PER_CHUNK, (c + 1) * TC_PER_CHUNK
                    yb = dram.tile([P, TC_PER_CHUNK, D], bf16, tag="arin")
                    ab = dram.tile([P, TC_PER_CHUNK, D], bf16, tag="arout")
                    nc.sync.dma_start(yb[:], y[:, t0:t1])
                    if SKIP_COLLECTIVE:
                        nc.gpsimd.dma_start(ab[:], yb[:])
                    else:
                        nc.gpsimd.collective_compute(
                            "AllReduce",
                            OP.add,
                            replica_groups=RG,
                            ins=[yb[:].opt()],
                            outs=[ab[:].opt()],
                        )
                    nc.gpsimd.dma_start(
                        scaled_add_dst[:, t0:t1],
                        ab[:],
                        accum_op=OP.add,
                    )

            def block_eval(k, dump=False):
                # ---- LN1 -> z -> zT
                layernorm(q_res, 0, NT, z_sb)
                transpose_tiles(z_sb, zt_sb, 0, NT)
                if dump:
                    nc.sync.dma_start(dbg["dbg_z"][:], z_sb[:])
                    nc.sync.dma_start(dbg["dbg_zt"][:], zt_sb[:])

                # ---- projections (q, k, v)
                phi = [None, None, None]
                for pi, wsb in enumerate((wq_sb, wk_sb, wv_sb)):
                    pp = ps_big.tile([P, 1024], f32, tag="big")
                    for nb in range(2):
                        for dtt in range(DT):
                            nc.tensor.matmul(
                                pp[:, nb * 512 : (nb + 1) * 512],
                                lhsT=wsb[:, k, dtt, :],
                                rhs=zt_sb[:, dtt, nb * 512 : (nb + 1) * 512],
                                start=(dtt == 0),
                                stop=(dtt == DT - 1),
                            )
                    tag = ("phiq", "phik", "vpt")[pi]
                    res = work.tile([P, N], bf16, tag=tag)
                    if pi < 2:
                        ts_t = work2.tile([P, N], bf16, tag="tsum")
                        nc.vector.tensor_tensor(
                            ts_t[:], pp[:], xw_sb[:, k, pi, :], op=OP.add
                        )
                        rel = work.tile([P, N], bf16, tag="rel")
                        nc.vector.tensor_scalar_max(rel[:], ts_t[:], 0.0)
                        mn = work.tile([P, N], bf16, tag="mn")
                        nc.vector.tensor_scalar_min(mn[:], ts_t[:], 0.0)
                        ex = work.tile([P, N], bf16, tag="ex")
                        nc.scalar.activation(ex[:], mn[:], AF.Exp)
                        nc.vector.tensor_tensor(res[:], ex[:], rel[:], op=OP.add)
                    else:
                        nc.vector.tensor_tensor(
                            res[:], pp[:], xw_sb[:, k, pi, :], op=OP.add
                        )
                        for mt in range(NT):
                            for h in range(2):
                                eng = nc.sync if mt % 2 == 0 else nc.scalar
                                eng.dma_start_transpose(
                                    vp_aug[:, mt, h, 0:64],
                                    res[h * 64 : (h + 1) * 64, mt * P : (mt + 1) * P],
                                )
                    phi[pi] = res
                phiq, phik, vpt = phi
                if dump:
                    nc.sync.dma_start(dbg["dbg_phiq"][:], phiq[:])
                    nc.sync.dma_start(dbg["dbg_phik"][:], phik[:])
                    nc.sync.dma_start(dbg["dbg_vpt"][:], vpt[:])

                # ---- attention: nb-outer, heads packed via tile_position
                for nb in range(2):
                    nsl = slice(nb * 512, (nb + 1) * 512)
                    for mt in range(NT):
                        for h in range(2):
                            hs = h * 64
                            pw = ps_sml.tile([P, 512], f32, tag="sml")
                            nc.tensor.matmul(
                                pw[:],
                                lhsT=phik[hs : hs + 64, mt * P : (mt + 1) * P],
                                rhs=phiq[hs : hs + 64, nsl],
                                start=True,
                                stop=True,
                            )
                            dst = wt_sb[:, mt, h, :]
                            if (mt + h) % 2 == 0:
                                nc.scalar.activation(dst, pw[:], AF.Square)
                            else:
                                wc = work2.tile([P, 512], bf16, tag="wc")
                                nc.vector.tensor_copy(wc[:], pw[:])
                                nc.vector.tensor_tensor(dst, wc[:], wc[:], op=OP.mult)
                    pa = ps_sml.tile([P, 512], f32, tag="sml")
                    pss0 = ps_sml.tile([P, 512], f32, tag="sml")
                    pss1 = ps_sml.tile([P, 512], f32, tag="sml")
                    for mt in range(NT):
                        for h in range(2):
                            hs = h * 64
                            nc.tensor.matmul(
                                pa[hs : hs + 64, :],
                                lhsT=vp_aug[:, mt, h, :],
                                rhs=wt_sb[:, mt, h, :],
                                start=(mt == 0),
                                stop=(mt == NT - 1),
                                tile_position=(0, hs),
                                skip_group_check=True,
                            )
                            nc.tensor.matmul(
                                (pss0 if h == 0 else pss1)[0:1, :],
                                lhsT=onescol[:],
                                rhs=wt_sb[:, mt, h, :],
                                start=(mt == 0),
                                stop=(mt == NT - 1),
                            )
                    prr = work.tile([P, 512], bf16, tag="prr")
                    for h in range(2):
                        hs = h * 64
                        pss = pss0 if h == 0 else pss1
                        rr = work.tile([1, 512], f32, tag="rr")
                        nc.vector.tensor_scalar_add(rr[:], pss[0:1, :], 1.0)
                        nc.vector.reciprocal_approx_fast(rr[:], rr[:])
                        rrb = work.tile([1, 512], bf16, tag="rrb")
                        nc.vector.tensor_copy(rrb[:], rr[:])
                        rsc = dram.tile([1, 512], bf16, tag="rsc")
                        nc.sync.dma_start(rsc[:], rrb[:])
                        nc.scalar.dma_start(
                            prr[hs : hs + 64, :],
                            rsc[0][None, :].to_broadcast([64, 512]),
                        )
                    at = work.tile([P, 512], bf16, tag="atr")
                    if nb == 0:
                        nc.vector.tensor_copy(at[:], pa[:])
                    else:
                        nc.scalar.copy(at[:], pa[:])
                    tm = work.tile([P, 512], bf16, tag="tm")
                    nc.vector.tensor_tensor(tm[:], at[:], prr[:], op=OP.mult)
                    nc.gpsimd.tensor_tensor(
                        mcat_sb[:, nsl], tm[:], vpt[:, nsl], op=OP.subtract
                    )

                # ---- out-proj (wo pre-scaled by softplus(dt)) + chunked AR
                y = work.tile([P, NT, D], bf16, tag="y")
                for nt in range(NT):
                    po = ps_sml.tile([P, 512], f32, tag="sml")
                    nc.tensor.matmul(
                        po[:],
                        lhsT=mcat_sb[:, nt * P : (nt + 1) * P],
                        rhs=wo_sb[:, k, :],
                        start=True,
                        stop=True,
                    )
                    if nt % 2 == 0:
                        nc.vector.tensor_copy(y[:, nt], po[:])
                    else:
                        nc.scalar.copy(y[:, nt], po[:])
                if dump:
                    nc.sync.dma_start(dbg["dbg_mcat"][:], mcat_sb[:])
                    nc.sync.dma_start(dbg["dbg_y"][:], y[:])
                all_reduce_chunked(y, q_res)
                if dump:
                    nc.sync.dma_start(dbg["dbg_q1"][:], q_res[:])

                # ---- LN2 -> z2 -> z2T
                layernorm(q_res, 0, NT, z_sb)
                transpose_tiles(z_sb, zt_sb, 0, NT)

                # ---- up-proj + SwiGLU -> hf
                for nb in range(2):
                    for jp in range(CT):
                        pg = ps_sml.tile([P, 512], f32, tag="sml")
                        for dtt in range(DT):
                            nc.tensor.matmul(
                                pg[:],
                                lhsT=wup_sb[:, k, dtt, jp, :],
                                rhs=zt_sb[:, dtt, nb * 512 : (nb + 1) * 512],
                                start=(dtt == 0),
                                stop=(dtt == DT - 1),
                            )
                        pu = ps_sml.tile([P, 512], f32, tag="sml")
                        for dtt in range(DT):
                            nc.tensor.matmul(
                                pu[:],
                                lhsT=wup_sb[:, k, dtt, jp + CT, :],
                                rhs=zt_sb[:, dtt, nb * 512 : (nb + 1) * 512],
                                start=(dtt == 0),
                                stop=(dtt == DT - 1),
                            )
                        sg = work2.tile([P, 512], bf16, tag="sg")
                        nc.scalar.activation(
                            sg[:], pg[:], AF.Silu, bias=b2w_sb[:, k, jp : jp + 1]
                        )
                        uu = work2.tile([P, 512], bf16, tag="uu")
                        nc.vector.tensor_scalar(
                            uu[:], pu[:], b2w_sb[:, k, jp + CT : jp + CT + 1], None, op0=OP.add
                        )
                        nc.gpsimd.tensor_tensor(
                            hf_sb[:, jp, 1 + nb * 512 : 1 + (nb + 1) * 512],
                            sg[:],
                            uu[:],
                            op=OP.mult,
                        )

                # ---- depthwise conv (as 3 diag matmuls) + silu -> hcv
                hcv = work.tile([P, CT, N], bf16, tag="hcv")
                for ct in range(CT):
                    for nb in range(2):
                        pc = ps_sml.tile([P, 512], f32, tag="sml")
                        for tap in range(CK):
                            nc.tensor.matmul(
                                pc[:],
                                lhsT=diag_sb[:, k, ct, tap, :],
                                rhs=hf_sb[:, ct, nb * 512 + tap : nb * 512 + tap + 512],
                                start=(tap == 0),
                                stop=(tap == CK - 1),
                            )
                        nc.scalar.activation(
                            hcv[:, ct, nb * 512 : (nb + 1) * 512],
                            pc[:],
                            AF.Silu,
                            bias=dwb_sb[:, k, ct : ct + 1],
                        )

                if dump:
                    nc.sync.dma_start(dbg["dbg_hf"][:], hf_sb[:])
                    nc.sync.dma_start(dbg["dbg_hcv"][:], hcv[:])
                # ---- down-proj + chunked AR
                y2 = work.tile([P, NT, D], bf16, tag="y")
                for nt in range(NT):
                    pd = ps_sml.tile([P, 512], f32, tag="sml")
                    for ct in range(CT):
                        nc.tensor.matmul(
                            pd[:],
                            lhsT=hcv[:, ct, nt * P : (nt + 1) * P],
                            rhs=wdn_sb[:, k, ct, :],
                            start=(ct == 0),
                            stop=(ct == CT - 1),
                        )
                    if nt % 2 == 0:
                        nc.vector.tensor_copy(y2[:, nt], pd[:])
                    else:
                        nc.scalar.copy(y2[:, nt], pd[:])
                if dump:
                    nc.sync.dma_start(dbg["dbg_y2"][:], y2[:])
                all_reduce_chunked(y2, q_res)
                if dump:
                    nc.sync.dma_start(dbg["dbg_q2"][:], q_res[:])

            for _cyc in range(CYCLES):
                for k in range(K):
                    block_eval(k, dump=(DEBUG and _cyc == 0 and k == 0))

            # ---------------- final LN (with fin scale/bias) ----------------
            layernorm(q_res, 0, NT, z_sb)
            nc.vector.tensor_tensor(
                z_sb[:], z_sb[:], fins_bc[:, None, :].to_broadcast([P, NT, D]), op=OP.mult
            )
            nc.vector.tensor_tensor(
                z_sb[:], z_sb[:], finb_bc[:, None, :].to_broadcast([P, NT, D]), op=OP.add
            )
            transpose_tiles(z_sb, zft_sb, 0, NT)

            # ---- q_logits = mean_n(Qn) @ halt_w.T + halt_b
            qm = work.tile([P, DT], f32, tag="qm")
            nc.vector.reduce_sum(qm[:], zft_sb[:], axis=mybir.AxisListType.X)
            pq = ps_sml.tile([P, 512], f32, tag="sml")
            for dtt in range(DT):
                nc.tensor.matmul(
                    pq[0:1, 0:2],
                    lhsT=qm[:, dtt : dtt + 1],
                    rhs=hwt_sb[:, dtt, :],
                    start=(dtt == 0),
                    stop=(dtt == DT - 1),
                )
            ql = work.tile([1, 2], f32, tag="ql")
            nc.vector.tensor_scalar_mul(ql[:], pq[0:1, 0:2], 1.0 / N)
            nc.vector.tensor_tensor(ql[:], ql[:], hb_sb[:], op=OP.add)
            nc.sync.dma_start(qlog_d[:], ql[:])

        # ---------------- lm head (vocab-sharded) ----------------
        with tc.tile_pool(name="lmp", bufs=2) as lmp, tc.tile_pool(
            name="lmp1", bufs=1
        ) as lmp1:
            lg = logits_d[:].rearrange("(nt p) v -> p nt v", p=P)
            lmsrc = lmt_d[:].rearrange("(dt p) v -> p dt v", p=P)
            for vc in range(NVC):
                stage = lmp.tile([P, DT, VCH], f32, tag="lstage")
                nc.sync.dma_start(stage[:], lmsrc[:, :, vc * VCH : (vc + 1) * VCH])
                lc = lmp.tile([P, DT, VCH], bf16, tag="lc")
                if vc % 2 == 0:
                    nc.vector.tensor_copy(lc[:], stage[:])
                else:
                    nc.scalar.copy(lc[:], stage[:])
                ob = lmp1.tile([P, NT, VCH], f32, tag="ob")
                for nt in range(NT):
                    pl = ps_sml.tile([P, 512], f32, tag="sml")
                    for dtt in range(DT):
                        nc.tensor.matmul(
                            pl[:, 0:VCH],
                            lhsT=zft_sb[:, dtt, nt * P : (nt + 1) * P],
                            rhs=lc[:, dtt, :],
                            start=(dtt == 0),
                            stop=(dtt == DT - 1),
                        )
                    if (vc + nt) % 2 == 0:
                        nc.vector.tensor_copy(ob[:, nt, :], pl[:, 0:VCH])
                    else:
                        nc.scalar.copy(ob[:, nt, :], pl[:, 0:VCH])
                eng = nc.sync if vc % 2 == 0 else nc.scalar
                eng.dma_start(lg[:, :, vc * VCH : (vc + 1) * VCH], ob[:])

        dram.release()
        ps_sml.release()
        ps_big.release()
        pers.release()

    nc.compile()
    return nc


def _get_nc():
    if "nc" not in _CACHE:
        _CACHE["nc"] = _build()
    return _CACHE["nc"]


def _prep_in_maps(inputs):
    ii = {k: np.asarray(v) for k, v in inputs.items()}
    hm = ii["carry_halted"].astype(bool)
    ids = np.where(hm[:, None], ii["inputs"], ii["carry_inputs"]).astype(np.int32)
    init_h = ii["init_hidden"].astype(np.float32)
    q0 = np.where(
        hm[:, None, None],
        np.broadcast_to(init_h[None, None, :], (B, N, D)),
        ii["carry_hidden"].astype(np.float32),
    ).astype(np.float32)
    emb = np.ascontiguousarray(ii["emb"].astype(np.float32))
    posn = np.ascontiguousarray(ii["pos"].astype(np.float32)[:N])
    lmT = np.ascontiguousarray(ii["lm_w"].astype(np.float32).T)  # [D, V]
    hwT = np.ascontiguousarray(ii["halt_w"].astype(np.float32).T)  # [D, 2]
    hb = ii["halt_b"].astype(np.float32).reshape(1, 2)
    dtv = ii["dt"].astype(np.float32).reshape(1, K)
    wq = ii["W_Q"].astype(np.float32)
    wk = ii["W_K"].astype(np.float32)
    wv = ii["W_V"].astype(np.float32)
    wo = ii["W_O"].astype(np.float32)
    wup = ii["W_up"].astype(np.float32)
    dww = ii["dw_w"].astype(np.float32)[:, :, 0, :]  # [K, INNER, CK]
    dwb = ii["dw_b"].astype(np.float32)
    wdn = ii["W_down"].astype(np.float32)

    in_maps = []
    for c in range(8):
        b, g = c // GRP, c % GRP
        jlo = g * P  # head-col slice (2 heads x 64)
        clo = g * CT * P  # inner slice (384)
        m = {
            "ids": np.ascontiguousarray(ids[b]),
            "q0": np.ascontiguousarray(q0[b]),
            "emb": emb,
            "posn": posn,
            "ins_v": ii["in_s"].astype(np.float32),
            "inb_v": ii["in_b"].astype(np.float32),
            "fins_v": ii["fin_s"].astype(np.float32),
            "finb_v": ii["fin_b"].astype(np.float32),
            "dtv": dtv,
            "wq": np.ascontiguousarray(wq[:, :, jlo : jlo + P]),
            "wk": np.ascontiguousarray(wk[:, :, jlo : jlo + P]),
            "wv": np.ascontiguousarray(wv[:, :, jlo : jlo + P]),
            "wo": np.ascontiguousarray(wo[:, jlo : jlo + P, :]),
            "wup": np.ascontiguousarray(
                np.concatenate(
                    (
                        wup[:, :, clo : clo + CT * P],
                        wup[:, :, INNER + clo : INNER + clo + CT * P],
                    ),
                    axis=-1,
                )
            ),
            "dww": np.ascontiguousarray(dww[:, clo : clo + CT * P, :]),
            "dwb": np.ascontiguousarray(dwb[:, clo : clo + CT * P]),
            "wdn": np.ascontiguousarray(wdn[:, clo : clo + CT * P, :]),
            "n1s": ii["n1_s"].astype(np.float32),
            "n1b": ii["n1_b"].astype(np.float32),
            "n2s": ii["n2_s"].astype(np.float32),
            "n2b": ii["n2_b"].astype(np.float32),
            "lmt": np.ascontiguousarray(lmT[:, g * VC : (g + 1) * VC]),
            "hwt": hwT,
            "hb": hb,
        }
        in_maps.append(m)
    return in_maps


def kernel(**inputs):
    from concourse.bass_utils import run_bass_kernel_spmd

    nc = _get_nc()
    in_maps = _prep_in_maps(inputs)
    res = run_bass_kernel_spmd(nc, in_maps, core_ids=list(range(8))).results
    logits = np.zeros((B, N, V), np.float32)
    for c in range(8):
        b, g = c // GRP, c % GRP
        logits[b, :, g * VC : (g + 1) * VC] = res[c]["logits"]
    q_logits = np.stack([res[0]["qlog"][0], res[GRP]["qlog"][0]])
    return logits, q_logits


# revision 20
# speedup vs baseline: 2.7242x; 2.2937x over previous
"""Trainium2 Bass kernel for nn_AMKPDModel (linear-attention transformer,
K=4 blocks x 2 cycles, ConvSwiGLU FFN, 32k-vocab LM head) on 8 NeuronCores.

Sharding: 2 data-parallel groups of 4 cores (one per batch element).
Within a group: attention heads sharded 2/core, FFN inner dim sharded
384/core, lm_head vocab sharded 8000/core. Two group-local AllReduces
per block eval ([1024,512] bf16), chunked for compute/comm overlap.
"""

import sys

if "/opt/trn_rl_repo" not in sys.path:
    sys.path.insert(0, "/opt/trn_rl_repo")

import numpy as np

# model dims
B, N, D = 2, 1024, 512
K = 4
V = 32000
INNER = 1536
CK = 3
EPS = 1e-5
CYCLES = 2  # H_CYCLES runs of the 4-block stack

# sharding
GRP = 4            # cores per batch group
P = 128
NT = N // P        # 8 token tiles
DT = D // P        # 4 feature tiles
JT = 6             # up-proj 768/128 local tiles (3 G + 3 U)
CT = 3             # local inner tiles (384/128)
VC = V // GRP      # 8000 vocab rows per core
VCH = 500
NVC = VC // VCH    # 16 lm chunks
NCHUNK = 2         # AR chunks per block output (token-split)
TC_PER_CHUNK = NT // NCHUNK

RG = [[0, 1, 2, 3], [4, 5, 6, 7]]

_CACHE = {}
DEBUG = False
SKIP_COLLECTIVE = False  # timing-only: replace AR with local copy


def _build():
    import concourse.bass as bass
    import concourse.mybir as mybir
    import concourse.tile as tile
    from concourse import bacc
    from concourse.masks import make_identity

    f32 = mybir.dt.float32
    bf16 = mybir.dt.bfloat16
    i32 = mybir.dt.int32
    AF = mybir.ActivationFunctionType
    OP = mybir.AluOpType

    nc = bacc.Bacc(None, target_bir_lowering=False, debug=False, num_devices=8)

    # ---------------- DRAM params ----------------
    ids_d = nc.declare_dram_parameter("ids", [N], i32, isOutput=False)
    q0_d = nc.declare_dram_parameter("q0", [N, D], f32, isOutput=False)
    emb_d = nc.declare_dram_parameter("emb", [V, D], f32, isOutput=False)
    pos_d = nc.declare_dram_parameter("posn", [N, D], f32, isOutput=False)
    ins_d = nc.declare_dram_parameter("ins_v", [D], f32, isOutput=False)
    inb_d = nc.declare_dram_parameter("inb_v", [D], f32, isOutput=False)
    fins_d = nc.declare_dram_parameter("fins_v", [D], f32, isOutput=False)
    finb_d = nc.declare_dram_parameter("finb_v", [D], f32, isOutput=False)
    dtv_d = nc.declare_dram_parameter("dtv", [1, K], f32, isOutput=False)
    wq_d = nc.declare_dram_parameter("wq", [K, D, P], f32, isOutput=False)
    wk_d = nc.declare_dram_parameter("wk", [K, D, P], f32, isOutput=False)
    wv_d = nc.declare_dram_parameter("wv", [K, D, P], f32, isOutput=False)
    wo_d = nc.declare_dram_parameter("wo", [K, P, D], f32, isOutput=False)
    wup_d = nc.declare_dram_parameter("wup", [K, D, JT * P], f32, isOutput=False)
    dww_d = nc.declare_dram_parameter("dww", [K, CT * P, CK], f32, isOutput=False)
    dwb_d = nc.declare_dram_parameter("dwb", [K, CT * P], f32, isOutput=False)
    wdn_d = nc.declare_dram_parameter("wdn", [K, CT * P, D], f32, isOutput=False)
    n1s_d = nc.declare_dram_parameter("n1s", [K, D], f32, isOutput=False)
    n1b_d = nc.declare_dram_parameter("n1b", [K, D], f32, isOutput=False)
    n2s_d = nc.declare_dram_parameter("n2s", [K, D], f32, isOutput=False)
    n2b_d = nc.declare_dram_parameter("n2b", [K, D], f32, isOutput=False)
    lmt_d = nc.declare_dram_parameter("lmt", [D, VC], f32, isOutput=False)
    hwt_d = nc.declare_dram_parameter("hwt", [D, 2], f32, isOutput=False)
    hb_d = nc.declare_dram_parameter("hb", [1, 2], f32, isOutput=False)

    logits_d = nc.declare_dram_parameter("logits", [N, VC], f32, isOutput=True)
    qlog_d = nc.declare_dram_parameter("qlog", [1, 2], f32, isOutput=True)
    dbg = {}
    if DEBUG:
        bf16_ = mybir.dt.bfloat16
        for nm, shp, dt_ in (
            ("dbg_xb", [P, NT, D], bf16_),
            ("dbg_z", [P, NT, D], bf16_),
            ("dbg_zt", [P, DT, N], bf16_),
            ("dbg_phiq", [P, N], bf16_),
            ("dbg_phik", [P, N], bf16_),
            ("dbg_vpt", [P, N], bf16_),
            ("dbg_wt", [P, NT, N], bf16_),
            ("dbg_mcat", [P, N], bf16_),
            ("dbg_y", [P, NT, D], bf16_),
            ("dbg_q1", [P, NT, D], f32),
            ("dbg_hf", [P, CT, N + 2], bf16_),
            ("dbg_hcv", [P, CT, N], bf16_),
            ("dbg_y2", [P, NT, D], bf16_),
            ("dbg_q2", [P, NT, D], f32),
        ):
            dbg[nm] = nc.declare_dram_parameter(nm, shp, dt_, isOutput=True)

    with tile.TileContext(nc) as tc:
        pers = tc.alloc_tile_pool(name="pers", bufs=1)
        ps_big = tc.alloc_tile_pool(name="psb", bufs=2, space="PSUM")
        ps_sml = tc.alloc_tile_pool(name="pss", bufs=4, space="PSUM")
        dram = tc.alloc_tile_pool(name="dram", bufs=2, space="DRAM")

        # ---------------- persistent tiles ----------------
        eps_t = pers.tile([P, 1], f32, tag="eps")
        nc.vector.memset(eps_t[:], EPS)
        ones1 = pers.tile([1, P], bf16, tag="ones1")
        nc.vector.memset(ones1[:], 1.0)
        ident = pers.tile([P, P], bf16, tag="ident")
        make_identity(nc, ident[:])

        ids_sb = pers.tile([P, NT], i32, tag="ids")
        nc.sync.dma_start(ids_sb[:], ids_d[:].rearrange("(t p) -> p t", p=P))
        q_res = pers.tile([P, NT, D], f32, tag="qres")
        nc.sync.dma_start(q_res[:], q0_d[:].rearrange("(t p) d -> p t d", p=P))

        def bcast_row(src_d, tag):
            t = pers.tile([P, D], f32, tag=tag)
            nc.sync.dma_start(t[:], src_d[:][None, :].to_broadcast([P, D]))
            return t

        fins_bc = bcast_row(fins_d, "finsbc")
        finb_bc = bcast_row(finb_d, "finbbc")

        # softplus(dt) broadcast to all partitions
        dtv_sb = pers.tile([1, K], f32, tag="dtv")
        nc.sync.dma_start(dtv_sb[:], dtv_d[:])
        spe = pers.tile([1, K], f32, tag="spe")
        nc.scalar.activation(spe[:], dtv_sb[:], AF.Exp)
        nc.vector.tensor_scalar_add(spe[:], spe[:], 1.0)
        nc.scalar.activation(spe[:], spe[:], AF.Ln)
        sp_bc = pers.tile([P, K], f32, tag="spbc")
        nc.gpsimd.partition_broadcast(sp_bc[:], spe[:])

        # per-block norm scale vectors (transposed layouts)
        n1s_sb = pers.tile([P, K, DT], f32, tag="n1s")
        nc.sync.dma_start(n1s_sb[:], n1s_d[:].rearrange("k (dt p) -> p k dt", p=P))
        n2s_sb = pers.tile([P, K, DT], f32, tag="n2s")
        nc.sync.dma_start(n2s_sb[:], n2s_d[:].rearrange("k (dt p) -> p k dt", p=P))

        # weights (loaded f32, cast bf16)
        wq_sb = pers.tile([P, K, DT, P], bf16, tag="wq")
        wk_sb = pers.tile([P, K, DT, P], bf16, tag="wk")
        wv_sb = pers.tile([P, K, DT, P], bf16, tag="wv")
        wo_sb = pers.tile([P, K, D], bf16, tag="wo")
        wup_sb = pers.tile([P, K, DT, JT, P], bf16, tag="wup")
        wdn_sb = pers.tile([P, K, CT, D], bf16, tag="wdn")
        dww_sb = pers.tile([P, K, CT, CK], f32, tag="dww")
        nc.sync.dma_start(dww_sb[:], dww_d[:].rearrange("k (ct p) c -> p k ct c", p=P))
        dwb_sb = pers.tile([P, K, CT], f32, tag="dwb")
        nc.sync.dma_start(dwb_sb[:], dwb_d[:].rearrange("k (ct p) -> p k ct", p=P))
        diag_sb = pers.tile([P, K, CT, CK, P], bf16, tag="diag")
        b1t_sb = pers.tile([P, DT, K], bf16, tag="b1t")
        b2t_sb = pers.tile([P, DT, K], bf16, tag="b2t")
        b2w_sb = pers.tile([P, K, JT], f32, tag="b2w")
        hwt_sb = pers.tile([P, DT, 2], f32, tag="hwt")
        nc.sync.dma_start(hwt_sb[:], hwt_d[:].rearrange("(dt p) c -> p dt c", p=P))
        hb_sb = pers.tile([1, 2], f32, tag="hb")
        nc.sync.dma_start(hb_sb[:], hb_d[:])

        xw_sb = pers.tile([P, K, 3, N], bf16, tag="xw")
        vp_aug = pers.tile([P, NT, 2, 64], bf16, tag="vpaug")
        onescol = pers.tile([P, 1], bf16, tag="onescol")
        nc.vector.memset(onescol[:], 1.0)
        wt_sb = pers.tile([P, NT, 2, 512], bf16, tag="wt")  # both heads, one n-block
        hf_sb = pers.tile([P, CT, N + 2], bf16, tag="hf")
        nc.vector.memset(hf_sb[:], 0.0)
        zft_sb = pers.tile([P, DT, N], bf16, tag="zft")

        # z / zt shared across LN sites (persistent; evals are serial anyway)
        z_sb = pers.tile([P, NT, D], bf16, tag="z")
        zt_sb = pers.tile([P, DT, N], bf16, tag="zt")
        mcat_sb = pers.tile([P, N], bf16, tag="mcat")

        # ---------------- setup (scoped transients) ----------------
        with tc.tile_pool(name="setup", bufs=1) as setup, tc.tile_pool(
            name="setup2", bufs=2
        ) as setup2:
            # cast helper: DMA f32 -> stage, cast to dst (bf16)
            def load_cast(dst_ap, src_ap, shape, tag, eng):
                s = setup.tile(shape, f32, tag="wstage")
                nc.sync.dma_start(s[:], src_ap)
                if eng == 0:
                    nc.vector.tensor_copy(dst_ap, s[:])
                else:
                    nc.scalar.copy(dst_ap, s[:])

            for i, (dst, src) in enumerate(
                ((wq_sb, wq_d), (wk_sb, wk_d), (wv_sb, wv_d))
            ):
                load_cast(
                    dst[:],
                    src[:].rearrange("k (dt p) j -> p k dt j", p=P),
                    [P, K, DT, P],
                    "wstage",
                    i % 2,
                )
            load_cast(
                wo_sb[:],
                wo_d[:].rearrange("k p d -> p k d"),
                [P, K, D],
                "wstage",
                1,
            )
            for k in range(K):
                load_cast(
                    wup_sb[:, k],
                    wup_d[k].rearrange("(dt p) (jt jj) -> p dt jt jj", p=P, jj=P),
                    [P, DT, JT, P],
                    "wstage",
                    k % 2,
                )
                load_cast(
                    wdn_sb[:, k],
                    wdn_d[k].rearrange("(ct p) d -> p ct d", p=P),
                    [P, CT, D],
                    "wstage",
                    (k + 1) % 2,
                )
            for bd, bt in ((n1b_d, b1t_sb), (n2b_d, b2t_sb)):
                bcol = setup2.tile([P, DT, K], f32, tag="bcol")
                for k in range(K):
                    for dtt in range(DT):
                        nc.sync.dma_start(
                            bcol[:, dtt, k : k + 1],
                            bd[k, dtt * P : (dtt + 1) * P][:, None],
                        )
                nc.vector.tensor_copy(bt[:], bcol[:])

            # depthwise conv as matmul: diagonal weight mats per (k, ct, tap)
            for k in range(K):
                for ct in range(CT):
                    for tap in range(CK):
                        nc.vector.tensor_scalar_mul(
                            diag_sb[:, k, ct, tap, :],
                            ident[:],
                            dww_sb[:, k, ct, tap : tap + 1],
                        )

            # ---- embedding + input LN -> X (bf16) -> XT
            ins_bc = setup.tile([P, D], f32, tag="insbc")
            nc.sync.dma_start(ins_bc[:], ins_d[:][None, :].to_broadcast([P, D]))
            inb_bc = setup.tile([P, D], f32, tag="inbbc")
            nc.sync.dma_start(inb_bc[:], inb_d[:][None, :].to_broadcast([P, D]))

            xb_sb = setup.tile([P, NT, D], bf16, tag="xb0")
            st = setup.tile([P, NT, 6], f32, tag="st0")
            mv = setup.tile([P, NT, 2], f32, tag="mv0")
            rstd = setup.tile([P, NT], f32, tag="rstd0")
            for t in range(NT):
                xrt = setup2.tile([P, D], f32, tag="xrt")
                nc.gpsimd.indirect_dma_start(
                    out=xrt[:],
                    out_offset=None,
                    in_=emb_d[:],
                    in_offset=bass.IndirectOffsetOnAxis(ap=ids_sb[:, t : t + 1], axis=0),
                )
                ps = setup2.tile([P, D], f32, tag="posst")
                nc.sync.dma_start(
                    ps[:], pos_d[:].rearrange("(t p) d -> p t d", p=P)[:, t, :]
                )
                nc.vector.tensor_tensor(xrt[:], xrt[:], ps[:], op=OP.add)
                nc.vector.bn_stats(st[:, t], xrt[:])
                nc.vector.bn_aggr(mv[:, t], st[:, t])
                nc.scalar.activation(
                    rstd[:, t : t + 1], mv[:, t, 1:2], AF.Ln, bias=eps_t[:]
                )
                nc.scalar.activation(
                    rstd[:, t : t + 1], rstd[:, t : t + 1], AF.Exp, scale=-0.5
                )
                nc.vector.tensor_scalar(
                    xb_sb[:, t],
                    xrt[:],
                    mv[:, t, 0:1],
                    rstd[:, t : t + 1],
                    op0=OP.subtract,
                    op1=OP.mult,
                )
                nc.vector.tensor_tensor(xb_sb[:, t], xb_sb[:, t], ins_bc[:], op=OP.mult)
                nc.vector.tensor_tensor(xb_sb[:, t], xb_sb[:, t], inb_bc[:], op=OP.add)
            if DEBUG:
                nc.sync.dma_start(dbg["dbg_xb"][:], xb_sb[:])
            xt_sb = setup.tile([P, DT, N], bf16, tag="xt")
            for t in range(NT):
                for dtt in range(DT):
                    nc.sync.dma_start_transpose(
                        xt_sb[:, dtt, t * P : (t + 1) * P],
                        xb_sb[:, t, dtt * P : (dtt + 1) * P],
                    )

            # ---- XW = (b1 + X) @ W  (raw weights, before s1 fold)
            for k in range(K):
                for pi, wsb in enumerate((wq_sb, wk_sb, wv_sb)):
                    pb = ps_sml.tile([P, 512], f32, tag="sml")
                    for dtt in range(DT):
                        nc.tensor.matmul(
                            pb[:, 0:1],
                            lhsT=wsb[:, k, dtt, :],
                            rhs=b1t_sb[:, dtt, k : k + 1],
                            start=(dtt == 0),
                            stop=(dtt == DT - 1),
                        )
                    b1w = setup2.tile([P, 1], f32, tag="b1w")
                    nc.vector.tensor_copy(b1w[:], pb[:, 0:1])
                    for nb in range(2):
                        pp = ps_sml.tile([P, 512], f32, tag="sml")
                        for dtt in range(DT):
                            nc.tensor.matmul(
                                pp[:],
                                lhsT=wsb[:, k, dtt, :],
                                rhs=xt_sb[:, dtt, nb * 512 : (nb + 1) * 512],
                                start=(dtt == 0),
                                stop=(dtt == DT - 1),
                            )
                        nc.vector.tensor_scalar(
                            xw_sb[:, k, pi, nb * 512 : (nb + 1) * 512],
                            pp[:],
                            b1w[:],
                            None,
                            op0=OP.add,
                        )

            # ---- b2W per (k, jt) (raw wup)
            for k in range(K):
                for jt in range(JT):
                    pb = ps_sml.tile([P, 512], f32, tag="sml")
                    for dtt in range(DT):
                        nc.tensor.matmul(
                            pb[:, 0:1],
                            lhsT=wup_sb[:, k, dtt, jt, :],
                            rhs=b2t_sb[:, dtt, k : k + 1],
                            start=(dtt == 0),
                            stop=(dtt == DT - 1),
                        )
                    eng = nc.vector if (k + jt) % 2 == 0 else nc.scalar
                    if eng is nc.vector:
                        nc.vector.tensor_copy(b2w_sb[:, k, jt : jt + 1], pb[:, 0:1])
                    else:
                        nc.scalar.copy(b2w_sb[:, k, jt : jt + 1], pb[:, 0:1])

            # ---- fold s1 into wq/wk/wv, s2 into wup, softplus(dt) into wo
            for wsb in (wq_sb, wk_sb, wv_sb):
                for k in range(K):
                    nc.vector.tensor_tensor(
                        wsb[:, k],
                        wsb[:, k],
                        n1s_sb[:, k, :, None].to_broadcast([P, DT, P]),
                        op=OP.mult,
                    )
            for k in range(K):
                nc.vector.tensor_tensor(
                    wup_sb[:, k],
                    wup_sb[:, k],
                    n2s_sb[:, k, :, None, None].to_broadcast([P, DT, JT, P]),
                    op=OP.mult,
                )
                nc.vector.tensor_tensor(
                    wo_sb[:, k],
                    wo_sb[:, k],
                    sp_bc[:, k : k + 1].to_broadcast([P, D]),
                    op=OP.mult,
                )

        # ---------------- block evals ----------------
        with tc.tile_pool(name="work", bufs=1) as work, tc.tile_pool(
            name="work2", bufs=2
        ) as work2:

            def layernorm(src, t_lo, t_hi, out, scale_bc=None, bias_bc=None):
                """LN over d for token tiles [t_lo, t_hi) of src [P,NT,D] f32.
                Writes (x-mu)*rstd (optionally *scale+bias) to out (bf16)."""
                nt = t_hi - t_lo
                st = work.tile([P, NT, 6], f32, tag="st")
                mv = work.tile([P, NT, 2], f32, tag="mv")
                for t in range(t_lo, t_hi):
                    nc.vector.bn_stats(st[:, t], src[:, t])
                    nc.vector.bn_aggr(mv[:, t], st[:, t])
                rstd = work.tile([P, NT], f32, tag="rstd")
                nc.scalar.activation(
                    rstd[:, t_lo:t_hi], mv[:, t_lo:t_hi, 1], AF.Ln, bias=eps_t[:]
                )
                nc.scalar.activation(
                    rstd[:, t_lo:t_hi], rstd[:, t_lo:t_hi], AF.Exp, scale=-0.5
                )
                nmr = work.tile([P, NT], f32, tag="nmr")
                nc.vector.tensor_tensor(
                    nmr[:, t_lo:t_hi], mv[:, t_lo:t_hi, 0], rstd[:, t_lo:t_hi], op=OP.mult
                )
                nc.vector.tensor_scalar_mul(nmr[:, t_lo:t_hi], nmr[:, t_lo:t_hi], -1.0)
                for t in range(t_lo, t_hi):
                    if t % 2 == 0:
                        nc.vector.tensor_scalar(
                            out[:, t],
                            src[:, t],
                            mv[:, t, 0:1],
                            rstd[:, t : t + 1],
                            op0=OP.subtract,
                            op1=OP.mult,
                        )
                    else:
                        nc.scalar.activation(
                            out[:, t],
                            src[:, t],
                            AF.Identity,
                            bias=nmr[:, t : t + 1],
                            scale=rstd[:, t : t + 1],
                        )

            def transpose_tiles(src, dst, t_lo, t_hi):
                for t in range(t_lo, t_hi):
                    for dtt in range(DT):
                        eng = nc.sync if (t + dtt) % 2 == 0 else nc.scalar
                        eng.dma_start_transpose(
                            dst[:, dtt, t * P : (t + 1) * P],
                            src[:, t, dtt * P : (dtt + 1) * P],
                        )

            def all_reduce_chunked(y, scaled_add_dst):
                """AllReduce y [P,NT,D] bf16 in NCHUNK token chunks; add into
                scaled_add_dst (q_res) in place."""
                for c in range(NCHUNK):
                    t0, t1 = c * TC_PER_CHUNK, (c + 1) * TC_PER_CHUNK
                    yb = dram.tile([P, TC_PER_CHUNK, D], bf16, tag="arin")
                    ab = dram.tile([P, TC_PER_CHUNK, D], bf16, tag="arout")
                    nc.sync.dma_start(yb[:], y[:, t0:t1])
                    if SKIP_COLLECTIVE:
                        nc.gpsimd.dma_start(ab[:], yb[:])
                    else:
                        nc.gpsimd.collective_compute(
                            "AllReduce",
                            OP.add,
                            replica_groups=RG,
                            ins=[yb[:].opt()],
                            outs=[ab[:].opt()],
                        )
                    nc.gpsimd.dma_start(
                        scaled_add_dst[:, t0:t1],
                        ab[:],
                        accum_op=OP.add,
                    )

            def block_eval(k, dump=False):
                # ---- LN1 -> z -> zT
                layernorm(q_res, 0, NT, z_sb)
                transpose_tiles(z_sb, zt_sb, 0, NT)
                if dump:
                    nc.sync.dma_start(dbg["dbg_z"][:], z_sb[:])
                    nc.sync.dma_start(dbg["dbg_zt"][:], zt_sb[:])

                # ---- projections (q, k, v)
                phi = [None, None, None]
                for pi, wsb in enumerate((wq_sb, wk_sb, wv_sb)):
                    pp = ps_big.tile([P, 1024], f32, tag="big")
                    for nb in range(2):
                        for dtt in range(DT):
                            nc.tensor.matmul(
                                pp[:, nb * 512 : (nb + 1) * 512],
                                lhsT=wsb[:, k, dtt, :],
                                rhs=zt_sb[:, dtt, nb * 512 : (nb + 1) * 512],
                                start=(dtt == 0),
                                stop=(dtt == DT - 1),
                            )
                    tag = ("phiq", "phik", "vpt")[pi]
                    res = work.tile([P, N], bf16, tag=tag)
                    if pi < 2:
                        ts_t = work2.tile([P, N], bf16, tag="tsum")
                        nc.vector.tensor_tensor(
                            ts_t[:], pp[:], xw_sb[:, k, pi, :], op=OP.add
                        )
                        rel = work.tile([P, N], bf16, tag="rel")
                        nc.vector.tensor_scalar_max(rel[:], ts_t[:], 0.0)
                        mn = work.tile([P, N], bf16, tag="mn")
                        nc.vector.tensor_scalar_min(mn[:], ts_t[:], 0.0)
                        ex = work.tile([P, N], bf16, tag="ex")
                        nc.scalar.activation(ex[:], mn[:], AF.Exp)
                        nc.vector.tensor_tensor(res[:], ex[:], rel[:], op=OP.add)
                    else:
                        nc.vector.tensor_tensor(
                            res[:], pp[:], xw_sb[:, k, pi, :], op=OP.add
                        )
                        for mt in range(NT):
                            for h in range(2):
                                eng = nc.sync if mt % 2 == 0 else nc.scalar
                                eng.dma_start_transpose(
                                    vp_aug[:, mt, h, 0:64],
                                    res[h * 64 : (h + 1) * 64, mt * P : (mt + 1) * P],
                                )
                    phi[pi] = res
                phiq, phik, vpt = phi
                if dump:
                    nc.sync.dma_start(dbg["dbg_phiq"][:], phiq[:])
                    nc.sync.dma_start(dbg["dbg_phik"][:], phik[:])
                    nc.sync.dma_start(dbg["dbg_vpt"][:], vpt[:])

                # ---- attention: nb-outer, heads packed via tile_position
                for nb in range(2):
                    nsl = slice(nb * 512, (nb + 1) * 512)
                    for mt in range(NT):
                        for h in range(2):
                            hs = h * 64
                            pw = ps_sml.tile([P, 512], f32, tag="sml")
                            nc.tensor.matmul(
                                pw[:],
                                lhsT=phik[hs : hs + 64, mt * P : (mt + 1) * P],
                                rhs=phiq[hs : hs + 64, nsl],
                                start=True,
                                stop=True,
                            )
                            dst = wt_sb[:, mt, h, :]
                            if (mt + h) % 2 == 0:
                                nc.scalar.activation(dst, pw[:], AF.Square)
                            else:
                                wc = work2.tile([P, 512], bf16, tag="wc")
                                nc.vector.tensor_copy(wc[:], pw[:])
                                nc.vector.tensor_tensor(dst, wc[:], wc[:], op=OP.mult)
                    pa = ps_sml.tile([P, 512], f32, tag="sml")
                    pss0 = ps_sml.tile([P, 512], f32, tag="sml")
                    pss1 = ps_sml.tile([P, 512], f32, tag="sml")
                    for mt in range(NT):
                        for h in range(2):
                            hs = h * 64
                            nc.tensor.matmul(
                                pa[hs : hs + 64, :],
                                lhsT=vp_aug[:, mt, h, :],
                                rhs=wt_sb[:, mt, h, :],
                                start=(mt == 0),
                                stop=(mt == NT - 1),
                                tile_position=(0, hs),
                                skip_group_check=True,
                            )
                            nc.tensor.matmul(
                                (pss0 if h == 0 else pss1)[0:1, :],
                                lhsT=onescol[:],
                                rhs=wt_sb[:, mt, h, :],
                                start=(mt == 0),
                                stop=(mt == NT - 1),
                            )
                    prr = work.tile([P, 512], bf16, tag="prr")
                    for h in range(2):
                        hs = h * 64
                        pss = pss0 if h == 0 else pss1
                        rr = work.tile([1, 512], f32, tag="rr")
                        nc.vector.tensor_scalar_add(rr[:], pss[0:1, :], 1.0)
                        nc.vector.reciprocal_approx_fast(rr[:], rr[:])
                        rrb = work.tile([1, 512], bf16, tag="rrb")
                        nc.vector.tensor_copy(rrb[:], rr[:])
                        rsc = dram.tile([1, 512], bf16, tag="rsc")
                        nc.sync.dma_start(rsc[:], rrb[:])
                        nc.scalar.dma_start(
                            prr[hs : hs + 64, :],
                            rsc[0][None, :].to_broadcast([64, 512]),
                        )
                    at = work.tile([P, 512], bf16, tag="atr")
                    if nb == 0:
                        nc.vector.tensor_copy(at[:], pa[:])
                    else:
                        nc.scalar.copy(at[:], pa[:])
                    tm = work.tile([P, 512], bf16, tag="tm")
                    nc.vector.tensor_tensor(tm[:], at[:], prr[:], op=OP.mult)
                    nc.gpsimd.tensor_tensor(
                        mcat_sb[:, nsl], tm[:], vpt[:, nsl], op=OP.subtract
                    )

                # ---- out-proj (wo pre-scaled by softplus(dt)) + chunked AR
                y = work.tile([P, NT, D], bf16, tag="y")
                for nt in range(NT):
                    po = ps_sml.tile([P, 512], f32, tag="sml")
                    nc.tensor.matmul(
                        po[:],
                        lhsT=mcat_sb[:, nt * P : (nt + 1) * P],
                        rhs=wo_sb[:, k, :],
                        start=True,
                        stop=True,
                    )
                    if nt % 2 == 0:
                        nc.vector.tensor_copy(y[:, nt], po[:])
                    else:
                        nc.scalar.copy(y[:, nt], po[:])
                if dump:
                    nc.sync.dma_start(dbg["dbg_mcat"][:], mcat_sb[:])
                    nc.sync.dma_start(dbg["dbg_y"][:], y[:])
                all_reduce_chunked(y, q_res)
                if dump:
                    nc.sync.dma_start(dbg["dbg_q1"][:], q_res[:])

                # ---- LN2 -> z2 -> z2T
                layernorm(q_res, 0, NT, z_sb)
                transpose_tiles(z_sb, zt_sb, 0, NT)

                # ---- up-proj + SwiGLU -> hf
                for nb in range(2):
                    for jp in range(CT):
                        pg = ps_sml.tile([P, 512], f32, tag="sml")
                        for dtt in range(DT):
                            nc.tensor.matmul(
                                pg[:],
                                lhsT=wup_sb[:, k, dtt, jp, :],
                                rhs=zt_sb[:, dtt, nb * 512 : (nb + 1) * 512],
                                start=(dtt == 0),
                                stop=(dtt == DT - 1),
                            )
                        pu = ps_sml.tile([P, 512], f32, tag="sml")
                        for dtt in range(DT):
                            nc.tensor.matmul(
                                pu[:],
                                lhsT=wup_sb[:, k, dtt, jp + CT, :],
                                rhs=zt_sb[:, dtt, nb * 512 : (nb + 1) * 512],
                                start=(dtt == 0),
                                stop=(dtt == DT - 1),
                            )
                        sg = work2.tile([P, 512], bf16, tag="sg")
                        nc.scalar.activation(
                            sg[:], pg[:], AF.Silu, bias=b2w_sb[:, k, jp : jp + 1]
                        )
                        uu = work2.tile([P, 512], bf16, tag="uu")
                        nc.vector.tensor_scalar(
                            uu[:], pu[:], b2w_sb[:, k, jp + CT : jp + CT + 1], None, op0=OP.add
                        )
                        nc.gpsimd.tensor_tensor(
                            hf_sb[:, jp, 1 + nb * 512 : 1 + (nb + 1) * 512],
                            sg[:],
                            uu[:],
                            op=OP.mult,
                        )

                # ---- depthwise conv (as 3 diag matmuls) + silu -> hcv
                hcv = work.tile([P, CT, N], bf16, tag="hcv")
                for ct in range(CT):
                    for nb in range(2):
                        pc = ps_sml.tile([P, 512], f32, tag="sml")
                        for tap in range(CK):
                            nc.tensor.matmul(
                                pc[:],
                                lhsT=diag_sb[:, k, ct, tap, :],
                                rhs=hf_sb[:, ct, nb * 512 + tap : nb * 512 + tap + 512],
                                start=(tap == 0),
                                stop=(tap == CK - 1),
                            )
                        nc.scalar.activation(
                            hcv[:, ct, nb * 512 : (nb + 1) * 512],
                            pc[:],
                            AF.Silu,
                            bias=dwb_sb[:, k, ct : ct + 1],
                        )

                if dump:
                    nc.sync.dma_start(dbg["dbg_hf"][:], hf_sb[:])
                    nc.sync.dma_start(dbg["dbg_hcv"][:], hcv[:])
                # ---- down-proj + chunked AR
                y2 = work.tile([P, NT, D], bf16, tag="y")
                for nt in range(NT):
                    pd = ps_sml.tile([P, 512], f32, tag="sml")
                    for ct in range(CT):
                        nc.tensor.matmul(
                            pd[:],
                            lhsT=hcv[:, ct, nt * P : (nt + 1) * P],
                            rhs=wdn_sb[:, k, ct, :],
                            start=(ct == 0),
                            stop=(ct == CT - 1),
                        )
                    if nt % 2 == 0:
                        nc.vector.tensor_copy(y2[:, nt], pd[:])
                    else:
                        nc.scalar.copy(y2[:, nt], pd[:])
                if dump:
                    nc.sync.dma_start(dbg["dbg_y2"][:], y2[:])
                all_reduce_chunked(y2, q_res)
                if dump:
                    nc.sync.dma_start(dbg["dbg_q2"][:], q_res[:])

            for _cyc in range(CYCLES):
                for k in range(K):
                    block_eval(k, dump=(DEBUG and _cyc == 0 and k == 0))

            # ---------------- final LN (with fin scale/bias) ----------------
            layernorm(q_res, 0, NT, z_sb)
            nc.vector.tensor_tensor(
                z_sb[:], z_sb[:], fins_bc[:, None, :].to_broadcast([P, NT, D]), op=OP.mult
            )
            nc.vector.tensor_tensor(
                z_sb[:], z_sb[:], finb_bc[:, None, :].to_broadcast([P, NT, D]), op=OP.add
            )
            transpose_tiles(z_sb, zft_sb, 0, NT)

            # ---- q_logits = mean_n(Qn) @ halt_w.T + halt_b
            qm = work.tile([P, DT], f32, tag="qm")
            nc.vector.reduce_sum(qm[:], zft_sb[:], axis=mybir.AxisListType.X)
            pq = ps_sml.tile([P, 512], f32, tag="sml")
            for dtt in range(DT):
                nc.tensor.matmul(
                    pq[0:1, 0:2],
                    lhsT=qm[:, dtt : dtt + 1],
                    rhs=hwt_sb[:, dtt, :],
                    start=(dtt == 0),
                    stop=(dtt == DT - 1),
                )
            ql = work.tile([1, 2], f32, tag="ql")
            nc.vector.tensor_scalar_mul(ql[:], pq[0:1, 0:2], 1.0 / N)
            nc.vector.tensor_tensor(ql[:], ql[:], hb_sb[:], op=OP.add)
            nc.sync.dma_start(qlog_d[:], ql[:])

        # ---------------- lm head (vocab-sharded) ----------------
        with tc.tile_pool(name="lmp", bufs=2) as lmp, tc.tile_pool(
            name="lmp1", bufs=1
        ) as lmp1:
            lg = logits_d[:].rearrange("(nt p) v -> p nt v", p=P)
            lmsrc = lmt_d[:].rearrange("(dt p) v -> p dt v", p=P)
            for vc in range(NVC):
                stage = lmp.tile([P, DT, VCH], f32, tag="lstage")
                nc.sync.dma_start(stage[:], lmsrc[:, :, vc * VCH : (vc + 1) * VCH])
                lc = lmp.tile([P, DT, VCH], bf16, tag="lc")
                if vc % 2 == 0:
                    nc.vector.tensor_copy(lc[:], stage[:])
                else:
                    nc.scalar.copy(lc[:], stage[:])
                ob = lmp1.tile([P, NT, VCH], f32, tag="ob")
                for nt in range(NT):
                    pl = ps_sml.tile([P, 512], f32, tag="sml")
                    for dtt in range(DT):
                        nc.tensor.matmul(
                            pl[:, 0:VCH],
                            lhsT=zft_sb[:, dtt, nt * P : (nt + 1) * P],
                            rhs=lc[:, dtt, :],
                            start=(dtt == 0),
                            stop=(dtt == DT - 1),
                        )
                    if (vc + nt) % 2 == 0:
                        nc.vector.tensor_copy(ob[:, nt, :], pl[:, 0:VCH])
                    else:
                        nc.scalar.copy(ob[:, nt, :], pl[:, 0:VCH])
                eng = nc.sync if vc % 2 == 0 else nc.scalar
                eng.dma_start(lg[:, :, vc * VCH : (vc + 1) * VCH], ob[:])

        dram.release()
        ps_sml.release()
        ps_big.release()
        pers.release()

    nc.compile()
    return nc


def _get_nc():
    if "nc" not in _CACHE:
        _CACHE["nc"] = _build()
    return _CACHE["nc"]


def _prep_in_maps(inputs):
    ii = {k: np.asarray(v) for k, v in inputs.items()}
    hm = ii["carry_halted"].astype(bool)
    ids = np.where(hm[:, None], ii["inputs"], ii["carry_inputs"]).astype(np.int32)
    init_h = ii["init_hidden"].astype(np.float32)
    q0 = np.where(
        hm[:, None, None],
        np.broadcast_to(init_h[None, None, :], (B, N, D)),
        ii["carry_hidden"].astype(np.float32),
    ).astype(np.float32)
    emb = np.ascontiguousarray(ii["emb"].astype(np.float32))
    posn = np.ascontiguousarray(ii["pos"].astype(np.float32)[:N])
    lmT = np.ascontiguousarray(ii["lm_w"].astype(np.float32).T)  # [D, V]
    hwT = np.ascontiguousarray(ii["halt_w"].astype(np.float32).T)  # [D, 2]
    hb = ii["halt_b"].astype(np.float32).reshape(1, 2)
    dtv = ii["dt"].astype(np.float32).reshape(1, K)
    wq = ii["W_Q"].astype(np.float32)
    wk = ii["W_K"].astype(np.float32)
    wv = ii["W_V"].astype(np.float32)
    wo = ii["W_O"].astype(np.float32)
    wup = ii["W_up"].astype(np.float32)
    dww = ii["dw_w"].astype(np.float32)[:, :, 0, :]  # [K, INNER, CK]
    dwb = ii["dw_b"].astype(np.float32)
    wdn = ii["W_down"].astype(np.float32)

    in_maps = []
    for c in range(8):
        b, g = c // GRP, c % GRP
        jlo = g * P  # head-col slice (2 heads x 64)
        clo = g * CT * P  # inner slice (384)
        m = {
            "ids": np.ascontiguousarray(ids[b]),
            "q0": np.ascontiguousarray(q0[b]),
            "emb": emb,
            "posn": posn,
            "ins_v": ii["in_s"].astype(np.float32),
            "inb_v": ii["in_b"].astype(np.float32),
            "fins_v": ii["fin_s"].astype(np.float32),
            "finb_v": ii["fin_b"].astype(np.float32),
            "dtv": dtv,
            "wq": np.ascontiguousarray(wq[:, :, jlo : jlo + P]),
            "wk": np.ascontiguousarray(wk[:, :, jlo : jlo + P]),
            "wv": np.ascontiguousarray(wv[:, :, jlo : jlo + P]),
            "wo": np.ascontiguousarray(wo[:, jlo : jlo + P, :]),
            "wup": np.ascontiguousarray(
                np.concatenate(
                    (
                        wup[:, :, clo : clo + CT * P],
                        wup[:, :, INNER + clo : INNER + clo + CT * P],
                    ),
                    axis=-1,
                )
            ),
            "dww": np.ascontiguousarray(dww[:, clo : clo + CT * P, :]),
            "dwb": np.ascontiguousarray(dwb[:, clo : clo + CT * P]),
            "wdn": np.ascontiguousarray(wdn[:, clo : clo + CT * P, :]),
            "n1s": ii["n1_s"].astype(np.float32),
            "n1b": ii["n1_b"].astype(np.float32),
            "n2s": ii["n2_s"].astype(np.float32),
            "n2b": ii["n2_b"].astype(np.float32),
            "lmt": np.ascontiguousarray(lmT[:, g * VC : (g + 1) * VC]),
            "hwt": hwT,
            "hb": hb,
        }
        in_maps.append(m)
    return in_maps


def _run_fast(nc, in_maps):
    """Cached jitted executor (avoids per-call jit retrace). Falls back to
    run_bass_kernel_spmd on any failure."""
    import jax
    import jax.numpy as jnp
    import concourse.mybir as mybir
    from jax.sharding import Mesh, PartitionSpec, NamedSharding
    from jax.experimental.shard_map import shard_map
    from concourse.bass2jax import (
        _bass_exec_p,
        partition_id_tensor,
        install_neuronx_cc_hook,
    )

    if "fast" not in _CACHE:
        install_neuronx_cc_hook()
        partition_name = (
            nc.partition_id_tensor.name if nc.partition_id_tensor else None
        )
        in_names, out_names, out_avals = [], [], []
        for alloc in nc.m.functions[0].allocations:
            if not isinstance(alloc, mybir.MemoryLocationSet):
                continue
            name = alloc.memorylocations[0].name
            if alloc.kind == "ExternalInput":
                if name != partition_name:
                    in_names.append(name)
            elif alloc.kind == "ExternalOutput":
                out_names.append(name)
                out_avals.append(
                    jax.core.ShapedArray(
                        tuple(alloc.tensor_shape), mybir.dt.np(alloc.dtype)
                    )
                )
        n_params = len(in_names)
        all_in = in_names + out_names + ([partition_name] if partition_name else [])

        def _body(*args):
            ins = list(args[:n_params])
            outs = list(args[n_params:])
            pid = [partition_id_tensor()] if partition_name else []
            return tuple(
                _bass_exec_p.bind(
                    *ins,
                    *outs,
                    *pid,
                    out_avals=tuple(out_avals),
                    in_names=tuple(all_in),
                    out_names=tuple(out_names),
                    lowering_input_output_aliases=(),
                    sim_require_finite=True,
                    sim_require_nnan=True,
                    nc=nc,
                )
            )

        devices = jax.devices()[:8]
        mesh = Mesh(np.asarray(devices), ("core",))
        n_outs = len(out_names)
        f = jax.jit(
            shard_map(
                _body,
                mesh=mesh,
                in_specs=(PartitionSpec("core"),) * (n_params + n_outs),
                out_specs=(PartitionSpec("core"),) * n_outs,
                check_rep=False,
            ),
            donate_argnums=tuple(range(n_params, n_params + n_outs)),
            keep_unused=True,
        )
        _CACHE["fast"] = (f, in_names, out_names, out_avals, mesh)
    f, in_names, out_names, out_avals, mesh = _CACHE["fast"]
    import jax

    sh = jax.sharding.NamedSharding(mesh, PartitionSpec("core"))
    concat_in = [
        np.concatenate([np.asarray(in_maps[c][nm]) for c in range(8)], axis=0)
        for nm in in_names
    ]
    dev_in = [jax.device_put(a, sh) for a in concat_in]
    zeros = [
        jax.device_put(np.zeros((av.shape[0] * 8,) + av.shape[1:], av.dtype), sh)
        for av in out_avals
    ]
    outs = f(*dev_in, *zeros)
    jax.block_until_ready(outs)
    res = []
    for c in range(8):
        m = {}
        for i, nm in enumerate(out_names):
            av = out_avals[i]
            m[nm] = np.asarray(outs[i])[c * av.shape[0] : (c + 1) * av.shape[0]]
        res.append(m)
    return res


def kernel(**inputs):
    from concourse.bass_utils import run_bass_kernel_spmd

    nc = _get_nc()
    in_maps = _prep_in_maps(inputs)
    try:
        res = _run_fast(nc, in_maps)
    except Exception:
        res = run_bass_kernel_spmd(nc, in_maps, core_ids=list(range(8))).results
    logits = np.zeros((B, N, V), np.float32)
    for c in range(8):
        b, g = c // GRP, c % GRP
        logits[b, :, g * VC : (g + 1) * VC] = res[c]["logits"]
    q_logits = np.stack([res[0]["qlog"][0], res[GRP]["qlog"][0]])
    return logits, q_logits


# revision 21
# speedup vs baseline: 2.8913x; 1.0613x over previous
"""Trainium2 Bass kernel for nn_AMKPDModel (linear-attention transformer,
K=4 blocks x 2 cycles, ConvSwiGLU FFN, 32k-vocab LM head) on 8 NeuronCores.

Sharding: 2 data-parallel groups of 4 cores (one per batch element).
Within a group: attention heads sharded 2/core, FFN inner dim sharded
384/core, lm_head vocab sharded 8000/core. Two group-local AllReduces
per block eval ([1024,512] bf16), chunked for compute/comm overlap.
"""

import sys

if "/opt/trn_rl_repo" not in sys.path:
    sys.path.insert(0, "/opt/trn_rl_repo")

import numpy as np

# model dims
B, N, D = 2, 1024, 512
K = 4
V = 32000
INNER = 1536
CK = 3
EPS = 1e-5
CYCLES = 2  # H_CYCLES runs of the 4-block stack

# sharding
GRP = 4            # cores per batch group
P = 128
NT = N // P        # 8 token tiles
DT = D // P        # 4 feature tiles
JT = 6             # up-proj 768/128 local tiles (3 G + 3 U)
CT = 3             # local inner tiles (384/128)
VC = V // GRP      # 8000 vocab rows per core
VCH = 500
NVC = VC // VCH    # 16 lm chunks
NCHUNK = 2         # AR chunks per block output (token-split)
TC_PER_CHUNK = NT // NCHUNK

RG = [[0, 1, 2, 3], [4, 5, 6, 7]]

_CACHE = {}
DEBUG = False
SKIP_COLLECTIVE = False  # timing-only: replace AR with local copy


def _build():
    import concourse.bass as bass
    import concourse.mybir as mybir
    import concourse.tile as tile
    from concourse import bacc
    from concourse.masks import make_identity

    f32 = mybir.dt.float32
    bf16 = mybir.dt.bfloat16
    i32 = mybir.dt.int32
    AF = mybir.ActivationFunctionType
    OP = mybir.AluOpType

    nc = bacc.Bacc(None, target_bir_lowering=False, debug=False, num_devices=8)

    # ---------------- DRAM params ----------------
    ids_d = nc.declare_dram_parameter("ids", [N], i32, isOutput=False)
    q0_d = nc.declare_dram_parameter("q0", [N, D], f32, isOutput=False)
    emb_d = nc.declare_dram_parameter("emb", [V, D], f32, isOutput=False)
    pos_d = nc.declare_dram_parameter("posn", [N, D], f32, isOutput=False)
    ins_d = nc.declare_dram_parameter("ins_v", [D], f32, isOutput=False)
    inb_d = nc.declare_dram_parameter("inb_v", [D], f32, isOutput=False)
    fins_d = nc.declare_dram_parameter("fins_v", [D], f32, isOutput=False)
    finb_d = nc.declare_dram_parameter("finb_v", [D], f32, isOutput=False)
    dtv_d = nc.declare_dram_parameter("dtv", [1, K], f32, isOutput=False)
    wq_d = nc.declare_dram_parameter("wq", [K, D, P], f32, isOutput=False)
    wk_d = nc.declare_dram_parameter("wk", [K, D, P], f32, isOutput=False)
    wv_d = nc.declare_dram_parameter("wv", [K, D, P], f32, isOutput=False)
    wo_d = nc.declare_dram_parameter("wo", [K, P, D], f32, isOutput=False)
    wup_d = nc.declare_dram_parameter("wup", [K, D, JT * P], f32, isOutput=False)
    dww_d = nc.declare_dram_parameter("dww", [K, CT * P, CK], f32, isOutput=False)
    dwb_d = nc.declare_dram_parameter("dwb", [K, CT * P], f32, isOutput=False)
    wdn_d = nc.declare_dram_parameter("wdn", [K, CT * P, D], f32, isOutput=False)
    n1s_d = nc.declare_dram_parameter("n1s", [K, D], f32, isOutput=False)
    n1b_d = nc.declare_dram_parameter("n1b", [K, D], f32, isOutput=False)
    n2s_d = nc.declare_dram_parameter("n2s", [K, D], f32, isOutput=False)
    n2b_d = nc.declare_dram_parameter("n2b", [K, D], f32, isOutput=False)
    lmt_d = nc.declare_dram_parameter("lmt", [D, VC], f32, isOutput=False)
    hwt_d = nc.declare_dram_parameter("hwt", [D, 2], f32, isOutput=False)
    hb_d = nc.declare_dram_parameter("hb", [1, 2], f32, isOutput=False)

    logits_d = nc.declare_dram_parameter("logits", [N, VC], f32, isOutput=True)
    qlog_d = nc.declare_dram_parameter("qlog", [1, 2], f32, isOutput=True)
    dbg = {}
    if DEBUG:
        bf16_ = mybir.dt.bfloat16
        for nm, shp, dt_ in (
            ("dbg_xb", [P, NT, D], bf16_),
            ("dbg_z", [P, NT, D], bf16_),
            ("dbg_zt", [P, DT, N], bf16_),
            ("dbg_phiq", [P, N], bf16_),
            ("dbg_phik", [P, N], bf16_),
            ("dbg_vpt", [P, N], bf16_),
            ("dbg_wt", [P, NT, N], bf16_),
            ("dbg_mcat", [P, N], bf16_),
            ("dbg_y", [P, NT, D], bf16_),
            ("dbg_q1", [P, NT, D], f32),
            ("dbg_hf", [P, CT, N + 2], bf16_),
            ("dbg_hcv", [P, CT, N], bf16_),
            ("dbg_y2", [P, NT, D], bf16_),
            ("dbg_q2", [P, NT, D], f32),
        ):
            dbg[nm] = nc.declare_dram_parameter(nm, shp, dt_, isOutput=True)

    with tile.TileContext(nc) as tc:
        pers = tc.alloc_tile_pool(name="pers", bufs=1)
        ps_big = tc.alloc_tile_pool(name="psb", bufs=1, space="PSUM")
        ps_sml = tc.alloc_tile_pool(name="pss", bufs=6, space="PSUM")
        dram = tc.alloc_tile_pool(name="dram", bufs=2, space="DRAM")

        # ---------------- persistent tiles ----------------
        eps_t = pers.tile([P, 1], f32, tag="eps")
        nc.vector.memset(eps_t[:], EPS)
        ones1 = pers.tile([1, P], bf16, tag="ones1")
        nc.vector.memset(ones1[:], 1.0)
        ident = pers.tile([P, P], bf16, tag="ident")
        make_identity(nc, ident[:])

        ids_sb = pers.tile([P, NT], i32, tag="ids")
        nc.sync.dma_start(ids_sb[:], ids_d[:].rearrange("(t p) -> p t", p=P))
        q_res = pers.tile([P, NT, D], f32, tag="qres")
        nc.sync.dma_start(q_res[:], q0_d[:].rearrange("(t p) d -> p t d", p=P))

        def bcast_row(src_d, tag):
            t = pers.tile([P, D], f32, tag=tag)
            nc.sync.dma_start(t[:], src_d[:][None, :].to_broadcast([P, D]))
            return t

        fins_bc = bcast_row(fins_d, "finsbc")
        finb_bc = bcast_row(finb_d, "finbbc")

        # softplus(dt) broadcast to all partitions
        dtv_sb = pers.tile([1, K], f32, tag="dtv")
        nc.sync.dma_start(dtv_sb[:], dtv_d[:])
        spe = pers.tile([1, K], f32, tag="spe")
        nc.scalar.activation(spe[:], dtv_sb[:], AF.Exp)
        nc.vector.tensor_scalar_add(spe[:], spe[:], 1.0)
        nc.scalar.activation(spe[:], spe[:], AF.Ln)
        sp_bc = pers.tile([P, K], f32, tag="spbc")
        nc.gpsimd.partition_broadcast(sp_bc[:], spe[:])

        # per-block norm scale vectors (transposed layouts)
        n1s_sb = pers.tile([P, K, DT], f32, tag="n1s")
        nc.sync.dma_start(n1s_sb[:], n1s_d[:].rearrange("k (dt p) -> p k dt", p=P))
        n2s_sb = pers.tile([P, K, DT], f32, tag="n2s")
        nc.sync.dma_start(n2s_sb[:], n2s_d[:].rearrange("k (dt p) -> p k dt", p=P))

        # weights (loaded f32, cast bf16)
        wq_sb = pers.tile([P, K, DT, P], bf16, tag="wq")
        wk_sb = pers.tile([P, K, DT, P], bf16, tag="wk")
        wv_sb = pers.tile([P, K, DT, P], bf16, tag="wv")
        wo_sb = pers.tile([P, K, D], bf16, tag="wo")
        wup_sb = pers.tile([P, K, DT, JT, P], bf16, tag="wup")
        wdn_sb = pers.tile([P, K, CT, D], bf16, tag="wdn")
        dww_sb = pers.tile([P, K, CT, CK], f32, tag="dww")
        nc.sync.dma_start(dww_sb[:], dww_d[:].rearrange("k (ct p) c -> p k ct c", p=P))
        dwb_sb = pers.tile([P, K, CT], f32, tag="dwb")
        nc.sync.dma_start(dwb_sb[:], dwb_d[:].rearrange("k (ct p) -> p k ct", p=P))
        diag_sb = pers.tile([P, K, CT, CK, P], bf16, tag="diag")
        b1t_sb = pers.tile([P, DT, K], bf16, tag="b1t")
        b2t_sb = pers.tile([P, DT, K], bf16, tag="b2t")
        b2w_sb = pers.tile([P, K, JT], f32, tag="b2w")
        hwt_sb = pers.tile([P, DT, 2], f32, tag="hwt")
        nc.sync.dma_start(hwt_sb[:], hwt_d[:].rearrange("(dt p) c -> p dt c", p=P))
        hb_sb = pers.tile([1, 2], f32, tag="hb")
        nc.sync.dma_start(hb_sb[:], hb_d[:])

        xw_sb = pers.tile([P, K, 3, N], bf16, tag="xw")
        vp_aug = pers.tile([P, NT, 2, 64], bf16, tag="vpaug")
        onescol = pers.tile([P, 1], bf16, tag="onescol")
        nc.vector.memset(onescol[:], 1.0)
        wt_sb = pers.tile([P, NT, 2, 512], bf16, tag="wt")  # both heads, one n-block
        hf_sb = pers.tile([P, CT, N + 2], bf16, tag="hf")
        nc.vector.memset(hf_sb[:], 0.0)
        zft_sb = pers.tile([P, DT, N], bf16, tag="zft")

        # z / zt shared across LN sites (persistent; evals are serial anyway)
        z_sb = pers.tile([P, NT, D], bf16, tag="z")
        zt_sb = pers.tile([P, DT, N], bf16, tag="zt")
        mcat_sb = pers.tile([P, N], bf16, tag="mcat")

        # ---------------- setup (scoped transients) ----------------
        with tc.tile_pool(name="setup", bufs=1) as setup, tc.tile_pool(
            name="setup2", bufs=2
        ) as setup2:
            # cast helper: DMA f32 -> stage, cast to dst (bf16)
            def load_cast(dst_ap, src_ap, shape, tag, eng):
                s = setup.tile(shape, f32, tag="wstage")
                nc.sync.dma_start(s[:], src_ap)
                if eng == 0:
                    nc.vector.tensor_copy(dst_ap, s[:])
                else:
                    nc.scalar.copy(dst_ap, s[:])

            for i, (dst, src) in enumerate(
                ((wq_sb, wq_d), (wk_sb, wk_d), (wv_sb, wv_d))
            ):
                load_cast(
                    dst[:],
                    src[:].rearrange("k (dt p) j -> p k dt j", p=P),
                    [P, K, DT, P],
                    "wstage",
                    i % 2,
                )
            load_cast(
                wo_sb[:],
                wo_d[:].rearrange("k p d -> p k d"),
                [P, K, D],
                "wstage",
                1,
            )
            for k in range(K):
                load_cast(
                    wup_sb[:, k],
                    wup_d[k].rearrange("(dt p) (jt jj) -> p dt jt jj", p=P, jj=P),
                    [P, DT, JT, P],
                    "wstage",
                    k % 2,
                )
                load_cast(
                    wdn_sb[:, k],
                    wdn_d[k].rearrange("(ct p) d -> p ct d", p=P),
                    [P, CT, D],
                    "wstage",
                    (k + 1) % 2,
                )
            for bd, bt in ((n1b_d, b1t_sb), (n2b_d, b2t_sb)):
                bcol = setup2.tile([P, DT, K], f32, tag="bcol")
                for k in range(K):
                    for dtt in range(DT):
                        nc.sync.dma_start(
                            bcol[:, dtt, k : k + 1],
                            bd[k, dtt * P : (dtt + 1) * P][:, None],
                        )
                nc.vector.tensor_copy(bt[:], bcol[:])

            # depthwise conv as matmul: diagonal weight mats per (k, ct, tap)
            for k in range(K):
                for ct in range(CT):
                    for tap in range(CK):
                        nc.vector.tensor_scalar_mul(
                            diag_sb[:, k, ct, tap, :],
                            ident[:],
                            dww_sb[:, k, ct, tap : tap + 1],
                        )

            # ---- embedding + input LN -> X (bf16) -> XT
            ins_bc = setup.tile([P, D], f32, tag="insbc")
            nc.sync.dma_start(ins_bc[:], ins_d[:][None, :].to_broadcast([P, D]))
            inb_bc = setup.tile([P, D], f32, tag="inbbc")
            nc.sync.dma_start(inb_bc[:], inb_d[:][None, :].to_broadcast([P, D]))

            xb_sb = setup.tile([P, NT, D], bf16, tag="xb0")
            st = setup.tile([P, NT, 6], f32, tag="st0")
            mv = setup.tile([P, NT, 2], f32, tag="mv0")
            rstd = setup.tile([P, NT], f32, tag="rstd0")
            for t in range(NT):
                xrt = setup2.tile([P, D], f32, tag="xrt")
                nc.gpsimd.indirect_dma_start(
                    out=xrt[:],
                    out_offset=None,
                    in_=emb_d[:],
                    in_offset=bass.IndirectOffsetOnAxis(ap=ids_sb[:, t : t + 1], axis=0),
                )
                ps = setup2.tile([P, D], f32, tag="posst")
                nc.sync.dma_start(
                    ps[:], pos_d[:].rearrange("(t p) d -> p t d", p=P)[:, t, :]
                )
                nc.vector.tensor_tensor(xrt[:], xrt[:], ps[:], op=OP.add)
                nc.vector.bn_stats(st[:, t], xrt[:])
                nc.vector.bn_aggr(mv[:, t], st[:, t])
                nc.scalar.activation(
                    rstd[:, t : t + 1], mv[:, t, 1:2], AF.Ln, bias=eps_t[:]
                )
                nc.scalar.activation(
                    rstd[:, t : t + 1], rstd[:, t : t + 1], AF.Exp, scale=-0.5
                )
                nc.vector.tensor_scalar(
                    xb_sb[:, t],
                    xrt[:],
                    mv[:, t, 0:1],
                    rstd[:, t : t + 1],
                    op0=OP.subtract,
                    op1=OP.mult,
                )
                nc.vector.tensor_tensor(xb_sb[:, t], xb_sb[:, t], ins_bc[:], op=OP.mult)
                nc.vector.tensor_tensor(xb_sb[:, t], xb_sb[:, t], inb_bc[:], op=OP.add)
            if DEBUG:
                nc.sync.dma_start(dbg["dbg_xb"][:], xb_sb[:])
            xt_sb = setup.tile([P, DT, N], bf16, tag="xt")
            for t in range(NT):
                for dtt in range(DT):
                    nc.sync.dma_start_transpose(
                        xt_sb[:, dtt, t * P : (t + 1) * P],
                        xb_sb[:, t, dtt * P : (dtt + 1) * P],
                    )

            # ---- XW = (b1 + X) @ W  (raw weights, before s1 fold)
            for k in range(K):
                for pi, wsb in enumerate((wq_sb, wk_sb, wv_sb)):
                    pb = ps_sml.tile([P, 512], f32, tag="sml")
                    for dtt in range(DT):
                        nc.tensor.matmul(
                            pb[:, 0:1],
                            lhsT=wsb[:, k, dtt, :],
                            rhs=b1t_sb[:, dtt, k : k + 1],
                            start=(dtt == 0),
                            stop=(dtt == DT - 1),
                        )
                    b1w = setup2.tile([P, 1], f32, tag="b1w")
                    nc.vector.tensor_copy(b1w[:], pb[:, 0:1])
                    for nb in range(2):
                        pp = ps_sml.tile([P, 512], f32, tag="sml")
                        for dtt in range(DT):
                            nc.tensor.matmul(
                                pp[:],
                                lhsT=wsb[:, k, dtt, :],
                                rhs=xt_sb[:, dtt, nb * 512 : (nb + 1) * 512],
                                start=(dtt == 0),
                                stop=(dtt == DT - 1),
                            )
                        nc.vector.tensor_scalar(
                            xw_sb[:, k, pi, nb * 512 : (nb + 1) * 512],
                            pp[:],
                            b1w[:],
                            None,
                            op0=OP.add,
                        )

            # ---- b2W per (k, jt) (raw wup)
            for k in range(K):
                for jt in range(JT):
                    pb = ps_sml.tile([P, 512], f32, tag="sml")
                    for dtt in range(DT):
                        nc.tensor.matmul(
                            pb[:, 0:1],
                            lhsT=wup_sb[:, k, dtt, jt, :],
                            rhs=b2t_sb[:, dtt, k : k + 1],
                            start=(dtt == 0),
                            stop=(dtt == DT - 1),
                        )
                    eng = nc.vector if (k + jt) % 2 == 0 else nc.scalar
                    if eng is nc.vector:
                        nc.vector.tensor_copy(b2w_sb[:, k, jt : jt + 1], pb[:, 0:1])
                    else:
                        nc.scalar.copy(b2w_sb[:, k, jt : jt + 1], pb[:, 0:1])

            # ---- fold s1 into wq/wk/wv, s2 into wup, softplus(dt) into wo
            for wsb in (wq_sb, wk_sb, wv_sb):
                for k in range(K):
                    nc.vector.tensor_tensor(
                        wsb[:, k],
                        wsb[:, k],
                        n1s_sb[:, k, :, None].to_broadcast([P, DT, P]),
                        op=OP.mult,
                    )
            for k in range(K):
                nc.vector.tensor_tensor(
                    wup_sb[:, k],
                    wup_sb[:, k],
                    n2s_sb[:, k, :, None, None].to_broadcast([P, DT, JT, P]),
                    op=OP.mult,
                )
                nc.vector.tensor_tensor(
                    wo_sb[:, k],
                    wo_sb[:, k],
                    sp_bc[:, k : k + 1].to_broadcast([P, D]),
                    op=OP.mult,
                )

        # ---------------- block evals ----------------
        with tc.tile_pool(name="work", bufs=1) as work, tc.tile_pool(
            name="work2", bufs=2
        ) as work2:

            def layernorm(src, t_lo, t_hi, out, scale_bc=None, bias_bc=None):
                """LN over d for token tiles [t_lo, t_hi) of src [P,NT,D] f32.
                Writes (x-mu)*rstd (optionally *scale+bias) to out (bf16)."""
                nt = t_hi - t_lo
                st = work.tile([P, NT, 6], f32, tag="st")
                mv = work.tile([P, NT, 2], f32, tag="mv")
                for t in range(t_lo, t_hi):
                    nc.vector.bn_stats(st[:, t], src[:, t])
                    nc.vector.bn_aggr(mv[:, t], st[:, t])
                rstd = work.tile([P, NT], f32, tag="rstd")
                nc.scalar.activation(
                    rstd[:, t_lo:t_hi], mv[:, t_lo:t_hi, 1], AF.Ln, bias=eps_t[:]
                )
                nc.scalar.activation(
                    rstd[:, t_lo:t_hi], rstd[:, t_lo:t_hi], AF.Exp, scale=-0.5
                )
                nmr = work.tile([P, NT], f32, tag="nmr")
                nc.vector.tensor_tensor(
                    nmr[:, t_lo:t_hi], mv[:, t_lo:t_hi, 0], rstd[:, t_lo:t_hi], op=OP.mult
                )
                nc.vector.tensor_scalar_mul(nmr[:, t_lo:t_hi], nmr[:, t_lo:t_hi], -1.0)
                for t in range(t_lo, t_hi):
                    if t % 2 == 0:
                        nc.vector.tensor_scalar(
                            out[:, t],
                            src[:, t],
                            mv[:, t, 0:1],
                            rstd[:, t : t + 1],
                            op0=OP.subtract,
                            op1=OP.mult,
                        )
                    else:
                        nc.scalar.activation(
                            out[:, t],
                            src[:, t],
                            AF.Identity,
                            bias=nmr[:, t : t + 1],
                            scale=rstd[:, t : t + 1],
                        )

            def transpose_tiles(src, dst, t_lo, t_hi):
                for t in range(t_lo, t_hi):
                    for dtt in range(DT):
                        eng = nc.sync if (t + dtt) % 2 == 0 else nc.scalar
                        eng.dma_start_transpose(
                            dst[:, dtt, t * P : (t + 1) * P],
                            src[:, t, dtt * P : (dtt + 1) * P],
                        )

            def all_reduce_chunked(y, scaled_add_dst):
                """AllReduce y [P,NT,D] bf16 in NCHUNK token chunks; add into
                scaled_add_dst (q_res) in place."""
                for c in range(NCHUNK):
                    t0, t1 = c * TC_PER_CHUNK, (c + 1) * TC_PER_CHUNK
                    yb = dram.tile([P, TC_PER_CHUNK, D], bf16, tag="arin")
                    ab = dram.tile([P, TC_PER_CHUNK, D], bf16, tag="arout")
                    nc.sync.dma_start(yb[:], y[:, t0:t1])
                    if SKIP_COLLECTIVE:
                        nc.gpsimd.dma_start(ab[:], yb[:])
                    else:
                        nc.gpsimd.collective_compute(
                            "AllReduce",
                            OP.add,
                            replica_groups=RG,
                            ins=[yb[:].opt()],
                            outs=[ab[:].opt()],
                        )
                    nc.gpsimd.dma_start(
                        scaled_add_dst[:, t0:t1],
                        ab[:],
                        accum_op=OP.add,
                    )

            def block_eval(k, dump=False):
                # ---- LN1 -> z -> zT
                layernorm(q_res, 0, NT, z_sb)
                transpose_tiles(z_sb, zt_sb, 0, NT)
                if dump:
                    nc.sync.dma_start(dbg["dbg_z"][:], z_sb[:])
                    nc.sync.dma_start(dbg["dbg_zt"][:], zt_sb[:])

                # ---- projections (q, k, v)
                phi = [None, None, None]
                for pi, wsb in enumerate((wq_sb, wk_sb, wv_sb)):
                    pp = ps_big.tile([P, 1024], f32, tag="big")
                    for nb in range(2):
                        for dtt in range(DT):
                            nc.tensor.matmul(
                                pp[:, nb * 512 : (nb + 1) * 512],
                                lhsT=wsb[:, k, dtt, :],
                                rhs=zt_sb[:, dtt, nb * 512 : (nb + 1) * 512],
                                start=(dtt == 0),
                                stop=(dtt == DT - 1),
                            )
                    tag = ("phiq", "phik", "vpt")[pi]
                    res = work.tile([P, N], bf16, tag=tag)
                    if pi < 2:
                        ts_t = work2.tile([P, N], bf16, tag="tsum")
                        nc.vector.tensor_tensor(
                            ts_t[:], pp[:], xw_sb[:, k, pi, :], op=OP.add
                        )
                        rel = work.tile([P, N], bf16, tag="rel")
                        nc.vector.tensor_scalar_max(rel[:], ts_t[:], 0.0)
                        mn = work.tile([P, N], bf16, tag="mn")
                        nc.vector.tensor_scalar_min(mn[:], ts_t[:], 0.0)
                        ex = work.tile([P, N], bf16, tag="ex")
                        nc.scalar.activation(ex[:], mn[:], AF.Exp)
                        nc.vector.tensor_tensor(res[:], ex[:], rel[:], op=OP.add)
                    else:
                        nc.vector.tensor_tensor(
                            res[:], pp[:], xw_sb[:, k, pi, :], op=OP.add
                        )
                        for mt in range(NT):
                            for h in range(2):
                                eng = nc.sync if mt % 2 == 0 else nc.scalar
                                eng.dma_start_transpose(
                                    vp_aug[:, mt, h, 0:64],
                                    res[h * 64 : (h + 1) * 64, mt * P : (mt + 1) * P],
                                )
                    phi[pi] = res
                phiq, phik, vpt = phi
                if dump:
                    nc.sync.dma_start(dbg["dbg_phiq"][:], phiq[:])
                    nc.sync.dma_start(dbg["dbg_phik"][:], phik[:])
                    nc.sync.dma_start(dbg["dbg_vpt"][:], vpt[:])

                # ---- attention: nb-outer, heads packed via tile_position
                for nb in range(2):
                    nsl = slice(nb * 512, (nb + 1) * 512)
                    for mt in range(NT):
                        for h in range(2):
                            hs = h * 64
                            pw = ps_sml.tile([P, 512], f32, tag="sml")
                            nc.tensor.matmul(
                                pw[:],
                                lhsT=phik[hs : hs + 64, mt * P : (mt + 1) * P],
                                rhs=phiq[hs : hs + 64, nsl],
                                start=True,
                                stop=True,
                            )
                            dst = wt_sb[:, mt, h, :]
                            if (mt + h) % 2 == 0:
                                nc.scalar.activation(dst, pw[:], AF.Square)
                            else:
                                wc = work2.tile([P, 512], bf16, tag="wc")
                                nc.vector.tensor_copy(wc[:], pw[:])
                                nc.gpsimd.tensor_tensor(dst, wc[:], wc[:], op=OP.mult)
                    pa = ps_sml.tile([P, 512], f32, tag="sml")
                    pss0 = ps_sml.tile([P, 512], f32, tag="sml")
                    pss1 = ps_sml.tile([P, 512], f32, tag="sml")
                    for mt in range(NT):
                        for h in range(2):
                            hs = h * 64
                            nc.tensor.matmul(
                                pa[hs : hs + 64, :],
                                lhsT=vp_aug[:, mt, h, :],
                                rhs=wt_sb[:, mt, h, :],
                                start=(mt == 0),
                                stop=(mt == NT - 1),
                                tile_position=(0, hs),
                                skip_group_check=True,
                            )
                            nc.tensor.matmul(
                                (pss0 if h == 0 else pss1)[0:1, :],
                                lhsT=onescol[:],
                                rhs=wt_sb[:, mt, h, :],
                                start=(mt == 0),
                                stop=(mt == NT - 1),
                            )
                    prr = work.tile([P, 512], bf16, tag="prr")
                    for h in range(2):
                        hs = h * 64
                        pss = pss0 if h == 0 else pss1
                        rr = work.tile([1, 512], f32, tag="rr")
                        nc.vector.tensor_scalar_add(rr[:], pss[0:1, :], 1.0)
                        nc.vector.reciprocal_approx_fast(rr[:], rr[:])
                        rrb = work.tile([1, 512], bf16, tag="rrb")
                        nc.vector.tensor_copy(rrb[:], rr[:])
                        rsc = dram.tile([1, 512], bf16, tag="rsc")
                        nc.sync.dma_start(rsc[:], rrb[:])
                        nc.scalar.dma_start(
                            prr[hs : hs + 64, :],
                            rsc[0][None, :].to_broadcast([64, 512]),
                        )
                    at = work.tile([P, 512], bf16, tag="atr")
                    if nb == 0:
                        nc.vector.tensor_copy(at[:], pa[:])
                    else:
                        nc.scalar.copy(at[:], pa[:])
                    tm = work.tile([P, 512], bf16, tag="tm")
                    nc.vector.tensor_tensor(tm[:], at[:], prr[:], op=OP.mult)
                    nc.gpsimd.tensor_tensor(
                        mcat_sb[:, nsl], tm[:], vpt[:, nsl], op=OP.subtract
                    )

                # ---- out-proj (wo pre-scaled by softplus(dt)) + chunked AR
                y = work.tile([P, NT, D], bf16, tag="y")
                for nt in range(NT):
                    po = ps_sml.tile([P, 512], f32, tag="sml")
                    nc.tensor.matmul(
                        po[:],
                        lhsT=mcat_sb[:, nt * P : (nt + 1) * P],
                        rhs=wo_sb[:, k, :],
                        start=True,
                        stop=True,
                    )
                    if nt % 2 == 0:
                        nc.vector.tensor_copy(y[:, nt], po[:])
                    else:
                        nc.scalar.copy(y[:, nt], po[:])
                if dump:
                    nc.sync.dma_start(dbg["dbg_mcat"][:], mcat_sb[:])
                    nc.sync.dma_start(dbg["dbg_y"][:], y[:])
                all_reduce_chunked(y, q_res)
                if dump:
                    nc.sync.dma_start(dbg["dbg_q1"][:], q_res[:])

                # ---- LN2 -> z2 -> z2T
                layernorm(q_res, 0, NT, z_sb)
                transpose_tiles(z_sb, zt_sb, 0, NT)

                # ---- up-proj + SwiGLU -> hf
                for nb in range(2):
                    for jp in range(CT):
                        pg = ps_sml.tile([P, 512], f32, tag="sml")
                        for dtt in range(DT):
                            nc.tensor.matmul(
                                pg[:],
                                lhsT=wup_sb[:, k, dtt, jp, :],
                                rhs=zt_sb[:, dtt, nb * 512 : (nb + 1) * 512],
                                start=(dtt == 0),
                                stop=(dtt == DT - 1),
                            )
                        pu = ps_sml.tile([P, 512], f32, tag="sml")
                        for dtt in range(DT):
                            nc.tensor.matmul(
                                pu[:],
                                lhsT=wup_sb[:, k, dtt, jp + CT, :],
                                rhs=zt_sb[:, dtt, nb * 512 : (nb + 1) * 512],
                                start=(dtt == 0),
                                stop=(dtt == DT - 1),
                            )
                        sg = work2.tile([P, 512], bf16, tag="sg")
                        nc.scalar.activation(
                            sg[:], pg[:], AF.Silu, bias=b2w_sb[:, k, jp : jp + 1]
                        )
                        uu = work2.tile([P, 512], bf16, tag="uu")
                        nc.vector.tensor_scalar(
                            uu[:], pu[:], b2w_sb[:, k, jp + CT : jp + CT + 1], None, op0=OP.add
                        )
                        nc.gpsimd.tensor_tensor(
                            hf_sb[:, jp, 1 + nb * 512 : 1 + (nb + 1) * 512],
                            sg[:],
                            uu[:],
                            op=OP.mult,
                        )

                # ---- depthwise conv (as 3 diag matmuls) + silu -> hcv
                hcv = work.tile([P, CT, N], bf16, tag="hcv")
                for ct in range(CT):
                    for nb in range(2):
                        pc = ps_sml.tile([P, 512], f32, tag="sml")
                        for tap in range(CK):
                            nc.tensor.matmul(
                                pc[:],
                                lhsT=diag_sb[:, k, ct, tap, :],
                                rhs=hf_sb[:, ct, nb * 512 + tap : nb * 512 + tap + 512],
                                start=(tap == 0),
                                stop=(tap == CK - 1),
                            )
                        nc.scalar.activation(
                            hcv[:, ct, nb * 512 : (nb + 1) * 512],
                            pc[:],
                            AF.Silu,
                            bias=dwb_sb[:, k, ct : ct + 1],
                        )

                if dump:
                    nc.sync.dma_start(dbg["dbg_hf"][:], hf_sb[:])
                    nc.sync.dma_start(dbg["dbg_hcv"][:], hcv[:])
                # ---- down-proj + chunked AR
                y2 = work.tile([P, NT, D], bf16, tag="y")
                for nt in range(NT):
                    pd = ps_sml.tile([P, 512], f32, tag="sml")
                    for ct in range(CT):
                        nc.tensor.matmul(
                            pd[:],
                            lhsT=hcv[:, ct, nt * P : (nt + 1) * P],
                            rhs=wdn_sb[:, k, ct, :],
                            start=(ct == 0),
                            stop=(ct == CT - 1),
                        )
                    if nt % 2 == 0:
                        nc.vector.tensor_copy(y2[:, nt], pd[:])
                    else:
                        nc.scalar.copy(y2[:, nt], pd[:])
                if dump:
                    nc.sync.dma_start(dbg["dbg_y2"][:], y2[:])
                all_reduce_chunked(y2, q_res)
                if dump:
                    nc.sync.dma_start(dbg["dbg_q2"][:], q_res[:])

            for _cyc in range(CYCLES):
                for k in range(K):
                    block_eval(k, dump=(DEBUG and _cyc == 0 and k == 0))

            # ---------------- final LN (with fin scale/bias) ----------------
            layernorm(q_res, 0, NT, z_sb)
            nc.vector.tensor_tensor(
                z_sb[:], z_sb[:], fins_bc[:, None, :].to_broadcast([P, NT, D]), op=OP.mult
            )
            nc.vector.tensor_tensor(
                z_sb[:], z_sb[:], finb_bc[:, None, :].to_broadcast([P, NT, D]), op=OP.add
            )
            transpose_tiles(z_sb, zft_sb, 0, NT)

            # ---- q_logits = mean_n(Qn) @ halt_w.T + halt_b
            qm = work.tile([P, DT], f32, tag="qm")
            nc.vector.reduce_sum(qm[:], zft_sb[:], axis=mybir.AxisListType.X)
            pq = ps_sml.tile([P, 512], f32, tag="sml")
            for dtt in range(DT):
                nc.tensor.matmul(
                    pq[0:1, 0:2],
                    lhsT=qm[:, dtt : dtt + 1],
                    rhs=hwt_sb[:, dtt, :],
                    start=(dtt == 0),
                    stop=(dtt == DT - 1),
                )
            ql = work.tile([1, 2], f32, tag="ql")
            nc.vector.tensor_scalar_mul(ql[:], pq[0:1, 0:2], 1.0 / N)
            nc.vector.tensor_tensor(ql[:], ql[:], hb_sb[:], op=OP.add)
            nc.sync.dma_start(qlog_d[:], ql[:])

        # ---------------- lm head (vocab-sharded) ----------------
        with tc.tile_pool(name="lmp", bufs=2) as lmp, tc.tile_pool(
            name="lmp1", bufs=1
        ) as lmp1:
            lg = logits_d[:].rearrange("(nt p) v -> p nt v", p=P)
            lmsrc = lmt_d[:].rearrange("(dt p) v -> p dt v", p=P)
            for vc in range(NVC):
                stage = lmp.tile([P, DT, VCH], f32, tag="lstage")
                nc.sync.dma_start(stage[:], lmsrc[:, :, vc * VCH : (vc + 1) * VCH])
                lc = lmp.tile([P, DT, VCH], bf16, tag="lc")
                if vc % 2 == 0:
                    nc.vector.tensor_copy(lc[:], stage[:])
                else:
                    nc.scalar.copy(lc[:], stage[:])
                ob = lmp1.tile([P, NT, VCH], f32, tag="ob")
                for nt in range(NT):
                    pl = ps_sml.tile([P, 512], f32, tag="sml")
                    for dtt in range(DT):
                        nc.tensor.matmul(
                            pl[:, 0:VCH],
                            lhsT=zft_sb[:, dtt, nt * P : (nt + 1) * P],
                            rhs=lc[:, dtt, :],
                            start=(dtt == 0),
                            stop=(dtt == DT - 1),
                        )
                    if (vc + nt) % 2 == 0:
                        nc.vector.tensor_copy(ob[:, nt, :], pl[:, 0:VCH])
                    else:
                        nc.scalar.copy(ob[:, nt, :], pl[:, 0:VCH])
                eng = nc.sync if vc % 2 == 0 else nc.scalar
                eng.dma_start(lg[:, :, vc * VCH : (vc + 1) * VCH], ob[:])

        dram.release()
        ps_sml.release()
        ps_big.release()
        pers.release()

    nc.compile()
    return nc


def _get_nc():
    if "nc" not in _CACHE:
        _CACHE["nc"] = _build()
    return _CACHE["nc"]


def _prep_in_maps(inputs):
    ii = {k: np.asarray(v) for k, v in inputs.items()}
    hm = ii["carry_halted"].astype(bool)
    ids = np.where(hm[:, None], ii["inputs"], ii["carry_inputs"]).astype(np.int32)
    init_h = ii["init_hidden"].astype(np.float32)
    q0 = np.where(
        hm[:, None, None],
        np.broadcast_to(init_h[None, None, :], (B, N, D)),
        ii["carry_hidden"].astype(np.float32),
    ).astype(np.float32)
    emb = np.ascontiguousarray(ii["emb"].astype(np.float32))
    posn = np.ascontiguousarray(ii["pos"].astype(np.float32)[:N])
    lmT = np.ascontiguousarray(ii["lm_w"].astype(np.float32).T)  # [D, V]
    hwT = np.ascontiguousarray(ii["halt_w"].astype(np.float32).T)  # [D, 2]
    hb = ii["halt_b"].astype(np.float32).reshape(1, 2)
    dtv = ii["dt"].astype(np.float32).reshape(1, K)
    wq = ii["W_Q"].astype(np.float32)
    wk = ii["W_K"].astype(np.float32)
    wv = ii["W_V"].astype(np.float32)
    wo = ii["W_O"].astype(np.float32)
    wup = ii["W_up"].astype(np.float32)
    dww = ii["dw_w"].astype(np.float32)[:, :, 0, :]  # [K, INNER, CK]
    dwb = ii["dw_b"].astype(np.float32)
    wdn = ii["W_down"].astype(np.float32)

    in_maps = []
    for c in range(8):
        b, g = c // GRP, c % GRP
        jlo = g * P  # head-col slice (2 heads x 64)
        clo = g * CT * P  # inner slice (384)
        m = {
            "ids": np.ascontiguousarray(ids[b]),
            "q0": np.ascontiguousarray(q0[b]),
            "emb": emb,
            "posn": posn,
            "ins_v": ii["in_s"].astype(np.float32),
            "inb_v": ii["in_b"].astype(np.float32),
            "fins_v": ii["fin_s"].astype(np.float32),
            "finb_v": ii["fin_b"].astype(np.float32),
            "dtv": dtv,
            "wq": np.ascontiguousarray(wq[:, :, jlo : jlo + P]),
            "wk": np.ascontiguousarray(wk[:, :, jlo : jlo + P]),
            "wv": np.ascontiguousarray(wv[:, :, jlo : jlo + P]),
            "wo": np.ascontiguousarray(wo[:, jlo : jlo + P, :]),
            "wup": np.ascontiguousarray(
                np.concatenate(
                    (
                        wup[:, :, clo : clo + CT * P],
                        wup[:, :, INNER + clo : INNER + clo + CT * P],
                    ),
                    axis=-1,
                )
            ),
            "dww": np.ascontiguousarray(dww[:, clo : clo + CT * P, :]),
            "dwb": np.ascontiguousarray(dwb[:, clo : clo + CT * P]),
            "wdn": np.ascontiguousarray(wdn[:, clo : clo + CT * P, :]),
            "n1s": ii["n1_s"].astype(np.float32),
            "n1b": ii["n1_b"].astype(np.float32),
            "n2s": ii["n2_s"].astype(np.float32),
            "n2b": ii["n2_b"].astype(np.float32),
            "lmt": np.ascontiguousarray(lmT[:, g * VC : (g + 1) * VC]),
            "hwt": hwT,
            "hb": hb,
        }
        in_maps.append(m)
    return in_maps


def _run_fast(nc, in_maps):
    """Cached jitted executor (avoids per-call jit retrace). Falls back to
    run_bass_kernel_spmd on any failure."""
    import jax
    import jax.numpy as jnp
    import concourse.mybir as mybir
    from jax.sharding import Mesh, PartitionSpec, NamedSharding
    from jax.experimental.shard_map import shard_map
    from concourse.bass2jax import (
        _bass_exec_p,
        partition_id_tensor,
        install_neuronx_cc_hook,
    )

    if "fast" not in _CACHE:
        install_neuronx_cc_hook()
        partition_name = (
            nc.partition_id_tensor.name if nc.partition_id_tensor else None
        )
        in_names, out_names, out_avals = [], [], []
        for alloc in nc.m.functions[0].allocations:
            if not isinstance(alloc, mybir.MemoryLocationSet):
                continue
            name = alloc.memorylocations[0].name
            if alloc.kind == "ExternalInput":
                if name != partition_name:
                    in_names.append(name)
            elif alloc.kind == "ExternalOutput":
                out_names.append(name)
                out_avals.append(
                    jax.core.ShapedArray(
                        tuple(alloc.tensor_shape), mybir.dt.np(alloc.dtype)
                    )
                )
        n_params = len(in_names)
        all_in = in_names + out_names + ([partition_name] if partition_name else [])

        def _body(*args):
            ins = list(args[:n_params])
            outs = list(args[n_params:])
            pid = [partition_id_tensor()] if partition_name else []
            return tuple(
                _bass_exec_p.bind(
                    *ins,
                    *outs,
                    *pid,
                    out_avals=tuple(out_avals),
                    in_names=tuple(all_in),
                    out_names=tuple(out_names),
                    lowering_input_output_aliases=(),
                    sim_require_finite=True,
                    sim_require_nnan=True,
                    nc=nc,
                )
            )

        devices = jax.devices()[:8]
        mesh = Mesh(np.asarray(devices), ("core",))
        n_outs = len(out_names)
        f = jax.jit(
            shard_map(
                _body,
                mesh=mesh,
                in_specs=(PartitionSpec("core"),) * (n_params + n_outs),
                out_specs=(PartitionSpec("core"),) * n_outs,
                check_rep=False,
            ),
            donate_argnums=tuple(range(n_params, n_params + n_outs)),
            keep_unused=True,
        )
        _CACHE["fast"] = (f, in_names, out_names, out_avals, mesh)
    f, in_names, out_names, out_avals, mesh = _CACHE["fast"]
    import jax

    sh = jax.sharding.NamedSharding(mesh, PartitionSpec("core"))
    concat_in = [
        np.concatenate([np.asarray(in_maps[c][nm]) for c in range(8)], axis=0)
        for nm in in_names
    ]
    dev_in = [jax.device_put(a, sh) for a in concat_in]
    zeros = [
        jax.device_put(np.zeros((av.shape[0] * 8,) + av.shape[1:], av.dtype), sh)
        for av in out_avals
    ]
    outs = f(*dev_in, *zeros)
    jax.block_until_ready(outs)
    res = []
    for c in range(8):
        m = {}
        for i, nm in enumerate(out_names):
            av = out_avals[i]
            m[nm] = np.asarray(outs[i])[c * av.shape[0] : (c + 1) * av.shape[0]]
        res.append(m)
    return res


def kernel(**inputs):
    from concourse.bass_utils import run_bass_kernel_spmd

    nc = _get_nc()
    in_maps = _prep_in_maps(inputs)
    try:
        res = _run_fast(nc, in_maps)
    except Exception:
        res = run_bass_kernel_spmd(nc, in_maps, core_ids=list(range(8))).results
    logits = np.zeros((B, N, V), np.float32)
    for c in range(8):
        b, g = c // GRP, c % GRP
        logits[b, :, g * VC : (g + 1) * VC] = res[c]["logits"]
    q_logits = np.stack([res[0]["qlog"][0], res[GRP]["qlog"][0]])
    return logits, q_logits


# revision 23
# speedup vs baseline: 3.4288x; 1.1859x over previous
"""Trainium2 Bass kernel for nn_AMKPDModel (linear-attention transformer,
K=4 blocks x 2 cycles, ConvSwiGLU FFN, 32k-vocab LM head) on 8 NeuronCores.

Sharding: 2 data-parallel groups of 4 cores (one per batch element).
Within a group: attention heads sharded 2/core, FFN inner dim sharded
384/core, lm_head vocab sharded 8000/core. Two group-local AllReduces
per block eval ([1024,512] bf16), chunked for compute/comm overlap.
"""

import sys

if "/opt/trn_rl_repo" not in sys.path:
    sys.path.insert(0, "/opt/trn_rl_repo")

import numpy as np

# model dims
B, N, D = 2, 1024, 512
K = 4
V = 32000
INNER = 1536
CK = 3
EPS = 1e-5
CYCLES = 2  # H_CYCLES runs of the 4-block stack

# sharding
GRP = 4            # cores per batch group
P = 128
NT = N // P        # 8 token tiles
DT = D // P        # 4 feature tiles
JT = 6             # up-proj 768/128 local tiles (3 G + 3 U)
CT = 3             # local inner tiles (384/128)
VC = V // GRP      # 8000 vocab rows per core
VCH = 500
NVC = VC // VCH    # 16 lm chunks
NCHUNK = 2         # AR chunks per block output (token-split)
TC_PER_CHUNK = NT // NCHUNK

RG = [[0, 1, 2, 3], [4, 5, 6, 7]]

_CACHE = {}
DEBUG = False
SKIP_COLLECTIVE = False  # timing-only: replace AR with local copy


def _build():
    import concourse.bass as bass
    import concourse.mybir as mybir
    import concourse.tile as tile
    from concourse import bacc
    from concourse.bass import _add_dep_helper
    from concourse.masks import make_identity

    f32 = mybir.dt.float32
    bf16 = mybir.dt.bfloat16
    i32 = mybir.dt.int32
    AF = mybir.ActivationFunctionType
    OP = mybir.AluOpType

    nc = bacc.Bacc(None, target_bir_lowering=False, debug=False, num_devices=8)

    # ---------------- DRAM params ----------------
    ids_d = nc.declare_dram_parameter("ids", [N], i32, isOutput=False)
    q0_d = nc.declare_dram_parameter("q0", [N, D], f32, isOutput=False)
    emb_d = nc.declare_dram_parameter("emb", [V, D], f32, isOutput=False)
    pos_d = nc.declare_dram_parameter("posn", [N, D], f32, isOutput=False)
    ins_d = nc.declare_dram_parameter("ins_v", [D], f32, isOutput=False)
    inb_d = nc.declare_dram_parameter("inb_v", [D], f32, isOutput=False)
    fins_d = nc.declare_dram_parameter("fins_v", [D], f32, isOutput=False)
    finb_d = nc.declare_dram_parameter("finb_v", [D], f32, isOutput=False)
    dtv_d = nc.declare_dram_parameter("dtv", [1, K], f32, isOutput=False)
    wq_d = nc.declare_dram_parameter("wq", [K, D, P], f32, isOutput=False)
    wk_d = nc.declare_dram_parameter("wk", [K, D, P], f32, isOutput=False)
    wv_d = nc.declare_dram_parameter("wv", [K, D, P], f32, isOutput=False)
    wo_d = nc.declare_dram_parameter("wo", [K, P, D], f32, isOutput=False)
    wup_d = nc.declare_dram_parameter("wup", [K, D, JT * P], f32, isOutput=False)
    dww_d = nc.declare_dram_parameter("dww", [K, CT * P, CK], f32, isOutput=False)
    dwb_d = nc.declare_dram_parameter("dwb", [K, CT * P], f32, isOutput=False)
    wdn_d = nc.declare_dram_parameter("wdn", [K, CT * P, D], f32, isOutput=False)
    n1s_d = nc.declare_dram_parameter("n1s", [K, D], f32, isOutput=False)
    n1b_d = nc.declare_dram_parameter("n1b", [K, D], f32, isOutput=False)
    n2s_d = nc.declare_dram_parameter("n2s", [K, D], f32, isOutput=False)
    n2b_d = nc.declare_dram_parameter("n2b", [K, D], f32, isOutput=False)
    lmt_d = nc.declare_dram_parameter("lmt", [D, VC], f32, isOutput=False)
    hwt_d = nc.declare_dram_parameter("hwt", [D, 2], f32, isOutput=False)
    hb_d = nc.declare_dram_parameter("hb", [1, 2], f32, isOutput=False)

    logits_d = nc.declare_dram_parameter("logits", [N, VC], f32, isOutput=True)
    qlog_d = nc.declare_dram_parameter("qlog", [1, 2], f32, isOutput=True)
    dbg = {}
    if DEBUG:
        bf16_ = mybir.dt.bfloat16
        for nm, shp, dt_ in (
            ("dbg_xb", [P, NT, D], bf16_),
            ("dbg_z", [P, NT, D], bf16_),
            ("dbg_zt", [P, DT, N], bf16_),
            ("dbg_phiq", [P, N], bf16_),
            ("dbg_phik", [P, N], bf16_),
            ("dbg_vpt", [P, N], bf16_),
            ("dbg_wt", [P, NT, N], bf16_),
            ("dbg_mcat", [P, N], bf16_),
            ("dbg_y", [P, NT, D], bf16_),
            ("dbg_q1", [P, NT, D], f32),
            ("dbg_hf", [P, CT, N + 2], bf16_),
            ("dbg_hcv", [P, CT, N], bf16_),
            ("dbg_y2", [P, NT, D], bf16_),
            ("dbg_q2", [P, NT, D], f32),
        ):
            dbg[nm] = nc.declare_dram_parameter(nm, shp, dt_, isOutput=True)

    with tile.TileContext(nc) as tc:
        pers = tc.alloc_tile_pool(name="pers", bufs=1)
        ps_big = tc.alloc_tile_pool(name="psb", bufs=1, space="PSUM")
        ps_sml = tc.alloc_tile_pool(name="pss", bufs=6, space="PSUM")
        dram = tc.alloc_tile_pool(name="dram", bufs=2, space="DRAM")

        # ---------------- persistent tiles ----------------
        eps_t = pers.tile([P, 1], f32, tag="eps")
        nc.vector.memset(eps_t[:], EPS)
        ones1 = pers.tile([1, P], bf16, tag="ones1")
        nc.vector.memset(ones1[:], 1.0)
        ident = pers.tile([P, P], bf16, tag="ident")
        make_identity(nc, ident[:])

        ids_sb = pers.tile([P, NT], i32, tag="ids")
        nc.sync.dma_start(ids_sb[:], ids_d[:].rearrange("(t p) -> p t", p=P))
        q_res = pers.tile([P, NT, D], f32, tag="qres")
        nc.sync.dma_start(q_res[:], q0_d[:].rearrange("(t p) d -> p t d", p=P))

        def bcast_row(src_d, tag):
            t = pers.tile([P, D], f32, tag=tag)
            nc.sync.dma_start(t[:], src_d[:][None, :].to_broadcast([P, D]))
            return t

        fins_bc = bcast_row(fins_d, "finsbc")
        finb_bc = bcast_row(finb_d, "finbbc")

        # softplus(dt) broadcast to all partitions
        dtv_sb = pers.tile([1, K], f32, tag="dtv")
        nc.sync.dma_start(dtv_sb[:], dtv_d[:])
        spe = pers.tile([1, K], f32, tag="spe")
        nc.scalar.activation(spe[:], dtv_sb[:], AF.Exp)
        nc.vector.tensor_scalar_add(spe[:], spe[:], 1.0)
        nc.scalar.activation(spe[:], spe[:], AF.Ln)
        sp_bc = pers.tile([P, K], f32, tag="spbc")
        nc.gpsimd.partition_broadcast(sp_bc[:], spe[:])

        # per-block norm scale vectors (transposed layouts)
        n1s_sb = pers.tile([P, K, DT], f32, tag="n1s")
        nc.sync.dma_start(n1s_sb[:], n1s_d[:].rearrange("k (dt p) -> p k dt", p=P))
        n2s_sb = pers.tile([P, K, DT], f32, tag="n2s")
        nc.sync.dma_start(n2s_sb[:], n2s_d[:].rearrange("k (dt p) -> p k dt", p=P))

        # weights (loaded f32, cast bf16)
        wq_sb = pers.tile([P, K, DT, P], bf16, tag="wq")
        wk_sb = pers.tile([P, K, DT, P], bf16, tag="wk")
        wv_sb = pers.tile([P, K, DT, P], bf16, tag="wv")
        wo_sb = pers.tile([P, K, D], bf16, tag="wo")
        wup_sb = pers.tile([P, K, DT, JT, P], bf16, tag="wup")
        wdn_sb = pers.tile([P, K, CT, D], bf16, tag="wdn")
        dww_sb = pers.tile([P, K, CT, CK], f32, tag="dww")
        nc.sync.dma_start(dww_sb[:], dww_d[:].rearrange("k (ct p) c -> p k ct c", p=P))
        dwb_sb = pers.tile([P, K, CT], f32, tag="dwb")
        nc.sync.dma_start(dwb_sb[:], dwb_d[:].rearrange("k (ct p) -> p k ct", p=P))
        diag_sb = pers.tile([P, K, CT, CK, P], bf16, tag="diag")
        b1t_sb = pers.tile([P, DT, K], bf16, tag="b1t")
        b2t_sb = pers.tile([P, DT, K], bf16, tag="b2t")
        b2w_sb = pers.tile([P, K, JT], f32, tag="b2w")
        hwt_sb = pers.tile([P, DT, 2], f32, tag="hwt")
        nc.sync.dma_start(hwt_sb[:], hwt_d[:].rearrange("(dt p) c -> p dt c", p=P))
        hb_sb = pers.tile([1, 2], f32, tag="hb")
        nc.sync.dma_start(hb_sb[:], hb_d[:])

        xw_sb = pers.tile([P, K, 3, N], bf16, tag="xw")
        vp_aug = pers.tile([P, NT, 2, 64], bf16, tag="vpaug")
        onescol = pers.tile([P, 1], bf16, tag="onescol")
        nc.vector.memset(onescol[:], 1.0)
        wt_sb = pers.tile([P, NT, 2, 512], bf16, tag="wt")  # both heads, one n-block
        hf_sb = pers.tile([P, CT, N + 2], bf16, tag="hf")
        nc.vector.memset(hf_sb[:], 0.0)
        zft_sb = pers.tile([P, DT, N], bf16, tag="zft")

        # z / zt shared across LN sites (persistent; evals are serial anyway)
        z_sb = pers.tile([P, NT, D], bf16, tag="z")
        zt_sb = pers.tile([P, DT, N], bf16, tag="zt")
        mcat_sb = pers.tile([P, N], bf16, tag="mcat")

        # ---------------- setup (scoped transients) ----------------
        with tc.tile_pool(name="setup", bufs=1) as setup, tc.tile_pool(
            name="setup2", bufs=2
        ) as setup2:
            # cast helper: DMA f32 -> stage, cast to dst (bf16)
            def load_cast(dst_ap, src_ap, shape, tag, eng):
                s = setup.tile(shape, f32, tag="wstage")
                nc.sync.dma_start(s[:], src_ap)
                if eng == 0:
                    nc.vector.tensor_copy(dst_ap, s[:])
                else:
                    nc.scalar.copy(dst_ap, s[:])

            for i, (dst, src) in enumerate(
                ((wq_sb, wq_d), (wk_sb, wk_d), (wv_sb, wv_d))
            ):
                load_cast(
                    dst[:],
                    src[:].rearrange("k (dt p) j -> p k dt j", p=P),
                    [P, K, DT, P],
                    "wstage",
                    i % 2,
                )
            load_cast(
                wo_sb[:],
                wo_d[:].rearrange("k p d -> p k d"),
                [P, K, D],
                "wstage",
                1,
            )
            for k in range(K):
                load_cast(
                    wup_sb[:, k],
                    wup_d[k].rearrange("(dt p) (jt jj) -> p dt jt jj", p=P, jj=P),
                    [P, DT, JT, P],
                    "wstage",
                    k % 2,
                )
                load_cast(
                    wdn_sb[:, k],
                    wdn_d[k].rearrange("(ct p) d -> p ct d", p=P),
                    [P, CT, D],
                    "wstage",
                    (k + 1) % 2,
                )
            for bd, bt in ((n1b_d, b1t_sb), (n2b_d, b2t_sb)):
                bcol = setup2.tile([P, DT, K], f32, tag="bcol")
                for k in range(K):
                    for dtt in range(DT):
                        nc.sync.dma_start(
                            bcol[:, dtt, k : k + 1],
                            bd[k, dtt * P : (dtt + 1) * P][:, None],
                        )
                nc.vector.tensor_copy(bt[:], bcol[:])

            # depthwise conv as matmul: diagonal weight mats per (k, ct, tap)
            for k in range(K):
                for ct in range(CT):
                    for tap in range(CK):
                        nc.vector.tensor_scalar_mul(
                            diag_sb[:, k, ct, tap, :],
                            ident[:],
                            dww_sb[:, k, ct, tap : tap + 1],
                        )

            # ---- embedding + input LN -> X (bf16) -> XT
            ins_bc = setup.tile([P, D], f32, tag="insbc")
            nc.sync.dma_start(ins_bc[:], ins_d[:][None, :].to_broadcast([P, D]))
            inb_bc = setup.tile([P, D], f32, tag="inbbc")
            nc.sync.dma_start(inb_bc[:], inb_d[:][None, :].to_broadcast([P, D]))

            xb_sb = setup.tile([P, NT, D], bf16, tag="xb0")
            st = setup.tile([P, NT, 6], f32, tag="st0")
            mv = setup.tile([P, NT, 2], f32, tag="mv0")
            rstd = setup.tile([P, NT], f32, tag="rstd0")
            for t in range(NT):
                xrt = setup2.tile([P, D], f32, tag="xrt")
                nc.gpsimd.indirect_dma_start(
                    out=xrt[:],
                    out_offset=None,
                    in_=emb_d[:],
                    in_offset=bass.IndirectOffsetOnAxis(ap=ids_sb[:, t : t + 1], axis=0),
                )
                ps = setup2.tile([P, D], f32, tag="posst")
                nc.sync.dma_start(
                    ps[:], pos_d[:].rearrange("(t p) d -> p t d", p=P)[:, t, :]
                )
                nc.vector.tensor_tensor(xrt[:], xrt[:], ps[:], op=OP.add)
                nc.vector.bn_stats(st[:, t], xrt[:])
                nc.vector.bn_aggr(mv[:, t], st[:, t])
                nc.vector.tensor_scalar_add(rstd[:, t : t + 1], mv[:, t, 1:2], EPS)
                nc.vector.reciprocal_approx_fast(rstd[:, t : t + 1], rstd[:, t : t + 1])
                nc.scalar.activation(rstd[:, t : t + 1], rstd[:, t : t + 1], AF.Sqrt)
                nc.vector.tensor_scalar(
                    xb_sb[:, t],
                    xrt[:],
                    mv[:, t, 0:1],
                    rstd[:, t : t + 1],
                    op0=OP.subtract,
                    op1=OP.mult,
                )
                nc.vector.tensor_tensor(xb_sb[:, t], xb_sb[:, t], ins_bc[:], op=OP.mult)
                nc.vector.tensor_tensor(xb_sb[:, t], xb_sb[:, t], inb_bc[:], op=OP.add)
            if DEBUG:
                nc.sync.dma_start(dbg["dbg_xb"][:], xb_sb[:])
            xt_sb = setup.tile([P, DT, N], bf16, tag="xt")
            for t in range(NT):
                for dtt in range(DT):
                    nc.sync.dma_start_transpose(
                        xt_sb[:, dtt, t * P : (t + 1) * P],
                        xb_sb[:, t, dtt * P : (dtt + 1) * P],
                    )

            # ---- XW = (b1 + X) @ W  (raw weights, before s1 fold)
            for k in range(K):
                for pi, wsb in enumerate((wq_sb, wk_sb, wv_sb)):
                    pb = ps_sml.tile([P, 512], f32, tag="sml")
                    for dtt in range(DT):
                        nc.tensor.matmul(
                            pb[:, 0:1],
                            lhsT=wsb[:, k, dtt, :],
                            rhs=b1t_sb[:, dtt, k : k + 1],
                            start=(dtt == 0),
                            stop=(dtt == DT - 1),
                        )
                    b1w = setup2.tile([P, 1], f32, tag="b1w")
                    nc.vector.tensor_copy(b1w[:], pb[:, 0:1])
                    for nb in range(2):
                        pp = ps_sml.tile([P, 512], f32, tag="sml")
                        for dtt in range(DT):
                            nc.tensor.matmul(
                                pp[:],
                                lhsT=wsb[:, k, dtt, :],
                                rhs=xt_sb[:, dtt, nb * 512 : (nb + 1) * 512],
                                start=(dtt == 0),
                                stop=(dtt == DT - 1),
                            )
                        nc.vector.tensor_scalar(
                            xw_sb[:, k, pi, nb * 512 : (nb + 1) * 512],
                            pp[:],
                            b1w[:],
                            None,
                            op0=OP.add,
                        )

            # ---- b2W per (k, jt) (raw wup)
            for k in range(K):
                for jt in range(JT):
                    pb = ps_sml.tile([P, 512], f32, tag="sml")
                    for dtt in range(DT):
                        nc.tensor.matmul(
                            pb[:, 0:1],
                            lhsT=wup_sb[:, k, dtt, jt, :],
                            rhs=b2t_sb[:, dtt, k : k + 1],
                            start=(dtt == 0),
                            stop=(dtt == DT - 1),
                        )
                    eng = nc.vector if (k + jt) % 2 == 0 else nc.scalar
                    if eng is nc.vector:
                        nc.vector.tensor_copy(b2w_sb[:, k, jt : jt + 1], pb[:, 0:1])
                    else:
                        nc.scalar.copy(b2w_sb[:, k, jt : jt + 1], pb[:, 0:1])

            # ---- fold s1 into wq/wk/wv, s2 into wup, softplus(dt) into wo
            for wsb in (wq_sb, wk_sb, wv_sb):
                for k in range(K):
                    nc.vector.tensor_tensor(
                        wsb[:, k],
                        wsb[:, k],
                        n1s_sb[:, k, :, None].to_broadcast([P, DT, P]),
                        op=OP.mult,
                    )
            for k in range(K):
                nc.vector.tensor_tensor(
                    wup_sb[:, k],
                    wup_sb[:, k],
                    n2s_sb[:, k, :, None, None].to_broadcast([P, DT, JT, P]),
                    op=OP.mult,
                )
                nc.vector.tensor_tensor(
                    wo_sb[:, k],
                    wo_sb[:, k],
                    sp_bc[:, k : k + 1].to_broadcast([P, D]),
                    op=OP.mult,
                )

        # ---------------- block evals ----------------
        with tc.tile_pool(name="work", bufs=1) as work, tc.tile_pool(
            name="work2", bufs=2
        ) as work2:

            last_silu = [None]

            def layernorm(src, t_lo, t_hi, out, scale_bc=None, bias_bc=None):
                """LN over d for token tiles [t_lo, t_hi) of src [P,NT,D] f32.
                Writes (x-mu)*rstd (optionally *scale+bias) to out (bf16)."""
                nt = t_hi - t_lo
                st = work.tile([P, NT, 6], f32, tag="st")
                mv = work.tile([P, NT, 2], f32, tag="mv")
                for t in range(t_lo, t_hi):
                    nc.vector.bn_stats(st[:, t], src[:, t])
                    nc.vector.bn_aggr(mv[:, t], st[:, t])
                rstd = work.tile([P, NT], f32, tag="rstd")
                nc.vector.tensor_scalar_add(
                    rstd[:, t_lo:t_hi], mv[:, t_lo:t_hi, 1], EPS
                )
                nc.vector.reciprocal_approx_fast(rstd[:, t_lo:t_hi], rstd[:, t_lo:t_hi])
                sq_i = nc.scalar.activation(
                    rstd[:, t_lo:t_hi], rstd[:, t_lo:t_hi], AF.Sqrt
                )
                if last_silu[0] is not None:
                    _add_dep_helper(
                        sq_i.ins,
                        last_silu[0].ins,
                        sync=False,
                        reason="act-table grouping",
                    )
                nmr = work.tile([P, NT], f32, tag="nmr")
                nc.vector.tensor_tensor(
                    nmr[:, t_lo:t_hi], mv[:, t_lo:t_hi, 0], rstd[:, t_lo:t_hi], op=OP.mult
                )
                nc.vector.tensor_scalar_mul(nmr[:, t_lo:t_hi], nmr[:, t_lo:t_hi], -1.0)
                for t in range(t_lo, t_hi):
                    if t % 2 == 0:
                        nc.vector.tensor_scalar(
                            out[:, t],
                            src[:, t],
                            mv[:, t, 0:1],
                            rstd[:, t : t + 1],
                            op0=OP.subtract,
                            op1=OP.mult,
                        )
                    else:
                        nc.scalar.activation(
                            out[:, t],
                            src[:, t],
                            AF.Identity,
                            bias=nmr[:, t : t + 1],
                            scale=rstd[:, t : t + 1],
                        )

            def transpose_tiles(src, dst, t_lo, t_hi):
                for t in range(t_lo, t_hi):
                    for dtt in range(DT):
                        eng = nc.sync if (t + dtt) % 2 == 0 else nc.scalar
                        eng.dma_start_transpose(
                            dst[:, dtt, t * P : (t + 1) * P],
                            src[:, t, dtt * P : (dtt + 1) * P],
                        )

            def all_reduce_chunked(y, scaled_add_dst):
                """AllReduce y [P,NT,D] bf16 in NCHUNK token chunks; add into
                scaled_add_dst (q_res) in place."""
                for c in range(NCHUNK):
                    t0, t1 = c * TC_PER_CHUNK, (c + 1) * TC_PER_CHUNK
                    yb = dram.tile([P, TC_PER_CHUNK, D], bf16, tag="arin")
                    ab = dram.tile([P, TC_PER_CHUNK, D], bf16, tag="arout")
                    nc.sync.dma_start(yb[:], y[:, t0:t1])
                    if SKIP_COLLECTIVE:
                        nc.gpsimd.dma_start(ab[:], yb[:])
                    else:
                        nc.gpsimd.collective_compute(
                            "AllReduce",
                            OP.add,
                            replica_groups=RG,
                            ins=[yb[:].opt()],
                            outs=[ab[:].opt()],
                        )
                    nc.gpsimd.dma_start(
                        scaled_add_dst[:, t0:t1],
                        ab[:],
                        accum_op=OP.add,
                    )

            def block_eval(k, dump=False):
                # ---- LN1 -> z -> zT
                layernorm(q_res, 0, NT, z_sb)
                transpose_tiles(z_sb, zt_sb, 0, NT)
                if dump:
                    nc.sync.dma_start(dbg["dbg_z"][:], z_sb[:])
                    nc.sync.dma_start(dbg["dbg_zt"][:], zt_sb[:])

                # ---- projections (q, k, v)
                phi = [None, None, None]
                for pi, wsb in enumerate((wq_sb, wk_sb, wv_sb)):
                    pp = ps_big.tile([P, 1024], f32, tag="big")
                    for nb in range(2):
                        for dtt in range(DT):
                            nc.tensor.matmul(
                                pp[:, nb * 512 : (nb + 1) * 512],
                                lhsT=wsb[:, k, dtt, :],
                                rhs=zt_sb[:, dtt, nb * 512 : (nb + 1) * 512],
                                start=(dtt == 0),
                                stop=(dtt == DT - 1),
                            )
                    tag = ("phiq", "phik", "vpt")[pi]
                    res = work.tile([P, N], bf16, tag=tag)
                    if pi < 2:
                        ts_t = work2.tile([P, N], bf16, tag="tsum")
                        nc.vector.tensor_tensor(
                            ts_t[:], pp[:], xw_sb[:, k, pi, :], op=OP.add
                        )
                        rel = work.tile([P, N], bf16, tag="rel")
                        nc.vector.tensor_scalar_max(rel[:], ts_t[:], 0.0)
                        mn = work.tile([P, N], bf16, tag="mn")
                        nc.vector.tensor_scalar_min(mn[:], ts_t[:], 0.0)
                        ex = work.tile([P, N], bf16, tag="ex")
                        nc.scalar.activation(ex[:], mn[:], AF.Exp)
                        nc.vector.tensor_tensor(res[:], ex[:], rel[:], op=OP.add)
                    else:
                        nc.vector.tensor_tensor(
                            res[:], pp[:], xw_sb[:, k, pi, :], op=OP.add
                        )
                        for mt in range(NT):
                            for h in range(2):
                                eng = nc.sync if mt % 2 == 0 else nc.scalar
                                eng.dma_start_transpose(
                                    vp_aug[:, mt, h, 0:64],
                                    res[h * 64 : (h + 1) * 64, mt * P : (mt + 1) * P],
                                )
                    phi[pi] = res
                phiq, phik, vpt = phi
                if dump:
                    nc.sync.dma_start(dbg["dbg_phiq"][:], phiq[:])
                    nc.sync.dma_start(dbg["dbg_phik"][:], phik[:])
                    nc.sync.dma_start(dbg["dbg_vpt"][:], vpt[:])

                # ---- attention: nb-outer, heads packed via tile_position
                for nb in range(2):
                    nsl = slice(nb * 512, (nb + 1) * 512)
                    for mt in range(NT):
                        for h in range(2):
                            hs = h * 64
                            pw = ps_sml.tile([P, 512], f32, tag="sml")
                            nc.tensor.matmul(
                                pw[:],
                                lhsT=phik[hs : hs + 64, mt * P : (mt + 1) * P],
                                rhs=phiq[hs : hs + 64, nsl],
                                start=True,
                                stop=True,
                            )
                            dst = wt_sb[:, mt, h, :]
                            if (mt + h) % 2 == 0:
                                nc.scalar.activation(dst, pw[:], AF.Square)
                            else:
                                wc = work2.tile([P, 512], bf16, tag="wc")
                                nc.vector.tensor_copy(wc[:], pw[:])
                                nc.gpsimd.tensor_tensor(dst, wc[:], wc[:], op=OP.mult)
                    pa = ps_sml.tile([P, 512], f32, tag="sml")
                    pss0 = ps_sml.tile([P, 512], f32, tag="sml")
                    pss1 = ps_sml.tile([P, 512], f32, tag="sml")
                    for mt in range(NT):
                        for h in range(2):
                            hs = h * 64
                            nc.tensor.matmul(
                                pa[hs : hs + 64, :],
                                lhsT=vp_aug[:, mt, h, :],
                                rhs=wt_sb[:, mt, h, :],
                                start=(mt == 0),
                                stop=(mt == NT - 1),
                                tile_position=(0, hs),
                                skip_group_check=True,
                            )
                            nc.tensor.matmul(
                                (pss0 if h == 0 else pss1)[0:1, :],
                                lhsT=onescol[:],
                                rhs=wt_sb[:, mt, h, :],
                                start=(mt == 0),
                                stop=(mt == NT - 1),
                            )
                    prr = work.tile([P, 512], bf16, tag="prr")
                    for h in range(2):
                        hs = h * 64
                        pss = pss0 if h == 0 else pss1
                        rr = work.tile([1, 512], f32, tag="rr")
                        nc.vector.tensor_scalar_add(rr[:], pss[0:1, :], 1.0)
                        nc.vector.reciprocal_approx_fast(rr[:], rr[:])
                        rrb = work.tile([1, 512], bf16, tag="rrb")
                        nc.vector.tensor_copy(rrb[:], rr[:])
                        rsc = dram.tile([1, 512], bf16, tag="rsc")
                        nc.sync.dma_start(rsc[:], rrb[:])
                        nc.scalar.dma_start(
                            prr[hs : hs + 64, :],
                            rsc[0][None, :].to_broadcast([64, 512]),
                        )
                    at = work.tile([P, 512], bf16, tag="atr")
                    if nb == 0:
                        nc.vector.tensor_copy(at[:], pa[:])
                    else:
                        nc.scalar.copy(at[:], pa[:])
                    tm = work.tile([P, 512], bf16, tag="tm")
                    nc.vector.tensor_tensor(tm[:], at[:], prr[:], op=OP.mult)
                    nc.gpsimd.tensor_tensor(
                        mcat_sb[:, nsl], tm[:], vpt[:, nsl], op=OP.subtract
                    )

                # ---- out-proj (wo pre-scaled by softplus(dt)) + chunked AR
                y = work.tile([P, NT, D], bf16, tag="y")
                for nt in range(NT):
                    po = ps_sml.tile([P, 512], f32, tag="sml")
                    nc.tensor.matmul(
                        po[:],
                        lhsT=mcat_sb[:, nt * P : (nt + 1) * P],
                        rhs=wo_sb[:, k, :],
                        start=True,
                        stop=True,
                    )
                    if nt % 2 == 0:
                        nc.vector.tensor_copy(y[:, nt], po[:])
                    else:
                        nc.scalar.copy(y[:, nt], po[:])
                if dump:
                    nc.sync.dma_start(dbg["dbg_mcat"][:], mcat_sb[:])
                    nc.sync.dma_start(dbg["dbg_y"][:], y[:])
                all_reduce_chunked(y, q_res)
                if dump:
                    nc.sync.dma_start(dbg["dbg_q1"][:], q_res[:])

                # ---- LN2 -> z2 -> z2T
                layernorm(q_res, 0, NT, z_sb)
                transpose_tiles(z_sb, zt_sb, 0, NT)

                # ---- up-proj + SwiGLU -> hf
                for nb in range(2):
                    for jp in range(CT):
                        pg = ps_sml.tile([P, 512], f32, tag="sml")
                        for dtt in range(DT):
                            nc.tensor.matmul(
                                pg[:],
                                lhsT=wup_sb[:, k, dtt, jp, :],
                                rhs=zt_sb[:, dtt, nb * 512 : (nb + 1) * 512],
                                start=(dtt == 0),
                                stop=(dtt == DT - 1),
                            )
                        pu = ps_sml.tile([P, 512], f32, tag="sml")
                        for dtt in range(DT):
                            nc.tensor.matmul(
                                pu[:],
                                lhsT=wup_sb[:, k, dtt, jp + CT, :],
                                rhs=zt_sb[:, dtt, nb * 512 : (nb + 1) * 512],
                                start=(dtt == 0),
                                stop=(dtt == DT - 1),
                            )
                        sg = work2.tile([P, 512], bf16, tag="sg")
                        nc.scalar.activation(
                            sg[:], pg[:], AF.Silu, bias=b2w_sb[:, k, jp : jp + 1]
                        )
                        uu = work2.tile([P, 512], bf16, tag="uu")
                        nc.vector.tensor_scalar(
                            uu[:], pu[:], b2w_sb[:, k, jp + CT : jp + CT + 1], None, op0=OP.add
                        )
                        nc.gpsimd.tensor_tensor(
                            hf_sb[:, jp, 1 + nb * 512 : 1 + (nb + 1) * 512],
                            sg[:],
                            uu[:],
                            op=OP.mult,
                        )

                # ---- depthwise conv (as 3 diag matmuls) + silu -> hcv
                hcv = work.tile([P, CT, N], bf16, tag="hcv")
                for ct in range(CT):
                    for nb in range(2):
                        pc = ps_sml.tile([P, 512], f32, tag="sml")
                        for tap in range(CK):
                            nc.tensor.matmul(
                                pc[:],
                                lhsT=diag_sb[:, k, ct, tap, :],
                                rhs=hf_sb[:, ct, nb * 512 + tap : nb * 512 + tap + 512],
                                start=(tap == 0),
                                stop=(tap == CK - 1),
                            )
                        si = nc.scalar.activation(
                            hcv[:, ct, nb * 512 : (nb + 1) * 512],
                            pc[:],
                            AF.Silu,
                            bias=dwb_sb[:, k, ct : ct + 1],
                        )
                        last_silu[0] = si

                if dump:
                    nc.sync.dma_start(dbg["dbg_hf"][:], hf_sb[:])
                    nc.sync.dma_start(dbg["dbg_hcv"][:], hcv[:])
                # ---- down-proj + chunked AR
                y2 = work.tile([P, NT, D], bf16, tag="y")
                for nt in range(NT):
                    pd = ps_sml.tile([P, 512], f32, tag="sml")
                    for ct in range(CT):
                        nc.tensor.matmul(
                            pd[:],
                            lhsT=hcv[:, ct, nt * P : (nt + 1) * P],
                            rhs=wdn_sb[:, k, ct, :],
                            start=(ct == 0),
                            stop=(ct == CT - 1),
                        )
                    if nt % 2 == 0:
                        nc.vector.tensor_copy(y2[:, nt], pd[:])
                    else:
                        nc.scalar.copy(y2[:, nt], pd[:])
                if dump:
                    nc.sync.dma_start(dbg["dbg_y2"][:], y2[:])
                all_reduce_chunked(y2, q_res)
                if dump:
                    nc.sync.dma_start(dbg["dbg_q2"][:], q_res[:])

            for _cyc in range(CYCLES):
                for k in range(K):
                    block_eval(k, dump=(DEBUG and _cyc == 0 and k == 0))

            # ---------------- final LN (with fin scale/bias) ----------------
            layernorm(q_res, 0, NT, z_sb)
            nc.vector.tensor_tensor(
                z_sb[:], z_sb[:], fins_bc[:, None, :].to_broadcast([P, NT, D]), op=OP.mult
            )
            nc.vector.tensor_tensor(
                z_sb[:], z_sb[:], finb_bc[:, None, :].to_broadcast([P, NT, D]), op=OP.add
            )
            transpose_tiles(z_sb, zft_sb, 0, NT)

            # ---- q_logits = mean_n(Qn) @ halt_w.T + halt_b
            qm = work.tile([P, DT], f32, tag="qm")
            nc.vector.reduce_sum(qm[:], zft_sb[:], axis=mybir.AxisListType.X)
            pq = ps_sml.tile([P, 512], f32, tag="sml")
            for dtt in range(DT):
                nc.tensor.matmul(
                    pq[0:1, 0:2],
                    lhsT=qm[:, dtt : dtt + 1],
                    rhs=hwt_sb[:, dtt, :],
                    start=(dtt == 0),
                    stop=(dtt == DT - 1),
                )
            ql = work.tile([1, 2], f32, tag="ql")
            nc.vector.tensor_scalar_mul(ql[:], pq[0:1, 0:2], 1.0 / N)
            nc.vector.tensor_tensor(ql[:], ql[:], hb_sb[:], op=OP.add)
            nc.sync.dma_start(qlog_d[:], ql[:])

        # ---------------- lm head (vocab-sharded) ----------------
        with tc.tile_pool(name="lmp", bufs=2) as lmp, tc.tile_pool(
            name="lmp1", bufs=1
        ) as lmp1:
            lg = logits_d[:].rearrange("(nt p) v -> p nt v", p=P)
            lmsrc = lmt_d[:].rearrange("(dt p) v -> p dt v", p=P)
            for vc in range(NVC):
                stage = lmp.tile([P, DT, VCH], f32, tag="lstage")
                nc.sync.dma_start(stage[:], lmsrc[:, :, vc * VCH : (vc + 1) * VCH])
                lc = lmp.tile([P, DT, VCH], bf16, tag="lc")
                if vc % 2 == 0:
                    nc.vector.tensor_copy(lc[:], stage[:])
                else:
                    nc.scalar.copy(lc[:], stage[:])
                ob = lmp1.tile([P, NT, VCH], f32, tag="ob")
                for nt in range(NT):
                    pl = ps_sml.tile([P, 512], f32, tag="sml")
                    for dtt in range(DT):
                        nc.tensor.matmul(
                            pl[:, 0:VCH],
                            lhsT=zft_sb[:, dtt, nt * P : (nt + 1) * P],
                            rhs=lc[:, dtt, :],
                            start=(dtt == 0),
                            stop=(dtt == DT - 1),
                        )
                    if (vc + nt) % 2 == 0:
                        nc.vector.tensor_copy(ob[:, nt, :], pl[:, 0:VCH])
                    else:
                        nc.scalar.copy(ob[:, nt, :], pl[:, 0:VCH])
                eng = nc.sync if vc % 2 == 0 else nc.scalar
                eng.dma_start(lg[:, :, vc * VCH : (vc + 1) * VCH], ob[:])

        dram.release()
        ps_sml.release()
        ps_big.release()
        pers.release()

    nc.compile()
    return nc


def _get_nc():
    if "nc" not in _CACHE:
        _CACHE["nc"] = _build()
    return _CACHE["nc"]


def _prep_in_maps(inputs):
    ii = {k: np.asarray(v) for k, v in inputs.items()}
    hm = ii["carry_halted"].astype(bool)
    ids = np.where(hm[:, None], ii["inputs"], ii["carry_inputs"]).astype(np.int32)
    init_h = ii["init_hidden"].astype(np.float32)
    q0 = np.where(
        hm[:, None, None],
        np.broadcast_to(init_h[None, None, :], (B, N, D)),
        ii["carry_hidden"].astype(np.float32),
    ).astype(np.float32)
    emb = np.ascontiguousarray(ii["emb"].astype(np.float32))
    posn = np.ascontiguousarray(ii["pos"].astype(np.float32)[:N])
    lmT = np.ascontiguousarray(ii["lm_w"].astype(np.float32).T)  # [D, V]
    hwT = np.ascontiguousarray(ii["halt_w"].astype(np.float32).T)  # [D, 2]
    hb = ii["halt_b"].astype(np.float32).reshape(1, 2)
    dtv = ii["dt"].astype(np.float32).reshape(1, K)
    wq = ii["W_Q"].astype(np.float32)
    wk = ii["W_K"].astype(np.float32)
    wv = ii["W_V"].astype(np.float32)
    wo = ii["W_O"].astype(np.float32)
    wup = ii["W_up"].astype(np.float32)
    dww = ii["dw_w"].astype(np.float32)[:, :, 0, :]  # [K, INNER, CK]
    dwb = ii["dw_b"].astype(np.float32)
    wdn = ii["W_down"].astype(np.float32)

    in_maps = []
    for c in range(8):
        b, g = c // GRP, c % GRP
        jlo = g * P  # head-col slice (2 heads x 64)
        clo = g * CT * P  # inner slice (384)
        m = {
            "ids": np.ascontiguousarray(ids[b]),
            "q0": np.ascontiguousarray(q0[b]),
            "emb": emb,
            "posn": posn,
            "ins_v": ii["in_s"].astype(np.float32),
            "inb_v": ii["in_b"].astype(np.float32),
            "fins_v": ii["fin_s"].astype(np.float32),
            "finb_v": ii["fin_b"].astype(np.float32),
            "dtv": dtv,
            "wq": np.ascontiguousarray(wq[:, :, jlo : jlo + P]),
            "wk": np.ascontiguousarray(wk[:, :, jlo : jlo + P]),
            "wv": np.ascontiguousarray(wv[:, :, jlo : jlo + P]),
            "wo": np.ascontiguousarray(wo[:, jlo : jlo + P, :]),
            "wup": np.ascontiguousarray(
                np.concatenate(
                    (
                        wup[:, :, clo : clo + CT * P],
                        wup[:, :, INNER + clo : INNER + clo + CT * P],
                    ),
                    axis=-1,
                )
            ),
            "dww": np.ascontiguousarray(dww[:, clo : clo + CT * P, :]),
            "dwb": np.ascontiguousarray(dwb[:, clo : clo + CT * P]),
            "wdn": np.ascontiguousarray(wdn[:, clo : clo + CT * P, :]),
            "n1s": ii["n1_s"].astype(np.float32),
            "n1b": ii["n1_b"].astype(np.float32),
            "n2s": ii["n2_s"].astype(np.float32),
            "n2b": ii["n2_b"].astype(np.float32),
            "lmt": np.ascontiguousarray(lmT[:, g * VC : (g + 1) * VC]),
            "hwt": hwT,
            "hb": hb,
        }
        in_maps.append(m)
    return in_maps


def _run_fast(nc, in_maps):
    """Cached jitted executor (avoids per-call jit retrace). Falls back to
    run_bass_kernel_spmd on any failure."""
    import jax
    import jax.numpy as jnp
    import concourse.mybir as mybir
    from jax.sharding import Mesh, PartitionSpec, NamedSharding
    from jax.experimental.shard_map import shard_map
    from concourse.bass2jax import (
        _bass_exec_p,
        partition_id_tensor,
        install_neuronx_cc_hook,
    )

    if "fast" not in _CACHE:
        install_neuronx_cc_hook()
        partition_name = (
            nc.partition_id_tensor.name if nc.partition_id_tensor else None
        )
        in_names, out_names, out_avals = [], [], []
        for alloc in nc.m.functions[0].allocations:
            if not isinstance(alloc, mybir.MemoryLocationSet):
                continue
            name = alloc.memorylocations[0].name
            if alloc.kind == "ExternalInput":
                if name != partition_name:
                    in_names.append(name)
            elif alloc.kind == "ExternalOutput":
                out_names.append(name)
                out_avals.append(
                    jax.core.ShapedArray(
                        tuple(alloc.tensor_shape), mybir.dt.np(alloc.dtype)
                    )
                )
        n_params = len(in_names)
        all_in = in_names + out_names + ([partition_name] if partition_name else [])

        def _body(*args):
            ins = list(args[:n_params])
            outs = list(args[n_params:])
            pid = [partition_id_tensor()] if partition_name else []
            return tuple(
                _bass_exec_p.bind(
                    *ins,
                    *outs,
                    *pid,
                    out_avals=tuple(out_avals),
                    in_names=tuple(all_in),
                    out_names=tuple(out_names),
                    lowering_input_output_aliases=(),
                    sim_require_finite=True,
                    sim_require_nnan=True,
                    nc=nc,
                )
            )

        devices = jax.devices()[:8]
        mesh = Mesh(np.asarray(devices), ("core",))
        n_outs = len(out_names)
        f = jax.jit(
            shard_map(
                _body,
                mesh=mesh,
                in_specs=(PartitionSpec("core"),) * (n_params + n_outs),
                out_specs=(PartitionSpec("core"),) * n_outs,
                check_rep=False,
            ),
            donate_argnums=tuple(range(n_params, n_params + n_outs)),
            keep_unused=True,
        )
        _CACHE["fast"] = (f, in_names, out_names, out_avals, mesh)
    f, in_names, out_names, out_avals, mesh = _CACHE["fast"]
    import jax

    sh = jax.sharding.NamedSharding(mesh, PartitionSpec("core"))
    concat_in = [
        np.concatenate([np.asarray(in_maps[c][nm]) for c in range(8)], axis=0)
        for nm in in_names
    ]
    dev_in = [jax.device_put(a, sh) for a in concat_in]
    zeros = [
        jax.device_put(np.zeros((av.shape[0] * 8,) + av.shape[1:], av.dtype), sh)
        for av in out_avals
    ]
    outs = f(*dev_in, *zeros)
    jax.block_until_ready(outs)
    res = []
    for c in range(8):
        m = {}
        for i, nm in enumerate(out_names):
            av = out_avals[i]
            m[nm] = np.asarray(outs[i])[c * av.shape[0] : (c + 1) * av.shape[0]]
        res.append(m)
    return res


def kernel(**inputs):
    from concourse.bass_utils import run_bass_kernel_spmd

    nc = _get_nc()
    in_maps = _prep_in_maps(inputs)
    try:
        res = _run_fast(nc, in_maps)
    except Exception:
        res = run_bass_kernel_spmd(nc, in_maps, core_ids=list(range(8))).results
    logits = np.zeros((B, N, V), np.float32)
    for c in range(8):
        b, g = c // GRP, c % GRP
        logits[b, :, g * VC : (g + 1) * VC] = res[c]["logits"]
    q_logits = np.stack([res[0]["qlog"][0], res[GRP]["qlog"][0]])
    return logits, q_logits


# revision 24
# speedup vs baseline: 3.4510x; 1.0065x over previous
"""Trainium2 Bass kernel for nn_AMKPDModel (linear-attention transformer,
K=4 blocks x 2 cycles, ConvSwiGLU FFN, 32k-vocab LM head) on 8 NeuronCores.

Sharding: 2 data-parallel groups of 4 cores (one per batch element).
Within a group: attention heads sharded 2/core, FFN inner dim sharded
384/core, lm_head vocab sharded 8000/core. Two group-local AllReduces
per block eval ([1024,512] bf16), chunked for compute/comm overlap.
"""

import sys

if "/opt/trn_rl_repo" not in sys.path:
    sys.path.insert(0, "/opt/trn_rl_repo")

import numpy as np

# model dims
B, N, D = 2, 1024, 512
K = 4
V = 32000
INNER = 1536
CK = 3
EPS = 1e-5
CYCLES = 2  # H_CYCLES runs of the 4-block stack

# sharding
GRP = 4            # cores per batch group
P = 128
NT = N // P        # 8 token tiles
DT = D // P        # 4 feature tiles
JT = 6             # up-proj 768/128 local tiles (3 G + 3 U)
CT = 3             # local inner tiles (384/128)
VC = V // GRP      # 8000 vocab rows per core
VCH = 500
NVC = VC // VCH    # 16 lm chunks
NCHUNK = 2         # AR chunks per block output (token-split)
TC_PER_CHUNK = NT // NCHUNK

RG = [[0, 1, 2, 3], [4, 5, 6, 7]]

_CACHE = {}
DEBUG = False
SKIP_COLLECTIVE = False  # timing-only: replace AR with local copy


def _build():
    import concourse.bass as bass
    import concourse.mybir as mybir
    import concourse.tile as tile
    from concourse import bacc
    from concourse.bass import _add_dep_helper
    from concourse.masks import make_identity

    f32 = mybir.dt.float32
    bf16 = mybir.dt.bfloat16
    i32 = mybir.dt.int32
    AF = mybir.ActivationFunctionType
    OP = mybir.AluOpType

    nc = bacc.Bacc(None, target_bir_lowering=False, debug=False, num_devices=8)

    # ---------------- DRAM params ----------------
    ids_d = nc.declare_dram_parameter("ids", [N], i32, isOutput=False)
    q0_d = nc.declare_dram_parameter("q0", [N, D], f32, isOutput=False)
    emb_d = nc.declare_dram_parameter("emb", [V, D], f32, isOutput=False)
    pos_d = nc.declare_dram_parameter("posn", [N, D], f32, isOutput=False)
    ins_d = nc.declare_dram_parameter("ins_v", [D], f32, isOutput=False)
    inb_d = nc.declare_dram_parameter("inb_v", [D], f32, isOutput=False)
    fins_d = nc.declare_dram_parameter("fins_v", [D], f32, isOutput=False)
    finb_d = nc.declare_dram_parameter("finb_v", [D], f32, isOutput=False)
    dtv_d = nc.declare_dram_parameter("dtv", [1, K], f32, isOutput=False)
    wq_d = nc.declare_dram_parameter("wq", [K, D, P], f32, isOutput=False)
    wk_d = nc.declare_dram_parameter("wk", [K, D, P], f32, isOutput=False)
    wv_d = nc.declare_dram_parameter("wv", [K, D, P], f32, isOutput=False)
    wo_d = nc.declare_dram_parameter("wo", [K, P, D], f32, isOutput=False)
    wup_d = nc.declare_dram_parameter("wup", [K, D, JT * P], f32, isOutput=False)
    dww_d = nc.declare_dram_parameter("dww", [K, CT * P, CK], f32, isOutput=False)
    dwb_d = nc.declare_dram_parameter("dwb", [K, CT * P], f32, isOutput=False)
    wdn_d = nc.declare_dram_parameter("wdn", [K, CT * P, D], f32, isOutput=False)
    n1s_d = nc.declare_dram_parameter("n1s", [K, D], f32, isOutput=False)
    n1b_d = nc.declare_dram_parameter("n1b", [K, D], f32, isOutput=False)
    n2s_d = nc.declare_dram_parameter("n2s", [K, D], f32, isOutput=False)
    n2b_d = nc.declare_dram_parameter("n2b", [K, D], f32, isOutput=False)
    lmt_d = nc.declare_dram_parameter("lmt", [D, VC], f32, isOutput=False)
    hwt_d = nc.declare_dram_parameter("hwt", [D, 2], f32, isOutput=False)
    hb_d = nc.declare_dram_parameter("hb", [1, 2], f32, isOutput=False)

    logits_d = nc.declare_dram_parameter("logits", [N, VC], f32, isOutput=True)
    qlog_d = nc.declare_dram_parameter("qlog", [1, 2], f32, isOutput=True)
    dbg = {}
    if DEBUG:
        bf16_ = mybir.dt.bfloat16
        for nm, shp, dt_ in (
            ("dbg_xb", [P, NT, D], bf16_),
            ("dbg_z", [P, NT, D], bf16_),
            ("dbg_zt", [P, DT, N], bf16_),
            ("dbg_phiq", [P, N], bf16_),
            ("dbg_phik", [P, N], bf16_),
            ("dbg_vpt", [P, N], bf16_),
            ("dbg_wt", [P, NT, N], bf16_),
            ("dbg_mcat", [P, N], bf16_),
            ("dbg_y", [P, NT, D], bf16_),
            ("dbg_q1", [P, NT, D], f32),
            ("dbg_hf", [P, CT, N + 2], bf16_),
            ("dbg_hcv", [P, CT, N], bf16_),
            ("dbg_y2", [P, NT, D], bf16_),
            ("dbg_q2", [P, NT, D], f32),
        ):
            dbg[nm] = nc.declare_dram_parameter(nm, shp, dt_, isOutput=True)

    with tile.TileContext(nc) as tc:
        pers = tc.alloc_tile_pool(name="pers", bufs=1)
        ps_big = tc.alloc_tile_pool(name="psb", bufs=1, space="PSUM")
        ps_sml = tc.alloc_tile_pool(name="pss", bufs=6, space="PSUM")
        dram = tc.alloc_tile_pool(name="dram", bufs=4, space="DRAM")

        # ---------------- persistent tiles ----------------
        eps_t = pers.tile([P, 1], f32, tag="eps")
        nc.vector.memset(eps_t[:], EPS)
        ones1 = pers.tile([1, P], bf16, tag="ones1")
        nc.vector.memset(ones1[:], 1.0)
        ident = pers.tile([P, P], bf16, tag="ident")
        make_identity(nc, ident[:])

        ids_sb = pers.tile([P, NT], i32, tag="ids")
        nc.sync.dma_start(ids_sb[:], ids_d[:].rearrange("(t p) -> p t", p=P))
        q_res = pers.tile([P, NT, D], f32, tag="qres")
        nc.sync.dma_start(q_res[:], q0_d[:].rearrange("(t p) d -> p t d", p=P))

        def bcast_row(src_d, tag):
            t = pers.tile([P, D], f32, tag=tag)
            nc.sync.dma_start(t[:], src_d[:][None, :].to_broadcast([P, D]))
            return t

        fins_bc = bcast_row(fins_d, "finsbc")
        finb_bc = bcast_row(finb_d, "finbbc")

        # softplus(dt) broadcast to all partitions
        dtv_sb = pers.tile([1, K], f32, tag="dtv")
        nc.sync.dma_start(dtv_sb[:], dtv_d[:])
        spe = pers.tile([1, K], f32, tag="spe")
        nc.scalar.activation(spe[:], dtv_sb[:], AF.Exp)
        nc.vector.tensor_scalar_add(spe[:], spe[:], 1.0)
        nc.scalar.activation(spe[:], spe[:], AF.Ln)
        sp_bc = pers.tile([P, K], f32, tag="spbc")
        nc.gpsimd.partition_broadcast(sp_bc[:], spe[:])

        # per-block norm scale vectors (transposed layouts)
        n1s_sb = pers.tile([P, K, DT], f32, tag="n1s")
        nc.sync.dma_start(n1s_sb[:], n1s_d[:].rearrange("k (dt p) -> p k dt", p=P))
        n2s_sb = pers.tile([P, K, DT], f32, tag="n2s")
        nc.sync.dma_start(n2s_sb[:], n2s_d[:].rearrange("k (dt p) -> p k dt", p=P))

        # weights (loaded f32, cast bf16)
        wq_sb = pers.tile([P, K, DT, P], bf16, tag="wq")
        wk_sb = pers.tile([P, K, DT, P], bf16, tag="wk")
        wv_sb = pers.tile([P, K, DT, P], bf16, tag="wv")
        wo_sb = pers.tile([P, K, D], bf16, tag="wo")
        wup_sb = pers.tile([P, K, DT, JT, P], bf16, tag="wup")
        wdn_sb = pers.tile([P, K, CT, D], bf16, tag="wdn")
        dww_sb = pers.tile([P, K, CT, CK], f32, tag="dww")
        nc.sync.dma_start(dww_sb[:], dww_d[:].rearrange("k (ct p) c -> p k ct c", p=P))
        dwb_sb = pers.tile([P, K, CT], f32, tag="dwb")
        nc.sync.dma_start(dwb_sb[:], dwb_d[:].rearrange("k (ct p) -> p k ct", p=P))
        diag_sb = pers.tile([P, K, CT, CK, P], bf16, tag="diag")
        b1t_sb = pers.tile([P, DT, K], bf16, tag="b1t")
        b2t_sb = pers.tile([P, DT, K], bf16, tag="b2t")
        b2w_sb = pers.tile([P, K, JT], f32, tag="b2w")
        hwt_sb = pers.tile([P, DT, 2], f32, tag="hwt")
        nc.sync.dma_start(hwt_sb[:], hwt_d[:].rearrange("(dt p) c -> p dt c", p=P))
        hb_sb = pers.tile([1, 2], f32, tag="hb")
        nc.sync.dma_start(hb_sb[:], hb_d[:])

        xw_sb = pers.tile([P, K, 3, N], bf16, tag="xw")
        vp_aug = pers.tile([P, NT, 2, 64], bf16, tag="vpaug")
        onescol = pers.tile([P, 1], bf16, tag="onescol")
        nc.vector.memset(onescol[:], 1.0)
        wt_sb = pers.tile([P, NT, 2, 512], bf16, tag="wt")  # both heads, one n-block
        hf_sb = pers.tile([P, CT, N + 2], bf16, tag="hf")
        nc.vector.memset(hf_sb[:], 0.0)
        zft_sb = pers.tile([P, DT, N], bf16, tag="zft")

        # z / zt shared across LN sites (persistent; evals are serial anyway)
        z_sb = pers.tile([P, NT, D], bf16, tag="z")
        zt_sb = pers.tile([P, DT, N], bf16, tag="zt")
        mcat_sb = pers.tile([P, N], bf16, tag="mcat")

        # ---------------- setup (scoped transients) ----------------
        with tc.tile_pool(name="setup", bufs=1) as setup, tc.tile_pool(
            name="setup2", bufs=2
        ) as setup2:
            # cast helper: DMA f32 -> stage, cast to dst (bf16)
            def load_cast(dst_ap, src_ap, shape, tag, eng):
                s = setup.tile(shape, f32, tag="wstage")
                nc.sync.dma_start(s[:], src_ap)
                if eng == 0:
                    nc.vector.tensor_copy(dst_ap, s[:])
                else:
                    nc.scalar.copy(dst_ap, s[:])

            for i, (dst, src) in enumerate(
                ((wq_sb, wq_d), (wk_sb, wk_d), (wv_sb, wv_d))
            ):
                load_cast(
                    dst[:],
                    src[:].rearrange("k (dt p) j -> p k dt j", p=P),
                    [P, K, DT, P],
                    "wstage",
                    i % 2,
                )
            load_cast(
                wo_sb[:],
                wo_d[:].rearrange("k p d -> p k d"),
                [P, K, D],
                "wstage",
                1,
            )
            for k in range(K):
                load_cast(
                    wup_sb[:, k],
                    wup_d[k].rearrange("(dt p) (jt jj) -> p dt jt jj", p=P, jj=P),
                    [P, DT, JT, P],
                    "wstage",
                    k % 2,
                )
                load_cast(
                    wdn_sb[:, k],
                    wdn_d[k].rearrange("(ct p) d -> p ct d", p=P),
                    [P, CT, D],
                    "wstage",
                    (k + 1) % 2,
                )
            for bd, bt in ((n1b_d, b1t_sb), (n2b_d, b2t_sb)):
                bcol = setup2.tile([P, DT, K], f32, tag="bcol")
                for k in range(K):
                    for dtt in range(DT):
                        nc.sync.dma_start(
                            bcol[:, dtt, k : k + 1],
                            bd[k, dtt * P : (dtt + 1) * P][:, None],
                        )
                nc.vector.tensor_copy(bt[:], bcol[:])

            # depthwise conv as matmul: diagonal weight mats per (k, ct, tap)
            for k in range(K):
                for ct in range(CT):
                    for tap in range(CK):
                        nc.vector.tensor_scalar_mul(
                            diag_sb[:, k, ct, tap, :],
                            ident[:],
                            dww_sb[:, k, ct, tap : tap + 1],
                        )

            # ---- embedding + input LN -> X (bf16) -> XT
            ins_bc = setup.tile([P, D], f32, tag="insbc")
            nc.sync.dma_start(ins_bc[:], ins_d[:][None, :].to_broadcast([P, D]))
            inb_bc = setup.tile([P, D], f32, tag="inbbc")
            nc.sync.dma_start(inb_bc[:], inb_d[:][None, :].to_broadcast([P, D]))

            xb_sb = setup.tile([P, NT, D], bf16, tag="xb0")
            st = setup.tile([P, NT, 6], f32, tag="st0")
            mv = setup.tile([P, NT, 2], f32, tag="mv0")
            rstd = setup.tile([P, NT], f32, tag="rstd0")
            for t in range(NT):
                xrt = setup2.tile([P, D], f32, tag="xrt")
                nc.gpsimd.indirect_dma_start(
                    out=xrt[:],
                    out_offset=None,
                    in_=emb_d[:],
                    in_offset=bass.IndirectOffsetOnAxis(ap=ids_sb[:, t : t + 1], axis=0),
                )
                ps = setup2.tile([P, D], f32, tag="posst")
                nc.sync.dma_start(
                    ps[:], pos_d[:].rearrange("(t p) d -> p t d", p=P)[:, t, :]
                )
                nc.vector.tensor_tensor(xrt[:], xrt[:], ps[:], op=OP.add)
                nc.vector.bn_stats(st[:, t], xrt[:])
                nc.vector.bn_aggr(mv[:, t], st[:, t])
                nc.vector.tensor_scalar_add(rstd[:, t : t + 1], mv[:, t, 1:2], EPS)
                nc.vector.reciprocal_approx_fast(rstd[:, t : t + 1], rstd[:, t : t + 1])
                nc.scalar.activation(rstd[:, t : t + 1], rstd[:, t : t + 1], AF.Sqrt)
                nc.vector.tensor_scalar(
                    xb_sb[:, t],
                    xrt[:],
                    mv[:, t, 0:1],
                    rstd[:, t : t + 1],
                    op0=OP.subtract,
                    op1=OP.mult,
                )
                nc.vector.tensor_tensor(xb_sb[:, t], xb_sb[:, t], ins_bc[:], op=OP.mult)
                nc.vector.tensor_tensor(xb_sb[:, t], xb_sb[:, t], inb_bc[:], op=OP.add)
            if DEBUG:
                nc.sync.dma_start(dbg["dbg_xb"][:], xb_sb[:])
            xt_sb = setup.tile([P, DT, N], bf16, tag="xt")
            for t in range(NT):
                for dtt in range(DT):
                    nc.sync.dma_start_transpose(
                        xt_sb[:, dtt, t * P : (t + 1) * P],
                        xb_sb[:, t, dtt * P : (dtt + 1) * P],
                    )

            # ---- XW = (b1 + X) @ W  (raw weights, before s1 fold)
            for k in range(K):
                for pi, wsb in enumerate((wq_sb, wk_sb, wv_sb)):
                    pb = ps_sml.tile([P, 512], f32, tag="sml")
                    for dtt in range(DT):
                        nc.tensor.matmul(
                            pb[:, 0:1],
                            lhsT=wsb[:, k, dtt, :],
                            rhs=b1t_sb[:, dtt, k : k + 1],
                            start=(dtt == 0),
                            stop=(dtt == DT - 1),
                        )
                    b1w = setup2.tile([P, 1], f32, tag="b1w")
                    nc.vector.tensor_copy(b1w[:], pb[:, 0:1])
                    for nb in range(2):
                        pp = ps_sml.tile([P, 512], f32, tag="sml")
                        for dtt in range(DT):
                            nc.tensor.matmul(
                                pp[:],
                                lhsT=wsb[:, k, dtt, :],
                                rhs=xt_sb[:, dtt, nb * 512 : (nb + 1) * 512],
                                start=(dtt == 0),
                                stop=(dtt == DT - 1),
                            )
                        nc.vector.tensor_scalar(
                            xw_sb[:, k, pi, nb * 512 : (nb + 1) * 512],
                            pp[:],
                            b1w[:],
                            None,
                            op0=OP.add,
                        )

            # ---- b2W per (k, jt) (raw wup)
            for k in range(K):
                for jt in range(JT):
                    pb = ps_sml.tile([P, 512], f32, tag="sml")
                    for dtt in range(DT):
                        nc.tensor.matmul(
                            pb[:, 0:1],
                            lhsT=wup_sb[:, k, dtt, jt, :],
                            rhs=b2t_sb[:, dtt, k : k + 1],
                            start=(dtt == 0),
                            stop=(dtt == DT - 1),
                        )
                    eng = nc.vector if (k + jt) % 2 == 0 else nc.scalar
                    if eng is nc.vector:
                        nc.vector.tensor_copy(b2w_sb[:, k, jt : jt + 1], pb[:, 0:1])
                    else:
                        nc.scalar.copy(b2w_sb[:, k, jt : jt + 1], pb[:, 0:1])

            # ---- fold s1 into wq/wk/wv, s2 into wup, softplus(dt) into wo
            for wsb in (wq_sb, wk_sb, wv_sb):
                for k in range(K):
                    nc.vector.tensor_tensor(
                        wsb[:, k],
                        wsb[:, k],
                        n1s_sb[:, k, :, None].to_broadcast([P, DT, P]),
                        op=OP.mult,
                    )
            for k in range(K):
                nc.vector.tensor_tensor(
                    wup_sb[:, k],
                    wup_sb[:, k],
                    n2s_sb[:, k, :, None, None].to_broadcast([P, DT, JT, P]),
                    op=OP.mult,
                )
                nc.vector.tensor_tensor(
                    wo_sb[:, k],
                    wo_sb[:, k],
                    sp_bc[:, k : k + 1].to_broadcast([P, D]),
                    op=OP.mult,
                )

        # ---------------- block evals ----------------
        with tc.tile_pool(name="work", bufs=1) as work, tc.tile_pool(
            name="work2", bufs=2
        ) as work2:

            last_silu = [None]

            def layernorm(src, t_lo, t_hi, out, scale_bc=None, bias_bc=None):
                """LN over d for token tiles [t_lo, t_hi) of src [P,NT,D] f32.
                Writes (x-mu)*rstd (optionally *scale+bias) to out (bf16)."""
                nt = t_hi - t_lo
                st = work.tile([P, NT, 6], f32, tag="st")
                mv = work.tile([P, NT, 2], f32, tag="mv")
                for t in range(t_lo, t_hi):
                    nc.vector.bn_stats(st[:, t], src[:, t])
                    nc.vector.bn_aggr(mv[:, t], st[:, t])
                rstd = work.tile([P, NT], f32, tag="rstd")
                nc.vector.tensor_scalar_add(
                    rstd[:, t_lo:t_hi], mv[:, t_lo:t_hi, 1], EPS
                )
                nc.vector.reciprocal_approx_fast(rstd[:, t_lo:t_hi], rstd[:, t_lo:t_hi])
                sq_i = nc.scalar.activation(
                    rstd[:, t_lo:t_hi], rstd[:, t_lo:t_hi], AF.Sqrt
                )
                if last_silu[0] is not None:
                    _add_dep_helper(
                        sq_i.ins,
                        last_silu[0].ins,
                        sync=False,
                        reason="act-table grouping",
                    )
                nmr = work.tile([P, NT], f32, tag="nmr")
                nc.vector.tensor_tensor(
                    nmr[:, t_lo:t_hi], mv[:, t_lo:t_hi, 0], rstd[:, t_lo:t_hi], op=OP.mult
                )
                nc.vector.tensor_scalar_mul(nmr[:, t_lo:t_hi], nmr[:, t_lo:t_hi], -1.0)
                for t in range(t_lo, t_hi):
                    if t % 2 == 0:
                        nc.vector.tensor_scalar(
                            out[:, t],
                            src[:, t],
                            mv[:, t, 0:1],
                            rstd[:, t : t + 1],
                            op0=OP.subtract,
                            op1=OP.mult,
                        )
                    else:
                        nc.scalar.activation(
                            out[:, t],
                            src[:, t],
                            AF.Identity,
                            bias=nmr[:, t : t + 1],
                            scale=rstd[:, t : t + 1],
                        )

            def transpose_tiles(src, dst, t_lo, t_hi):
                for t in range(t_lo, t_hi):
                    for dtt in range(DT):
                        eng = nc.sync if (t + dtt) % 2 == 0 else nc.scalar
                        eng.dma_start_transpose(
                            dst[:, dtt, t * P : (t + 1) * P],
                            src[:, t, dtt * P : (dtt + 1) * P],
                        )

            def all_reduce_chunked(y, scaled_add_dst):
                """AllReduce y [P,NT,D] bf16 in NCHUNK token chunks; add into
                scaled_add_dst (q_res) in place."""
                for c in range(NCHUNK):
                    t0, t1 = c * TC_PER_CHUNK, (c + 1) * TC_PER_CHUNK
                    yb = dram.tile([P, TC_PER_CHUNK, D], bf16, tag="arin")
                    ab = dram.tile([P, TC_PER_CHUNK, D], bf16, tag="arout")
                    nc.sync.dma_start(yb[:], y[:, t0:t1])
                    if SKIP_COLLECTIVE:
                        nc.gpsimd.dma_start(ab[:], yb[:])
                    else:
                        nc.gpsimd.collective_compute(
                            "AllReduce",
                            OP.add,
                            replica_groups=RG,
                            ins=[yb[:].opt()],
                            outs=[ab[:].opt()],
                        )
                    nc.gpsimd.dma_start(
                        scaled_add_dst[:, t0:t1],
                        ab[:],
                        accum_op=OP.add,
                    )

            def block_eval(k, dump=False):
                # ---- LN1 -> z -> zT
                layernorm(q_res, 0, NT, z_sb)
                transpose_tiles(z_sb, zt_sb, 0, NT)
                if dump:
                    nc.sync.dma_start(dbg["dbg_z"][:], z_sb[:])
                    nc.sync.dma_start(dbg["dbg_zt"][:], zt_sb[:])

                # ---- projections (q, k, v)
                phi = [None, None, None]
                mn2 = work.tile([P, 2, N], bf16, tag="mn")
                rel2 = work.tile([P, 2, N], bf16, tag="rel")
                for pi, wsb in enumerate((wq_sb, wk_sb, wv_sb)):
                    pp = ps_big.tile([P, 1024], f32, tag="big")
                    for nb in range(2):
                        for dtt in range(DT):
                            nc.tensor.matmul(
                                pp[:, nb * 512 : (nb + 1) * 512],
                                lhsT=wsb[:, k, dtt, :],
                                rhs=zt_sb[:, dtt, nb * 512 : (nb + 1) * 512],
                                start=(dtt == 0),
                                stop=(dtt == DT - 1),
                            )
                    tag = ("phiq", "phik", "vpt")[pi]
                    res = work.tile([P, N], bf16, tag=tag)
                    if pi < 2:
                        ts_t = work2.tile([P, N], bf16, tag="tsum")
                        nc.vector.tensor_tensor(
                            ts_t[:], pp[:], xw_sb[:, k, pi, :], op=OP.add
                        )
                        nc.vector.tensor_scalar_max(rel2[:, pi, :], ts_t[:], 0.0)
                        nc.vector.tensor_scalar_min(mn2[:, pi, :], ts_t[:], 0.0)
                        phi[pi] = res
                    else:
                        nc.vector.tensor_tensor(
                            res[:], pp[:], xw_sb[:, k, pi, :], op=OP.add
                        )
                        for mt in range(NT):
                            for h in range(2):
                                eng = nc.sync if mt % 2 == 0 else nc.scalar
                                eng.dma_start_transpose(
                                    vp_aug[:, mt, h, 0:64],
                                    res[h * 64 : (h + 1) * 64, mt * P : (mt + 1) * P],
                                )
                    phi[pi] = res
                ex2 = work.tile([P, 2, N], bf16, tag="ex")
                nc.scalar.activation(ex2[:], mn2[:], AF.Exp)
                for pi in range(2):
                    nc.vector.tensor_tensor(
                        phi[pi][:], ex2[:, pi, :], rel2[:, pi, :], op=OP.add
                    )
                phiq, phik, vpt = phi
                if dump:
                    nc.sync.dma_start(dbg["dbg_phiq"][:], phiq[:])
                    nc.sync.dma_start(dbg["dbg_phik"][:], phik[:])
                    nc.sync.dma_start(dbg["dbg_vpt"][:], vpt[:])

                # ---- attention: nb-outer, heads packed via tile_position
                for nb in range(2):
                    nsl = slice(nb * 512, (nb + 1) * 512)
                    for mt in range(NT):
                        for h in range(2):
                            hs = h * 64
                            pw = ps_sml.tile([P, 512], f32, tag="sml")
                            nc.tensor.matmul(
                                pw[:],
                                lhsT=phik[hs : hs + 64, mt * P : (mt + 1) * P],
                                rhs=phiq[hs : hs + 64, nsl],
                                start=True,
                                stop=True,
                            )
                            dst = wt_sb[:, mt, h, :]
                            if (mt + h) % 2 == 0:
                                nc.scalar.activation(dst, pw[:], AF.Square)
                            else:
                                wc = work2.tile([P, 512], bf16, tag="wc")
                                nc.vector.tensor_copy(wc[:], pw[:])
                                nc.gpsimd.tensor_tensor(dst, wc[:], wc[:], op=OP.mult)
                    pa = ps_sml.tile([P, 512], f32, tag="sml")
                    pss0 = ps_sml.tile([P, 512], f32, tag="sml")
                    pss1 = ps_sml.tile([P, 512], f32, tag="sml")
                    for mt in range(NT):
                        for h in range(2):
                            hs = h * 64
                            nc.tensor.matmul(
                                pa[hs : hs + 64, :],
                                lhsT=vp_aug[:, mt, h, :],
                                rhs=wt_sb[:, mt, h, :],
                                start=(mt == 0),
                                stop=(mt == NT - 1),
                                tile_position=(0, hs),
                                skip_group_check=True,
                            )
                            nc.tensor.matmul(
                                (pss0 if h == 0 else pss1)[0:1, :],
                                lhsT=onescol[:],
                                rhs=wt_sb[:, mt, h, :],
                                start=(mt == 0),
                                stop=(mt == NT - 1),
                            )
                    prr = work.tile([P, 512], bf16, tag="prr")
                    for h in range(2):
                        hs = h * 64
                        pss = pss0 if h == 0 else pss1
                        rr = work.tile([1, 512], f32, tag="rr")
                        nc.vector.tensor_scalar_add(rr[:], pss[0:1, :], 1.0)
                        nc.vector.reciprocal_approx_fast(rr[:], rr[:])
                        rrb = work.tile([1, 512], bf16, tag="rrb")
                        nc.vector.tensor_copy(rrb[:], rr[:])
                        rsc = dram.tile([1, 512], bf16, tag="rsc")
                        nc.sync.dma_start(rsc[:], rrb[:])
                        nc.scalar.dma_start(
                            prr[hs : hs + 64, :],
                            rsc[0][None, :].to_broadcast([64, 512]),
                        )
                    at = work.tile([P, 512], bf16, tag="atr")
                    if nb == 0:
                        nc.vector.tensor_copy(at[:], pa[:])
                    else:
                        nc.scalar.copy(at[:], pa[:])
                    tm = work.tile([P, 512], bf16, tag="tm")
                    nc.vector.tensor_tensor(tm[:], at[:], prr[:], op=OP.mult)
                    nc.gpsimd.tensor_tensor(
                        mcat_sb[:, nsl], tm[:], vpt[:, nsl], op=OP.subtract
                    )

                # ---- out-proj (wo pre-scaled by softplus(dt)) + chunked AR
                y = work.tile([P, NT, D], bf16, tag="y")
                for nt in range(NT):
                    po = ps_sml.tile([P, 512], f32, tag="sml")
                    nc.tensor.matmul(
                        po[:],
                        lhsT=mcat_sb[:, nt * P : (nt + 1) * P],
                        rhs=wo_sb[:, k, :],
                        start=True,
                        stop=True,
                    )
                    if nt % 2 == 0:
                        nc.vector.tensor_copy(y[:, nt], po[:])
                    else:
                        nc.scalar.copy(y[:, nt], po[:])
                if dump:
                    nc.sync.dma_start(dbg["dbg_mcat"][:], mcat_sb[:])
                    nc.sync.dma_start(dbg["dbg_y"][:], y[:])
                all_reduce_chunked(y, q_res)
                if dump:
                    nc.sync.dma_start(dbg["dbg_q1"][:], q_res[:])

                # ---- LN2 -> z2 -> z2T
                layernorm(q_res, 0, NT, z_sb)
                transpose_tiles(z_sb, zt_sb, 0, NT)

                # ---- up-proj + SwiGLU -> hf
                for nb in range(2):
                    for jp in range(CT):
                        pg = ps_sml.tile([P, 512], f32, tag="sml")
                        for dtt in range(DT):
                            nc.tensor.matmul(
                                pg[:],
                                lhsT=wup_sb[:, k, dtt, jp, :],
                                rhs=zt_sb[:, dtt, nb * 512 : (nb + 1) * 512],
                                start=(dtt == 0),
                                stop=(dtt == DT - 1),
                            )
                        pu = ps_sml.tile([P, 512], f32, tag="sml")
                        for dtt in range(DT):
                            nc.tensor.matmul(
                                pu[:],
                                lhsT=wup_sb[:, k, dtt, jp + CT, :],
                                rhs=zt_sb[:, dtt, nb * 512 : (nb + 1) * 512],
                                start=(dtt == 0),
                                stop=(dtt == DT - 1),
                            )
                        sg = work2.tile([P, 512], bf16, tag="sg")
                        nc.scalar.activation(
                            sg[:], pg[:], AF.Silu, bias=b2w_sb[:, k, jp : jp + 1]
                        )
                        uu = work2.tile([P, 512], bf16, tag="uu")
                        nc.vector.tensor_scalar(
                            uu[:], pu[:], b2w_sb[:, k, jp + CT : jp + CT + 1], None, op0=OP.add
                        )
                        nc.gpsimd.tensor_tensor(
                            hf_sb[:, jp, 1 + nb * 512 : 1 + (nb + 1) * 512],
                            sg[:],
                            uu[:],
                            op=OP.mult,
                        )

                # ---- depthwise conv (as 3 diag matmuls) + silu -> hcv
                hcv = work.tile([P, CT, N], bf16, tag="hcv")
                for ct in range(CT):
                    for nb in range(2):
                        pc = ps_sml.tile([P, 512], f32, tag="sml")
                        for tap in range(CK):
                            nc.tensor.matmul(
                                pc[:],
                                lhsT=diag_sb[:, k, ct, tap, :],
                                rhs=hf_sb[:, ct, nb * 512 + tap : nb * 512 + tap + 512],
                                start=(tap == 0),
                                stop=(tap == CK - 1),
                            )
                        si = nc.scalar.activation(
                            hcv[:, ct, nb * 512 : (nb + 1) * 512],
                            pc[:],
                            AF.Silu,
                            bias=dwb_sb[:, k, ct : ct + 1],
                        )
                        last_silu[0] = si

                if dump:
                    nc.sync.dma_start(dbg["dbg_hf"][:], hf_sb[:])
                    nc.sync.dma_start(dbg["dbg_hcv"][:], hcv[:])
                # ---- down-proj + chunked AR
                y2 = work.tile([P, NT, D], bf16, tag="y")
                for nt in range(NT):
                    pd = ps_sml.tile([P, 512], f32, tag="sml")
                    for ct in range(CT):
                        nc.tensor.matmul(
                            pd[:],
                            lhsT=hcv[:, ct, nt * P : (nt + 1) * P],
                            rhs=wdn_sb[:, k, ct, :],
                            start=(ct == 0),
                            stop=(ct == CT - 1),
                        )
                    if nt % 2 == 0:
                        nc.vector.tensor_copy(y2[:, nt], pd[:])
                    else:
                        nc.scalar.copy(y2[:, nt], pd[:])
                if dump:
                    nc.sync.dma_start(dbg["dbg_y2"][:], y2[:])
                all_reduce_chunked(y2, q_res)
                if dump:
                    nc.sync.dma_start(dbg["dbg_q2"][:], q_res[:])

            for _cyc in range(CYCLES):
                for k in range(K):
                    block_eval(k, dump=(DEBUG and _cyc == 0 and k == 0))

            # ---------------- final LN (with fin scale/bias) ----------------
            layernorm(q_res, 0, NT, z_sb)
            nc.vector.tensor_tensor(
                z_sb[:], z_sb[:], fins_bc[:, None, :].to_broadcast([P, NT, D]), op=OP.mult
            )
            nc.vector.tensor_tensor(
                z_sb[:], z_sb[:], finb_bc[:, None, :].to_broadcast([P, NT, D]), op=OP.add
            )
            transpose_tiles(z_sb, zft_sb, 0, NT)

            # ---- q_logits = mean_n(Qn) @ halt_w.T + halt_b
            qm = work.tile([P, DT], f32, tag="qm")
            nc.vector.reduce_sum(qm[:], zft_sb[:], axis=mybir.AxisListType.X)
            pq = ps_sml.tile([P, 512], f32, tag="sml")
            for dtt in range(DT):
                nc.tensor.matmul(
                    pq[0:1, 0:2],
                    lhsT=qm[:, dtt : dtt + 1],
                    rhs=hwt_sb[:, dtt, :],
                    start=(dtt == 0),
                    stop=(dtt == DT - 1),
                )
            ql = work.tile([1, 2], f32, tag="ql")
            nc.vector.tensor_scalar_mul(ql[:], pq[0:1, 0:2], 1.0 / N)
            nc.vector.tensor_tensor(ql[:], ql[:], hb_sb[:], op=OP.add)
            nc.sync.dma_start(qlog_d[:], ql[:])

        # ---------------- lm head (vocab-sharded) ----------------
        with tc.tile_pool(name="lmp", bufs=2) as lmp, tc.tile_pool(
            name="lmp1", bufs=1
        ) as lmp1:
            lg = logits_d[:].rearrange("(nt p) v -> p nt v", p=P)
            lmsrc = lmt_d[:].rearrange("(dt p) v -> p dt v", p=P)
            for vc in range(NVC):
                stage = lmp.tile([P, DT, VCH], f32, tag="lstage")
                nc.sync.dma_start(stage[:], lmsrc[:, :, vc * VCH : (vc + 1) * VCH])
                lc = lmp.tile([P, DT, VCH], bf16, tag="lc")
                if vc % 2 == 0:
                    nc.vector.tensor_copy(lc[:], stage[:])
                else:
                    nc.scalar.copy(lc[:], stage[:])
                ob = lmp1.tile([P, NT, VCH], f32, tag="ob")
                for nt in range(NT):
                    pl = ps_sml.tile([P, 512], f32, tag="sml")
                    for dtt in range(DT):
                        nc.tensor.matmul(
                            pl[:, 0:VCH],
                            lhsT=zft_sb[:, dtt, nt * P : (nt + 1) * P],
                            rhs=lc[:, dtt, :],
                            start=(dtt == 0),
                            stop=(dtt == DT - 1),
                        )
                    if (vc + nt) % 2 == 0:
                        nc.vector.tensor_copy(ob[:, nt, :], pl[:, 0:VCH])
                    else:
                        nc.scalar.copy(ob[:, nt, :], pl[:, 0:VCH])
                eng = nc.sync if vc % 2 == 0 else nc.scalar
                eng.dma_start(lg[:, :, vc * VCH : (vc + 1) * VCH], ob[:])

        dram.release()
        ps_sml.release()
        ps_big.release()
        pers.release()

    nc.compile()
    return nc


def _get_nc():
    if "nc" not in _CACHE:
        _CACHE["nc"] = _build()
    return _CACHE["nc"]


def _prep_in_maps(inputs):
    ii = {k: np.asarray(v) for k, v in inputs.items()}
    hm = ii["carry_halted"].astype(bool)
    ids = np.where(hm[:, None], ii["inputs"], ii["carry_inputs"]).astype(np.int32)
    init_h = ii["init_hidden"].astype(np.float32)
    q0 = np.where(
        hm[:, None, None],
        np.broadcast_to(init_h[None, None, :], (B, N, D)),
        ii["carry_hidden"].astype(np.float32),
    ).astype(np.float32)
    emb = np.ascontiguousarray(ii["emb"].astype(np.float32))
    posn = np.ascontiguousarray(ii["pos"].astype(np.float32)[:N])
    lmT = np.ascontiguousarray(ii["lm_w"].astype(np.float32).T)  # [D, V]
    hwT = np.ascontiguousarray(ii["halt_w"].astype(np.float32).T)  # [D, 2]
    hb = ii["halt_b"].astype(np.float32).reshape(1, 2)
    dtv = ii["dt"].astype(np.float32).reshape(1, K)
    wq = ii["W_Q"].astype(np.float32)
    wk = ii["W_K"].astype(np.float32)
    wv = ii["W_V"].astype(np.float32)
    wo = ii["W_O"].astype(np.float32)
    wup = ii["W_up"].astype(np.float32)
    dww = ii["dw_w"].astype(np.float32)[:, :, 0, :]  # [K, INNER, CK]
    dwb = ii["dw_b"].astype(np.float32)
    wdn = ii["W_down"].astype(np.float32)

    in_maps = []
    for c in range(8):
        b, g = c // GRP, c % GRP
        jlo = g * P  # head-col slice (2 heads x 64)
        clo = g * CT * P  # inner slice (384)
        m = {
            "ids": np.ascontiguousarray(ids[b]),
            "q0": np.ascontiguousarray(q0[b]),
            "emb": emb,
            "posn": posn,
            "ins_v": ii["in_s"].astype(np.float32),
            "inb_v": ii["in_b"].astype(np.float32),
            "fins_v": ii["fin_s"].astype(np.float32),
            "finb_v": ii["fin_b"].astype(np.float32),
            "dtv": dtv,
            "wq": np.ascontiguousarray(wq[:, :, jlo : jlo + P]),
            "wk": np.ascontiguousarray(wk[:, :, jlo : jlo + P]),
            "wv": np.ascontiguousarray(wv[:, :, jlo : jlo + P]),
            "wo": np.ascontiguousarray(wo[:, jlo : jlo + P, :]),
            "wup": np.ascontiguousarray(
                np.concatenate(
                    (
                        wup[:, :, clo : clo + CT * P],
                        wup[:, :, INNER + clo : INNER + clo + CT * P],
                    ),
                    axis=-1,
                )
            ),
            "dww": np.ascontiguousarray(dww[:, clo : clo + CT * P, :]),
            "dwb": np.ascontiguousarray(dwb[:, clo : clo + CT * P]),
            "wdn": np.ascontiguousarray(wdn[:, clo : clo + CT * P, :]),
            "n1s": ii["n1_s"].astype(np.float32),
            "n1b": ii["n1_b"].astype(np.float32),
            "n2s": ii["n2_s"].astype(np.float32),
            "n2b": ii["n2_b"].astype(np.float32),
            "lmt": np.ascontiguousarray(lmT[:, g * VC : (g + 1) * VC]),
            "hwt": hwT,
            "hb": hb,
        }
        in_maps.append(m)
    return in_maps


def _run_fast(nc, in_maps):
    """Cached jitted executor (avoids per-call jit retrace). Falls back to
    run_bass_kernel_spmd on any failure."""
    import jax
    import jax.numpy as jnp
    import concourse.mybir as mybir
    from jax.sharding import Mesh, PartitionSpec, NamedSharding
    from jax.experimental.shard_map import shard_map
    from concourse.bass2jax import (
        _bass_exec_p,
        partition_id_tensor,
        install_neuronx_cc_hook,
    )

    if "fast" not in _CACHE:
        install_neuronx_cc_hook()
        partition_name = (
            nc.partition_id_tensor.name if nc.partition_id_tensor else None
        )
        in_names, out_names, out_avals = [], [], []
        for alloc in nc.m.functions[0].allocations:
            if not isinstance(alloc, mybir.MemoryLocationSet):
                continue
            name = alloc.memorylocations[0].name
            if alloc.kind == "ExternalInput":
                if name != partition_name:
                    in_names.append(name)
            elif alloc.kind == "ExternalOutput":
                out_names.append(name)
                out_avals.append(
                    jax.core.ShapedArray(
                        tuple(alloc.tensor_shape), mybir.dt.np(alloc.dtype)
                    )
                )
        n_params = len(in_names)
        all_in = in_names + out_names + ([partition_name] if partition_name else [])

        def _body(*args):
            ins = list(args[:n_params])
            outs = list(args[n_params:])
            pid = [partition_id_tensor()] if partition_name else []
            return tuple(
                _bass_exec_p.bind(
                    *ins,
                    *outs,
                    *pid,
                    out_avals=tuple(out_avals),
                    in_names=tuple(all_in),
                    out_names=tuple(out_names),
                    lowering_input_output_aliases=(),
                    sim_require_finite=True,
                    sim_require_nnan=True,
                    nc=nc,
                )
            )

        devices = jax.devices()[:8]
        mesh = Mesh(np.asarray(devices), ("core",))
        n_outs = len(out_names)
        f = jax.jit(
            shard_map(
                _body,
                mesh=mesh,
                in_specs=(PartitionSpec("core"),) * (n_params + n_outs),
                out_specs=(PartitionSpec("core"),) * n_outs,
                check_rep=False,
            ),
            donate_argnums=tuple(range(n_params, n_params + n_outs)),
            keep_unused=True,
        )
        _CACHE["fast"] = (f, in_names, out_names, out_avals, mesh)
    f, in_names, out_names, out_avals, mesh = _CACHE["fast"]
    import jax

    sh = jax.sharding.NamedSharding(mesh, PartitionSpec("core"))
    concat_in = [
        np.concatenate([np.asarray(in_maps[c][nm]) for c in range(8)], axis=0)
        for nm in in_names
    ]
    dev_in = [jax.device_put(a, sh) for a in concat_in]
    zeros = [
        jax.device_put(np.zeros((av.shape[0] * 8,) + av.shape[1:], av.dtype), sh)
        for av in out_avals
    ]
    outs = f(*dev_in, *zeros)
    jax.block_until_ready(outs)
    res = []
    for c in range(8):
        m = {}
        for i, nm in enumerate(out_names):
            av = out_avals[i]
            m[nm] = np.asarray(outs[i])[c * av.shape[0] : (c + 1) * av.shape[0]]
        res.append(m)
    return res


def kernel(**inputs):
    from concourse.bass_utils import run_bass_kernel_spmd

    nc = _get_nc()
    in_maps = _prep_in_maps(inputs)
    try:
        res = _run_fast(nc, in_maps)
    except Exception:
        res = run_bass_kernel_spmd(nc, in_maps, core_ids=list(range(8))).results
    logits = np.zeros((B, N, V), np.float32)
    for c in range(8):
        b, g = c // GRP, c % GRP
        logits[b, :, g * VC : (g + 1) * VC] = res[c]["logits"]
    q_logits = np.stack([res[0]["qlog"][0], res[GRP]["qlog"][0]])
    return logits, q_logits
